# revision 1
# baseline (speedup 1.0000x reference)
"""Causal multi-head attention block (QKV proj -> causal attention -> out proj)
for Trainium2, sharded over 8 NeuronCores.

Sharding: tensor/data hybrid. Core c handles batch b = c//2 and half the heads
(g = c%2, 8 of 16 heads). Per core:
  - QKV projection for its 8 heads with fp32r matmuls (x^T resident in SBUF)
  - flash-style causal attention in S^T = K @ Q^T layout: exp on ScalarE,
    P^T (fp16) @ V_aug (fp16, ones column appended -> row sums for free)
  - normalize by DVE reciprocal of the fused row sums
  - DMA-xbar transpose of O, fp16 output projection -> partial y [T, C]
Host: y[b] = partial[2b] + partial[2b+1] (+ bias terms, see below).

Biases: b_attn Q/K slices are added on-device (per-partition add fused into
the PSUM->SBUF copies). The V-bias and b_proj contributions are exact row
vectors on the output (rows of softmax sum to 1): y += (b_v @ w_proj + b_proj),
added on host during the unshard.
"""

import math

import numpy as np

import concourse.bass as bass
import concourse.mybir as mybir
import concourse.tile as tile
from concourse import bacc
from concourse.bass_utils import run_bass_kernel_spmd

B, T, C = 4, 2048, 1024
NH, HD = 16, 64
NCORES = 8
HPC = NH // 2          # heads per core = 8
CPC = HPC * HD         # channels per core = 512
P = 128                # partitions
NT = T // P            # 16 t-tiles of 128
NCB = C // P           # 8 contraction blocks
NPAIR = HPC // 2       # 4 head pairs
QW = 512               # q-tile width
NQT = T // QW          # 4 q-tiles

F32 = mybir.dt.float32
F32R = mybir.dt.float32r
F16 = mybir.dt.float16
SCALE = HD ** -0.5


def _r(ap):
    return ap.bitcast(F32R)


def build_kernel(loop_n: int = 1):
    nc = bacc.Bacc("TRN2", target_bir_lowering=False, debug=False)
    xT = nc.dram_tensor("xT", [C, T], F16, kind="ExternalInput").ap()
    wq = nc.dram_tensor("wq", [C, CPC], F16, kind="ExternalInput").ap()
    wk = nc.dram_tensor("wk", [C, CPC], F16, kind="ExternalInput").ap()
    wv = nc.dram_tensor("wv", [C, CPC], F16, kind="ExternalInput").ap()
    wp = nc.dram_tensor("wp", [CPC, C], F32, kind="ExternalInput").ap()
    qb = nc.dram_tensor("qb", [CPC], F32, kind="ExternalInput").ap()
    kb = nc.dram_tensor("kb", [CPC], F32, kind="ExternalInput").ap()
    maskT = nc.dram_tensor("maskT", [P, P], F16, kind="ExternalInput").ap()
    iden = nc.dram_tensor("iden", [P, P], F16, kind="ExternalInput").ap()
    y = nc.dram_tensor("y", [T, C], F32, kind="ExternalOutput").ap()

    with tile.TileContext(nc) as tc:
        if loop_n == 1:
            _body(tc, nc, xT, wq, wk, wv, wp, qb, kb, maskT, iden, y)
        else:
            with tc.For_i(0, loop_n, 1):
                _body(tc, nc, xT, wq, wk, wv, wp, qb, kb, maskT, iden, y)
    nc.compile()
    return nc


def _body(tc, nc, xT, wq, wk, wv, wp, qb, kb, maskT, iden, y):
    from contextlib import ExitStack

    ctx = ExitStack()
    with ctx:
        const = ctx.enter_context(tc.tile_pool(name="const", bufs=1))
        xt_pool = ctx.enter_context(tc.tile_pool(name="xt", bufs=NCB))
        v_pool = ctx.enter_context(tc.tile_pool(name="vp", bufs=NT))
        wqk_pool = ctx.enter_context(tc.tile_pool(name="wqk", bufs=3))
        qtkt_pool = ctx.enter_context(tc.tile_pool(name="qtkt", bufs=3))
        bias_pool = ctx.enter_context(tc.tile_pool(name="biasp", bufs=2))
        pt_pool = ctx.enter_context(tc.tile_pool(name="ptp", bufs=11))
        osb_pool = ctx.enter_context(tc.tile_pool(name="osb", bufs=3))
        ot_pool = ctx.enter_context(tc.tile_pool(name="otp", bufs=NPAIR))
        r_pool = ctx.enter_context(tc.tile_pool(name="rp", bufs=4))
        wp_pool = ctx.enter_context(tc.tile_pool(name="wpp", bufs=NPAIR))
        y_pool = ctx.enter_context(tc.tile_pool(name="yp", bufs=3))
        mm_ps = ctx.enter_context(tc.tile_pool(name="mmps", bufs=3, space="PSUM"))
        s_ps = ctx.enter_context(tc.tile_pool(name="sps", bufs=2, space="PSUM"))
        o_ps = ctx.enter_context(tc.tile_pool(name="ops", bufs=1, space="PSUM"))

        mask_sb = const.tile([P, P], F16)
        nc.sync.dma_start(out=mask_sb, in_=maskT)
        iden_sb = const.tile([P, P], F16)
        nc.sync.dma_start(out=iden_sb, in_=iden)

        # ---- phase V: V for all 8 heads, fp32r matmuls ----
        v_sb = []
        with tc.tile_pool(name="wvp", bufs=NCB) as wv_pool:
            wv_sb = []
            for i in range(NCB):
                t_ = wv_pool.tile([P, CPC], F16, name=f"wv{i}", tag="wv")
                nc.sync.dma_start(out=t_, in_=wv[P * i:P * (i + 1), :])
                wv_sb.append(t_)
            xt_sb = []
            for i in range(NCB):
                t_ = xt_pool.tile([P, T], F16, name=f"xt{i}", tag="xt")
                nc.sync.dma_start(out=t_[:, 0:T // 2],
                                  in_=xT[P * i:P * (i + 1), 0:T // 2])
                xt_sb.append(t_)
            for i in range(NCB):
                nc.sync.dma_start(out=xt_sb[i][:, T // 2:T],
                                  in_=xT[P * i:P * (i + 1), T // 2:T])
            for t in range(NT):
                vps = mm_ps.tile([P, CPC], F32, name=f"vps{t}", tag="mm")
                for i in range(NCB):
                    nc.tensor.matmul(
                        vps, (xt_sb[i][:, P * t:P * (t + 1)]), (wv_sb[i]),
                        start=(i == 0), stop=(i == NCB - 1))
                vt = v_pool.tile([P, HPC, HD + 1], F16, name=f"v{t}", tag="v")
                nc.vector.memset(vt[:, :, HD], 1.0)
                nc.vector.tensor_copy(
                    out=vt[:, :, 0:HD],
                    in_=vps.rearrange("p (h d) -> p h d", h=HPC))
                v_sb.append(vt)

        wp16 = []
        for p in range(NPAIR):
            wps = wp_pool.tile([P, C], F32, name=f"wps{p}", tag="wps", bufs=1)
            nc.sync.dma_start(out=wps, in_=wp[P * p:P * (p + 1), :])
            w16 = wp_pool.tile([P, C], F16, name=f"wp16{p}", tag="wp16")
            nc.vector.tensor_copy(out=w16, in_=wps)
            wp16.append(w16)
        # ---- per head-pair: QT/KT projection + attention ----
        ot_sb = []
        for p in range(NPAIR):
            wq_sb = wqk_pool.tile([P, NCB, P], F16, name=f"wq{p}", tag="wq")
            wk_sb = wqk_pool.tile([P, NCB, P], F16, name=f"wk{p}", tag="wk")
            nc.sync.dma_start(
                out=wq_sb,
                in_=wq.rearrange("(i p) d -> p i d", p=P)[:, :, P * p:P * (p + 1)])
            nc.sync.dma_start(
                out=wk_sb,
                in_=wk.rearrange("(i p) d -> p i d", p=P)[:, :, P * p:P * (p + 1)])
            qb_sb = bias_pool.tile([P, 1], F32, name=f"qb{p}", tag="qb")
            kb_sb = bias_pool.tile([P, 1], F32, name=f"kb{p}", tag="kb")
            nc.sync.dma_start(out=qb_sb, in_=qb[P * p:P * (p + 1)].unsqueeze(1))
            nc.sync.dma_start(out=kb_sb, in_=kb[P * p:P * (p + 1)].unsqueeze(1))

            qt_sb = qtkt_pool.tile([P, T], F32R, name=f"qt{p}", tag="qt")
            kt_sb = qtkt_pool.tile([P, T], F32R, name=f"kt{p}", tag="kt")
            for tq in range(NQT):
                qps = mm_ps.tile([P, QW], F32, name=f"qps{p}{tq}", tag="mm")
                for i in range(NCB):
                    nc.tensor.matmul(
                        qps, (wq_sb[:, i, :]),
                        (xt_sb[i][:, QW * tq:QW * (tq + 1)]),
                        start=(i == 0), stop=(i == NCB - 1))
                nc.vector.tensor_scalar_add(
                    qt_sb[:, QW * tq:QW * (tq + 1)], qps, qb_sb)
                kps = mm_ps.tile([P, QW], F32, name=f"kps{p}{tq}", tag="mm")
                for i in range(NCB):
                    nc.tensor.matmul(
                        kps, (wk_sb[:, i, :]),
                        (xt_sb[i][:, QW * tq:QW * (tq + 1)]),
                        start=(i == 0), stop=(i == NCB - 1))
                nc.vector.tensor_scalar_add(
                    kt_sb[:, QW * tq:QW * (tq + 1)], kps, kb_sb)

            o_sb = osb_pool.tile([P, T], F16, name=f"o{p}", tag="o")
            for hl in range(2):
                hh = 2 * p + hl
                dlo, dhi = HD * hl, HD * (hl + 1)
                for qt_i in range(NQT):
                    ops_ = o_ps.tile([P, 4 * (HD + 1)], F32,
                                     name=f"o{p}{hl}{qt_i}", tag="o")
                    nkt = 4 * qt_i + 4
                    pts = []
                    for k0 in range(0, nkt, 2):
                        # restrict S / exp to the valid (causal) q-columns of
                        # each half; keep matmul moving >= 256 for fp32r rate
                        smin = [min(max(0, (k0 + u) - 4 * qt_i), 2)
                                for u in range(2)]
                        off = [P * s for s in smin]
                        sps = s_ps.tile([P, 2 * QW], F32,
                                        name=f"s{p}{hl}{qt_i}{k0}", tag="s")
                        for u in range(2):
                            k = k0 + u
                            nc.tensor.matmul(
                                sps[:, QW * u + off[u]:QW * (u + 1)],
                                (kt_sb[dlo:dhi, P * k:P * (k + 1)]),
                                (qt_sb[dlo:dhi,
                                       QW * qt_i + off[u]:QW * (qt_i + 1)]),
                                start=True, stop=True)
                        pt = pt_pool.tile([P, 2 * QW], F16,
                                          name=f"pt{p}{hl}{qt_i}{k0}", tag="pt")
                        if off == [0, 0]:
                            nc.scalar.activation(
                                out=pt, in_=sps,
                                func=mybir.ActivationFunctionType.Exp,
                                scale=SCALE)
                        elif off[0] == off[1]:
                            view = lambda ap: ap.rearrange(
                                "p (u c) -> p u c", u=2)[:, :, off[0]:QW]
                            nc.scalar.activation(
                                out=view(pt), in_=view(sps),
                                func=mybir.ActivationFunctionType.Exp,
                                scale=SCALE)
                        else:
                            for u in range(2):
                                nc.scalar.activation(
                                    out=pt[:, QW * u + off[u]:QW * (u + 1)],
                                    in_=sps[:, QW * u + off[u]:QW * (u + 1)],
                                    func=mybir.ActivationFunctionType.Exp,
                                    scale=SCALE)
                        for u in range(2):
                            k = k0 + u
                            for s in range(4):
                                gs = 4 * qt_i + s
                                if gs == k:
                                    sl = pt[:, QW * u + P * s:QW * u + P * (s + 1)]
                                    nc.vector.tensor_mul(sl, sl, mask_sb)
                        pts.append(pt)
                    # PV: one open accumulation group per PSUM bank at a time
                    # (start=True marks the whole 2KB zero region pending).
                    for s in range(4):
                        gs = 4 * qt_i + s
                        for k in range(gs + 1):
                            nc.tensor.matmul(
                                ops_[:, (HD + 1) * s:(HD + 1) * (s + 1)],
                                pts[k // 2][:, QW * (k % 2) + P * s:
                                            QW * (k % 2) + P * (s + 1)],
                                v_sb[k][:, hh, :],
                                start=(k == 0), stop=(k == gs))
                    r_ = r_pool.tile([P, 4], F32, name=f"r{p}{hl}{qt_i}", tag="r")
                    nc.vector.reciprocal(
                        r_, ops_.rearrange("p (s c) -> p s c", c=HD + 1)[:, :, HD])
                    out_ap = o_sb[:, QW * qt_i:QW * (qt_i + 1)].rearrange(
                        "p (s h d) -> p s h d", s=4, h=2)[:, :, hl, :]
                    nc.vector.tensor_mul(
                        out_ap,
                        ops_.rearrange("p (s c) -> p s c", c=HD + 1)[:, :, 0:HD],
                        r_.unsqueeze(2).broadcast_to((P, 4, HD)))
            ot = ot_pool.tile([P, T], F16, name=f"ot{p}", tag="ot")
            for tq in range(NQT):
                tp = s_ps.tile([P, QW], F16, name=f"tp{p}{tq}", tag="s")
                for j in range(4):
                    gs = 4 * tq + j
                    nc.tensor.transpose(
                        tp[:, P * j:P * (j + 1)],
                        o_sb[:, P * gs:P * (gs + 1)], iden_sb)
                nc.vector.tensor_copy(
                    out=ot[:, QW * tq:QW * (tq + 1)], in_=tp)
            ot_sb.append(ot)

        # ---- output projection (fp16) ----
        for t in range(NT):
            ysb = y_pool.tile([P, C], F32, name=f"y{t}", tag="y")
            for n2 in range(2):
                yps = mm_ps.tile([P, QW], F32, name=f"yps{t}{n2}", tag="mm")
                for p in range(NPAIR):
                    nc.tensor.matmul(
                        yps, ot_sb[p][:, P * t:P * (t + 1)],
                        wp16[p][:, QW * n2:QW * (n2 + 1)],
                        start=(p == 0), stop=(p == NPAIR - 1))
                if n2 == 0:
                    nc.vector.tensor_copy(out=ysb[:, QW * n2:QW * (n2 + 1)], in_=yps)
                else:
                    nc.scalar.copy(out=ysb[:, QW * n2:QW * (n2 + 1)], in_=yps)
            nc.sync.dma_start(out=y[P * t:P * (t + 1), :], in_=ysb)


def _prep_inputs(x, w_attn, b_attn, w_proj):
    """Per-core input maps."""
    in_maps = []
    for c in range(NCORES):
        b = c // 2
        g = c % 2
        qs = slice(CPC * g, CPC * (g + 1))
        ks = slice(C + CPC * g, C + CPC * (g + 1))
        vs = slice(2 * C + CPC * g, 2 * C + CPC * (g + 1))
        in_maps.append({
            "xT": np.ascontiguousarray(x[b].T.astype(np.float16)),
            "wq": np.ascontiguousarray(w_attn[:, qs].astype(np.float16)),
            "wk": np.ascontiguousarray(w_attn[:, ks].astype(np.float16)),
            "wv": np.ascontiguousarray(w_attn[:, vs].astype(np.float16)),
            "wp": np.ascontiguousarray(w_proj[CPC * g:CPC * (g + 1), :]),
            "qb": np.ascontiguousarray(b_attn[qs]),
            "kb": np.ascontiguousarray(b_attn[ks]),
            "maskT": np.triu(np.ones((P, P), dtype=np.float16)),
            "iden": np.eye(P, dtype=np.float16),
        })
    return in_maps


_CACHED_NC = None


def kernel(x, w_attn, b_attn, w_proj, b_proj):
    global _CACHED_NC
    x = np.asarray(x, dtype=np.float32)
    w_attn = np.asarray(w_attn, dtype=np.float32)
    b_attn = np.asarray(b_attn, dtype=np.float32)
    w_proj = np.asarray(w_proj, dtype=np.float32)
    b_proj = np.asarray(b_proj, dtype=np.float32)

    if _CACHED_NC is None:
        _CACHED_NC = build_kernel(loop_n=1)
    nc = _CACHED_NC
    in_maps = _prep_inputs(x, w_attn, b_attn, w_proj)
    res = run_bass_kernel_spmd(nc, in_maps, core_ids=list(range(NCORES)),
                               trace=False)
    out = np.empty((B, T, C), dtype=np.float32)
    # exact row-vector bias contribution: rows of softmax sum to 1
    for b in range(B):
        acc = res.results[2 * b]["y"] + res.results[2 * b + 1]["y"]
        out[b] = acc
    bias_row = b_attn[2 * C:3 * C] @ w_proj + b_proj
    out += bias_row[None, None, :]
    return out



# revision 24
# speedup vs baseline: 1.1374x; 1.1374x over previous
"""Causal multi-head attention block (QKV proj -> causal attention -> out proj)
for Trainium2, sharded over 8 NeuronCores.

Sharding: tensor/data hybrid. Core c handles batch b = c//2 and half the heads
(g = c%2, 8 of 16 heads). Per core:
  - QKV projection with error-compensated fp8e4m3 DoubleRow matmuls
    (x = x_hi + x_lo, w = w_hi + w_lo; compute hi*hi + hi*lo + lo*hi,
    each a 256-deep DoubleRow matmul). Weights scaled x64 on host so fp8
    stays in the normal range; rescaled on the PSUM evacuation.
  - flash-style causal attention in S^T = K @ Q^T layout (fp16): exp on
    ScalarE, P^T (fp16) @ V_aug (fp16, scaled ones column appended ->
    0.25/rowsum for free from the DVE reciprocal)
  - DVE normalize, PE transpose of O, fp8-compensated output projection
    -> partial y [T, C]
Host: y[b] = partial[2b] + partial[2b+1] (+ bias terms, see below).

Biases: b_attn Q/K slices are added on-device (fused into the PSUM->SBUF
copies). The V-bias and b_proj contributions are exact row vectors on the
output (rows of softmax sum to 1): y += (b_v @ w_proj + b_proj), added on
host during the unshard.

Scaling ledger (all powers of 2, exact):
  wq8/wk8/wv8 = 64*w (hi+lo fp8 pair)   -> q/k/v psum = 64*true
  qt/kt = psum/16 + 4*qb = 4*true (fp16)
  S psum = 16*S_true; exp scale = HD^-0.5/16
  vt = 64*V (fp16), ones col = 4.0  -> recip gives 0.25/rowsum
  o_sb = (64*O')*(0.25/r) = 16*O (fp16)
  ot8 = fp8 pair of 16*O ; wp8 = 64*w_proj (hi+lo) -> y psum = 1024*y
  y = psum/1024 (fp32)
"""

import math

import numpy as np
import ml_dtypes

import concourse.bass as bass
import concourse.mybir as mybir
import concourse.tile as tile
from concourse import bacc
from concourse.bass_utils import run_bass_kernel_spmd

B, T, C = 4, 2048, 1024
NH, HD = 16, 64
NCORES = 8
HPC = NH // 2          # heads per core = 8
CPC = HPC * HD         # channels per core = 512
P = 128                # partitions
NT = T // P            # 16 t-tiles of 128
NU = C // 256          # 4 DoubleRow contraction units of 256
NPAIR = HPC // 2       # 4 head pairs
QW = 512               # q-tile width
NQT = T // QW          # 4 q-tiles

F32 = mybir.dt.float32
F16 = mybir.dt.float16
F8 = mybir.dt.float8e4
DR = mybir.MatmulPerfMode.DoubleRow
NPF8 = ml_dtypes.float8_e4m3
SCALE = HD ** -0.5


def build_kernel(loop_n: int = 1):
    nc = bacc.Bacc("TRN2", target_bir_lowering=False, debug=False)
    x8h = nc.dram_tensor("x8h", [NU, P, 2, T], F8, kind="ExternalInput").ap()
    x8l = nc.dram_tensor("x8l", [NU, P, 2, T], F8, kind="ExternalInput").ap()
    wq8h = nc.dram_tensor("wq8h", [NU, P, 2, CPC], F8, kind="ExternalInput").ap()
    wq8l = nc.dram_tensor("wq8l", [NU, P, 2, CPC], F8, kind="ExternalInput").ap()
    wk8h = nc.dram_tensor("wk8h", [NU, P, 2, CPC], F8, kind="ExternalInput").ap()
    wk8l = nc.dram_tensor("wk8l", [NU, P, 2, CPC], F8, kind="ExternalInput").ap()
    wv8h = nc.dram_tensor("wv8h", [NU, P, 2, CPC], F8, kind="ExternalInput").ap()
    wv8l = nc.dram_tensor("wv8l", [NU, P, 2, CPC], F8, kind="ExternalInput").ap()
    wp8h = nc.dram_tensor("wp8h", [2, P, 2, C], F8, kind="ExternalInput").ap()
    wp8l = nc.dram_tensor("wp8l", [2, P, 2, C], F8, kind="ExternalInput").ap()
    qb4 = nc.dram_tensor("qb4", [CPC], F32, kind="ExternalInput").ap()
    kb4 = nc.dram_tensor("kb4", [CPC], F32, kind="ExternalInput").ap()
    maskT = nc.dram_tensor("maskT", [P, P], F16, kind="ExternalInput").ap()
    y = nc.dram_tensor("y", [T, C], F16, kind="ExternalOutput").ap()

    args = (x8h, x8l, wq8h, wq8l, wk8h, wk8l, wv8h, wv8l, wp8h, wp8l,
            qb4, kb4, maskT, y)
    with tile.TileContext(nc) as tc:
        if loop_n == 1:
            _body(tc, nc, *args)
        else:
            with tc.For_i(0, loop_n, 1):
                _body(tc, nc, *args)
    nc.compile()
    return nc


def _body(tc, nc, x8h, x8l, wq8h, wq8l, wk8h, wk8l, wv8h, wv8l,
          wp8h, wp8l, qb4, kb4, maskT, y):
    from contextlib import ExitStack

    ctx = ExitStack()
    with ctx:
        const = ctx.enter_context(tc.tile_pool(name="const", bufs=1))
        x_pool = ctx.enter_context(tc.tile_pool(name="xp", bufs=2 * NU))
        w8_pool = ctx.enter_context(tc.tile_pool(name="w8p", bufs=1))
        v_pool = ctx.enter_context(tc.tile_pool(name="vp", bufs=NT))
        qtkt_pool = ctx.enter_context(tc.tile_pool(name="qtkt", bufs=3))
        bias_pool = ctx.enter_context(tc.tile_pool(name="biasp", bufs=2))
        pt_pool = ctx.enter_context(tc.tile_pool(name="ptp", bufs=17))
        osb_pool = ctx.enter_context(tc.tile_pool(name="osb", bufs=3))
        ot_pool = ctx.enter_context(tc.tile_pool(name="otp", bufs=4))
        r_pool = ctx.enter_context(tc.tile_pool(name="rp", bufs=4))
        y_pool = ctx.enter_context(tc.tile_pool(name="yp", bufs=3))
        mm_ps = ctx.enter_context(tc.tile_pool(name="mmps", bufs=2, space="PSUM"))
        s_ps = ctx.enter_context(tc.tile_pool(name="sps", bufs=2, space="PSUM"))
        o_ps = ctx.enter_context(tc.tile_pool(name="ops", bufs=2, space="PSUM"))

        # ---- weight + x loads (fp8 hi/lo pairs) ----
        # order matters for the startup critical path: wv + x first (phase V
        # needs them, unit-interleaved so the first V matmuls start early),
        # wq/wk next, wp last. Two DGE queues (SP + ACT) in parallel.
        wv_sb, xh_sb, xl_sb = [], [], []
        for tag, dr_ in (("wvh", wv8h), ("wvl", wv8l)):
            t_ = w8_pool.tile([P, NU, 2, CPC], F8, name=tag, tag=tag)
            wv_sb.append(t_)
        for i in range(NU):
            xh_sb.append(x_pool.tile([P, 2, T], F8, name=f"xh{i}", tag="x"))
            xl_sb.append(x_pool.tile([P, 2, T], F8, name=f"xl{i}", tag="x"))
        for i in range(NU):
            nc.sync.dma_start(out=wv_sb[0][:, i], in_=wv8h[i])
            nc.scalar.dma_start(out=wv_sb[1][:, i], in_=wv8l[i])
            sl = slice(0, T // 4)
            nc.sync.dma_start(out=xh_sb[i][:, :, sl], in_=x8h[i][:, :, sl])
            nc.scalar.dma_start(out=xl_sb[i][:, :, sl], in_=x8l[i][:, :, sl])
        mask_sb = const.tile([P, P], F16, tag="mask")
        nc.sync.dma_start(out=mask_sb, in_=maskT)
        for half in range(2):
            sl = slice(T // 4 + T * 3 // 8 * half,
                       T // 4 + T * 3 // 8 * (half + 1))
            for i in range(NU):
                nc.sync.dma_start(out=xh_sb[i][:, :, sl], in_=x8h[i][:, :, sl])
                nc.scalar.dma_start(out=xl_sb[i][:, :, sl], in_=x8l[i][:, :, sl])
        wq_sb, wk_sb = [], []
        for nm, drh, drl, lst in (("wq", wq8h, wq8l, wq_sb),
                                  ("wk", wk8h, wk8l, wk_sb)):
            for tag, dr_ in ((f"{nm}h", drh), (f"{nm}l", drl)):
                t_ = w8_pool.tile([P, NU, 2, CPC], F8, name=tag, tag=tag)
                eng = nc.sync if nm == "wq" else nc.scalar
                eng.dma_start(out=t_, in_=dr_.rearrange("i p j m -> p i j m"))
                lst.append(t_)
        qb_all = bias_pool.tile([P, NPAIR], F32, tag="qb", bufs=1)
        kb_all = bias_pool.tile([P, NPAIR], F32, tag="kb", bufs=1)
        nc.sync.dma_start(out=qb_all, in_=qb4.rearrange("(a p) -> p a", p=P))
        nc.sync.dma_start(out=kb_all, in_=kb4.rearrange("(a p) -> p a", p=P))

        def comp_mms(ps, lhs_hl, rhs_hl, lslice, rslice):
            """hi*hi + lo*hi + hi*lo DoubleRow accumulation over NU units."""
            terms = [(0, 0), (1, 0), (0, 1)]
            n = NU * len(terms) - 1
            cnt = 0
            for i in range(NU):
                for (a, b_) in terms:
                    nc.tensor.matmul(
                        ps, lslice(lhs_hl[a], i), rslice(rhs_hl[b_], i),
                        start=(cnt == 0), stop=(cnt == n), perf_mode=DR)
                    cnt += 1

        # ---- phase V: V for all 8 heads ----
        v_sb = []
        for t in range(NT):
            vps = mm_ps.tile([P, CPC], F32, name=f"vps{t}", tag="mm")
            comp_mms(
                vps, (xh_sb, xl_sb), (wv_sb[0], wv_sb[1]),
                lambda xs, i: xs[i][:, :, P * t:P * (t + 1)],
                lambda w, i: w[:, i])
            vt = v_pool.tile([P, HPC, HD + 1], F16, name=f"v{t}", tag="v")
            nc.vector.memset(vt[:, :, HD], 4.0)
            nc.vector.tensor_copy(
                out=vt[:, :, 0:HD],
                in_=vps.rearrange("p (h d) -> p h d", h=HPC))
            v_sb.append(vt)

        # ---- fp8 wp tiles for the output projection ----
        wp_sb = []
        for tag, dr_ in (("wph", wp8h), ("wpl", wp8l)):
            t_ = w8_pool.tile([P, 2, 2, C], F8, name=tag, tag=tag)
            for g in range(2):
                nc.sync.dma_start(out=t_[:, g], in_=dr_[g])
            wp_sb.append(t_)
        ot8h, ot8l, ot16 = [], [], []
        for g in range(2):
            ot8h.append(ot_pool.tile([P, 2, T], F8, name=f"oth{g}", tag="ot8"))
            ot8l.append(ot_pool.tile([P, 2, T], F8, name=f"otl{g}", tag="ot8"))
            ot16.append(ot_pool.tile([P, 2, T], F16, name=f"ot16{g}",
                                     tag="ot16", bufs=2))

        # ---- per head-pair: O^T via DMA-XBAR transpose + GpSimd fp8 split --
        def transpose_tq(p, o_sb, tq):
            """Blocked transpose of one o_sb q-window into ot8 hi/lo fp8."""
            g, j = p // 2, p % 2
            win = slice(QW * tq, QW * (tq + 1))
            out3 = ot16[g][:, j, win].rearrange("p (b c) -> p b c", b=4)
            eng = nc.sync if tq % 2 == 0 else nc.scalar
            eng.dma_start(out=out3, in_=o_sb[:, win], transpose=True)
            nc.gpsimd.tensor_copy(out=ot8h[g][:, j, win], in_=ot16[g][:, j, win])
            nc.gpsimd.tensor_sub(ot8l[g][:, j, win], ot16[g][:, j, win],
                                 ot8h[g][:, j, win])

        def emit_y(tq):
            """Output-projection matmuls for the 4 t-tiles of one tq window."""
            for t in range(4 * tq, 4 * tq + 4):
                ysb = y_pool.tile([P, C], F16, name=f"y{t}", tag="y")
                for n2 in range(2):
                    yps = mm_ps.tile([P, QW], F32, name=f"yps{t}{n2}", tag="mm")
                    cnt = 0
                    for g in range(2):
                        for (osrc, wsrc) in ((ot8h[g], wp_sb[0]),
                                             (ot8h[g], wp_sb[1]),
                                             (ot8l[g], wp_sb[0])):
                            nc.tensor.matmul(
                                yps, osrc[:, :, P * t:P * (t + 1)],
                                wsrc[:, g, :, QW * n2:QW * (n2 + 1)],
                                start=(cnt == 0), stop=(cnt == 5), perf_mode=DR)
                            cnt += 1
                    nc.vector.tensor_scalar(
                        out=ysb[:, QW * n2:QW * (n2 + 1)], in0=yps,
                        scalar1=1.0 / 1024.0, scalar2=None,
                        op0=mybir.AluOpType.mult)
                nc.sync.dma_start(out=y[P * t:P * (t + 1), :], in_=ysb)

        def qk_proj(p, tq, qt_sb, kt_sb):
            """Q/K projection matmuls + PSUM evac for one t-window."""
            for (wsb, bsb, dst) in ((wq_sb, qb_all[:, p:p + 1], qt_sb),
                                    (wk_sb, kb_all[:, p:p + 1], kt_sb)):
                ps = mm_ps.tile([P, QW], F32, name=f"qk{p}{tq}", tag="mm")
                comp_mms(
                    ps, (wsb[0], wsb[1]), (xh_sb, xl_sb),
                    lambda w, i: w[:, i, :, P * p:P * (p + 1)],
                    lambda xs, i: xs[i][:, :, QW * tq:QW * (tq + 1)])
                nc.vector.tensor_scalar(
                    out=dst[:, QW * tq:QW * (tq + 1)], in0=ps,
                    scalar1=1.0 / 16.0, scalar2=bsb,
                    op0=mybir.AluOpType.mult, op1=mybir.AluOpType.add)

        qtkt = {}

        def get_qtkt(p):
            if p not in qtkt:
                qtkt[p] = (qtkt_pool.tile([P, T], F16, name=f"qt{p}", tag="qt"),
                           qtkt_pool.tile([P, T], F16, name=f"kt{p}", tag="kt"))
            return qtkt[p]

        for tq in range(NQT):
            qk_proj(0, tq, *get_qtkt(0))

        for p in range(NPAIR):
            qt_sb, kt_sb = get_qtkt(p)
            o_sb = osb_pool.tile([P, T], F16, name=f"o{p}", tag="o")

            def s_unit(qt_i, hl):
                """S^T matmuls + exp + causal mask for one (q-tile, head)."""
                dlo, dhi = HD * hl, HD * (hl + 1)
                nkt = 4 * qt_i + 4
                pts = []
                for k0 in range(0, nkt, 2):
                    smin = [min(max(0, (k0 + u) - 4 * qt_i), 2)
                            for u in range(2)]
                    off = [P * s for s in smin]
                    sps = s_ps.tile([P, 2 * QW], F32,
                                    name=f"s{p}{hl}{qt_i}{k0}", tag="s")
                    for u in range(2):
                        k = k0 + u
                        nc.tensor.matmul(
                            sps[:, QW * u + off[u]:QW * (u + 1)],
                            kt_sb[dlo:dhi, P * k:P * (k + 1)],
                            qt_sb[dlo:dhi,
                                  QW * qt_i + off[u]:QW * (qt_i + 1)],
                            start=True, stop=True)
                    pt = pt_pool.tile([P, 2 * QW], F16,
                                      name=f"pt{p}{hl}{qt_i}{k0}", tag="pt")
                    # exp in a single instruction per chunk: for unequal
                    # offsets, exp the union region (extra columns read stale
                    # psum; their pt slots are never consumed downstream)
                    eoff = min(off)
                    if eoff == 0:
                        nc.scalar.activation(
                            out=pt, in_=sps,
                            func=mybir.ActivationFunctionType.Exp,
                            scale=SCALE / 16.0)
                    else:
                        view = lambda ap: ap.rearrange(
                            "p (u c) -> p u c", u=2)[:, :, eoff:QW]
                        nc.scalar.activation(
                            out=view(pt), in_=view(sps),
                            func=mybir.ActivationFunctionType.Exp,
                            scale=SCALE / 16.0)
                    for u in range(2):
                        k = k0 + u
                        for s in range(4):
                            gs = 4 * qt_i + s
                            if gs == k:
                                sl = pt[:, QW * u + P * s:QW * u + P * (s + 1)]
                                nc.vector.tensor_mul(sl, sl, mask_sb)
                    pts.append(pt)
                return pts

            def pv_unit(qt_i, hl, pts):
                """P^T @ V_aug + normalize into o_sb for one unit."""
                hh = 2 * p + hl
                ops_ = o_ps.tile([P, 4 * (HD + 1)], F32,
                                 name=f"o{p}{hl}{qt_i}", tag="o")
                for s in range(4):
                    gs = 4 * qt_i + s
                    for k in range(gs + 1):
                        nc.tensor.matmul(
                            ops_[:, (HD + 1) * s:(HD + 1) * (s + 1)],
                            pts[k // 2][:, QW * (k % 2) + P * s:
                                        QW * (k % 2) + P * (s + 1)],
                            v_sb[k][:, hh, :],
                            start=(k == 0), stop=(k == gs))
                r_ = r_pool.tile([P, 4], F32, name=f"r{p}{hl}{qt_i}", tag="r")
                nc.vector.reciprocal(
                    r_, ops_.rearrange("p (s c) -> p s c", c=HD + 1)[:, :, HD])
                out_ap = o_sb[:, QW * qt_i:QW * (qt_i + 1)].rearrange(
                    "p (s h d) -> p s h d", s=4, h=2)[:, :, hl, :]
                nc.vector.tensor_mul(
                    out_ap,
                    ops_.rearrange("p (s c) -> p s c", c=HD + 1)[:, :, 0:HD],
                    r_.unsqueeze(2).broadcast_to((P, 4, HD)))

            # software pipeline: PV runs one unit behind S/exp. After each
            # q-window (hl == 1) completes: emit the NEXT pair's projection
            # matmuls for that window (PE filler for this ScalarE-bound
            # phase), then this window's transpose chain (DMA + GpSimd, no
            # PE). On the last pair the filler is the output projection,
            # lagged one window behind its transpose.
            last = p == NPAIR - 1
            y_ready = []

            def drain(pend):
                qt_i, hl, pts = pend
                pv_unit(qt_i, hl, pts)
                if hl == 1:
                    if not last:
                        qk_proj(p + 1, qt_i, *get_qtkt(p + 1))
                    transpose_tq(p, o_sb, qt_i)
                    if last:
                        if y_ready:
                            emit_y(y_ready.pop(0))
                        y_ready.append(qt_i)

            units = [(qt_i, hl) for qt_i in range(NQT) for hl in range(2)]
            pend = None
            for (qt_i, hl) in units:
                pts = s_unit(qt_i, hl)
                if pend is not None:
                    drain(pend)
                pend = (qt_i, hl, pts)
            drain(pend)
            for tq in y_ready:
                emit_y(tq)


def _comp8(a):
    hi = a.astype(NPF8)
    lo = (a - hi.astype(np.float32)).astype(NPF8)
    return hi, lo


def _prep_inputs(x, w_attn, b_attn, w_proj):
    """Per-core input maps."""
    in_maps = []
    # contraction layout [unit, p, j, ...]: c = 256*unit + 128*j + p
    def units(a, n_u):
        # a: [n_u*256, M] -> [n_u, 128, 2, M]
        return np.ascontiguousarray(
            a.reshape(n_u, 2, P, -1).transpose(0, 2, 1, 3))

    wq_h = {}
    for g in range(2):
        qs = slice(CPC * g, CPC * (g + 1))
        ks = slice(C + CPC * g, C + CPC * (g + 1))
        vs = slice(2 * C + CPC * g, 2 * C + CPC * (g + 1))
        wqh, wql = _comp8(64.0 * w_attn[:, qs])
        wkh, wkl = _comp8(64.0 * w_attn[:, ks])
        wvh, wvl = _comp8(64.0 * w_attn[:, vs])
        wph, wpl = _comp8(64.0 * w_proj[CPC * g:CPC * (g + 1), :])
        wq_h[g] = dict(
            wq8h=units(wqh, NU), wq8l=units(wql, NU),
            wk8h=units(wkh, NU), wk8l=units(wkl, NU),
            wv8h=units(wvh, NU), wv8l=units(wvl, NU),
            wp8h=units(wph, 2), wp8l=units(wpl, 2),
            qb4=np.ascontiguousarray(4.0 * b_attn[qs]),
            kb4=np.ascontiguousarray(4.0 * b_attn[ks]),
        )
    for c in range(NCORES):
        b = c // 2
        g = c % 2
        xT = np.ascontiguousarray(x[b].T)          # [C, T] fp32
        xh, xl = _comp8(xT)
        in_maps.append({
            "x8h": units(xh, NU),
            "x8l": units(xl, NU),
            **wq_h[g],
            "maskT": np.triu(np.ones((P, P), dtype=np.float16)),
        })
    return in_maps


_CACHED_NC = None


def kernel(x, w_attn, b_attn, w_proj, b_proj):
    global _CACHED_NC
    x = np.asarray(x, dtype=np.float32)
    w_attn = np.asarray(w_attn, dtype=np.float32)
    b_attn = np.asarray(b_attn, dtype=np.float32)
    w_proj = np.asarray(w_proj, dtype=np.float32)
    b_proj = np.asarray(b_proj, dtype=np.float32)

    if _CACHED_NC is None:
        _CACHED_NC = build_kernel(loop_n=1)
    nc = _CACHED_NC
    in_maps = _prep_inputs(x, w_attn, b_attn, w_proj)
    res = run_bass_kernel_spmd(nc, in_maps, core_ids=list(range(NCORES)),
                               trace=False)
    out = np.empty((B, T, C), dtype=np.float32)
    # exact row-vector bias contribution: rows of softmax sum to 1
    for b in range(B):
        out[b] = (res.results[2 * b]["y"].astype(np.float32)
                  + res.results[2 * b + 1]["y"].astype(np.float32))
    bias_row = b_attn[2 * C:3 * C] @ w_proj + b_proj
    out += bias_row[None, None, :]
    return out


# revision 44
# speedup vs baseline: 1.1468x; 1.0083x over previous
"""Causal multi-head attention block (QKV proj -> causal attention -> out proj)
for Trainium2, sharded over 8 NeuronCores.

Sharding: tensor/data hybrid. Core c handles batch b = c//2 and half the heads
(g = c%2, 8 of 16 heads). Per core:
  - QKV projection with error-compensated fp8e4m3 DoubleRow matmuls
    (x = x_hi + x_lo, w = w_hi + w_lo; compute hi*hi + hi*lo + lo*hi,
    each a 256-deep DoubleRow matmul). Weights scaled x64 on host so fp8
    stays in the normal range; rescaled on the PSUM evacuation.
  - flash-style causal attention in S^T = K @ Q^T layout (fp16): exp on
    ScalarE, P^T (fp16) @ V_aug (fp16, scaled ones column appended ->
    0.25/rowsum for free from the DVE reciprocal)
  - DVE normalize, PE transpose of O, fp8-compensated output projection
    -> partial y [T, C]
Host: y[b] = partial[2b] + partial[2b+1] (+ bias terms, see below).

Biases: b_attn Q/K slices are added on-device (fused into the PSUM->SBUF
copies). The V-bias and b_proj contributions are exact row vectors on the
output (rows of softmax sum to 1): y += (b_v @ w_proj + b_proj), added on
host during the unshard.

Scaling ledger (all powers of 2, exact):
  wq8/wk8/wv8 = 64*w (hi+lo fp8 pair)   -> q/k/v psum = 64*true
  qt/kt = psum/16 + 4*qb = 4*true (fp16)
  S psum = 16*S_true; exp scale = HD^-0.5/16
  vt = 64*V (fp16), ones col = 4.0  -> recip gives 0.25/rowsum
  o_sb = (64*O')*(0.25/r) = 16*O (fp16)
  ot8 = fp8 pair of 16*O ; wp8 = 64*w_proj (hi+lo) -> y psum = 1024*y
  y = psum/1024 (fp32)
"""

import math

import numpy as np
import ml_dtypes

import concourse.bass as bass
import concourse.mybir as mybir
import concourse.tile as tile
from concourse import bacc
from concourse.bass_utils import run_bass_kernel_spmd

B, T, C = 4, 2048, 1024
NH, HD = 16, 64
NCORES = 8
HPC = NH // 2          # heads per core = 8
CPC = HPC * HD         # channels per core = 512
P = 128                # partitions
NT = T // P            # 16 t-tiles of 128
NU = C // 256          # 4 DoubleRow contraction units of 256
NPAIR = HPC // 2       # 4 head pairs
QW = 512               # q-tile width
NQT = T // QW          # 4 q-tiles

F32 = mybir.dt.float32
F16 = mybir.dt.float16
F8 = mybir.dt.float8e4
DR = mybir.MatmulPerfMode.DoubleRow
NPF8 = ml_dtypes.float8_e4m3
SCALE = HD ** -0.5


def build_kernel(loop_n: int = 1):
    nc = bacc.Bacc("TRN2", target_bir_lowering=False, debug=False)
    x8h = nc.dram_tensor("x8h", [NU, P, 2, T], F8, kind="ExternalInput").ap()
    x8l = nc.dram_tensor("x8l", [NU, P, 2, T], F8, kind="ExternalInput").ap()
    wq8h = nc.dram_tensor("wq8h", [NU, P, 2, CPC], F8, kind="ExternalInput").ap()
    wq8l = nc.dram_tensor("wq8l", [NU, P, 2, CPC], F8, kind="ExternalInput").ap()
    wk8h = nc.dram_tensor("wk8h", [NU, P, 2, CPC], F8, kind="ExternalInput").ap()
    wk8l = nc.dram_tensor("wk8l", [NU, P, 2, CPC], F8, kind="ExternalInput").ap()
    wv8h = nc.dram_tensor("wv8h", [NU, P, 2, CPC], F8, kind="ExternalInput").ap()
    wv8l = nc.dram_tensor("wv8l", [NU, P, 2, CPC], F8, kind="ExternalInput").ap()
    wp8h = nc.dram_tensor("wp8h", [2, P, 2, C], F8, kind="ExternalInput").ap()
    wp8l = nc.dram_tensor("wp8l", [2, P, 2, C], F8, kind="ExternalInput").ap()
    qb4 = nc.dram_tensor("qb4", [CPC], F32, kind="ExternalInput").ap()
    kb4 = nc.dram_tensor("kb4", [CPC], F32, kind="ExternalInput").ap()
    maskT = nc.dram_tensor("maskT", [P, P], F16, kind="ExternalInput").ap()
    y = nc.dram_tensor("y", [T, C], F16, kind="ExternalOutput").ap()

    args = (x8h, x8l, wq8h, wq8l, wk8h, wk8l, wv8h, wv8l, wp8h, wp8l,
            qb4, kb4, maskT, y)
    with tile.TileContext(nc) as tc:
        if loop_n == 1:
            _body(tc, nc, *args)
        else:
            with tc.For_i(0, loop_n, 1):
                _body(tc, nc, *args)
    nc.compile()
    return nc


def _body(tc, nc, x8h, x8l, wq8h, wq8l, wk8h, wk8l, wv8h, wv8l,
          wp8h, wp8l, qb4, kb4, maskT, y):
    from contextlib import ExitStack

    ctx = ExitStack()
    with ctx:
        const = ctx.enter_context(tc.tile_pool(name="const", bufs=1))
        x_pool = ctx.enter_context(tc.tile_pool(name="xp", bufs=2 * NU))
        w8_pool = ctx.enter_context(tc.tile_pool(name="w8p", bufs=1))
        v_pool = ctx.enter_context(tc.tile_pool(name="vp", bufs=NT))
        qtkt_pool = ctx.enter_context(tc.tile_pool(name="qtkt", bufs=3))
        bias_pool = ctx.enter_context(tc.tile_pool(name="biasp", bufs=2))
        pt_pool = ctx.enter_context(tc.tile_pool(name="ptp", bufs=23))
        osb_pool = ctx.enter_context(tc.tile_pool(name="osb", bufs=3))
        ot_pool = ctx.enter_context(tc.tile_pool(name="otp", bufs=4))
        r_pool = ctx.enter_context(tc.tile_pool(name="rp", bufs=4))
        y_pool = ctx.enter_context(tc.tile_pool(name="yp", bufs=3))
        mm_ps = ctx.enter_context(tc.tile_pool(name="mmps", bufs=2, space="PSUM"))
        s_ps = ctx.enter_context(tc.tile_pool(name="sps", bufs=2, space="PSUM"))
        o_ps = ctx.enter_context(tc.tile_pool(name="ops", bufs=2, space="PSUM"))

        # ---- weight + x loads (fp8 hi/lo pairs) ----
        # order matters for the startup critical path: wv + x first (phase V
        # needs them, unit-interleaved so the first V matmuls start early),
        # wq/wk next, wp last. Two DGE queues (SP + ACT) in parallel.
        wv_sb, xh_sb, xl_sb = [], [], []
        for tag, dr_ in (("wvh", wv8h), ("wvl", wv8l)):
            t_ = w8_pool.tile([P, NU, 2, CPC], F8, name=tag, tag=tag)
            wv_sb.append(t_)
        for i in range(NU):
            xh_sb.append(x_pool.tile([P, 2, T], F8, name=f"xh{i}", tag="x"))
            xl_sb.append(x_pool.tile([P, 2, T], F8, name=f"xl{i}", tag="x"))
        for i in range(NU):
            nc.sync.dma_start(out=wv_sb[0][:, i], in_=wv8h[i])
            nc.scalar.dma_start(out=wv_sb[1][:, i], in_=wv8l[i])
            sl = slice(0, T // 4)
            nc.sync.dma_start(out=xh_sb[i][:, :, sl], in_=x8h[i][:, :, sl])
            nc.scalar.dma_start(out=xl_sb[i][:, :, sl], in_=x8l[i][:, :, sl])
        mask_sb = const.tile([P, P], F16, tag="mask")
        nc.sync.dma_start(out=mask_sb, in_=maskT)
        for half in range(2):
            sl = slice(T // 4 + T * 3 // 8 * half,
                       T // 4 + T * 3 // 8 * (half + 1))
            for i in range(NU):
                nc.sync.dma_start(out=xh_sb[i][:, :, sl], in_=x8h[i][:, :, sl])
                nc.scalar.dma_start(out=xl_sb[i][:, :, sl], in_=x8l[i][:, :, sl])
        wq_sb, wk_sb = [], []
        for nm, drh, drl, lst in (("wq", wq8h, wq8l, wq_sb),
                                  ("wk", wk8h, wk8l, wk_sb)):
            for tag, dr_ in ((f"{nm}h", drh), (f"{nm}l", drl)):
                t_ = w8_pool.tile([P, NU, 2, CPC], F8, name=tag, tag=tag)
                eng = nc.sync if nm == "wq" else nc.scalar
                eng.dma_start(out=t_, in_=dr_.rearrange("i p j m -> p i j m"))
                lst.append(t_)
        qb_all = bias_pool.tile([P, NPAIR], F32, tag="qb", bufs=1)
        kb_all = bias_pool.tile([P, NPAIR], F32, tag="kb", bufs=1)
        nc.sync.dma_start(out=qb_all, in_=qb4.rearrange("(a p) -> p a", p=P))
        nc.sync.dma_start(out=kb_all, in_=kb4.rearrange("(a p) -> p a", p=P))

        def comp_mms(ps, lhs_hl, rhs_hl, lslice, rslice):
            """hi*hi + lo*hi + hi*lo DoubleRow accumulation over NU units."""
            terms = [(0, 0), (1, 0), (0, 1)]
            n = NU * len(terms) - 1
            cnt = 0
            for i in range(NU):
                for (a, b_) in terms:
                    nc.tensor.matmul(
                        ps, lslice(lhs_hl[a], i), rslice(rhs_hl[b_], i),
                        start=(cnt == 0), stop=(cnt == n), perf_mode=DR)
                    cnt += 1

        # ---- phase V: V for all 8 heads ----
        v_sb = []
        for t in range(NT):
            vps = mm_ps.tile([P, CPC], F32, name=f"vps{t}", tag="mm")
            comp_mms(
                vps, (xh_sb, xl_sb), (wv_sb[0], wv_sb[1]),
                lambda xs, i: xs[i][:, :, P * t:P * (t + 1)],
                lambda w, i: w[:, i])
            vt = v_pool.tile([P, HPC, HD + 1], F16, name=f"v{t}", tag="v")
            nc.vector.memset(vt[:, :, HD], 4.0)
            nc.vector.tensor_copy(
                out=vt[:, :, 0:HD],
                in_=vps.rearrange("p (h d) -> p h d", h=HPC))
            v_sb.append(vt)

        # ---- fp8 wp tiles for the output projection ----
        wp_sb = []
        for tag, dr_ in (("wph", wp8h), ("wpl", wp8l)):
            t_ = w8_pool.tile([P, 2, 2, C], F8, name=tag, tag=tag)
            for g in range(2):
                nc.sync.dma_start(out=t_[:, g], in_=dr_[g])
            wp_sb.append(t_)
        ot8h, ot8l, ot16 = [], [], []
        for g in range(2):
            ot8h.append(ot_pool.tile([P, 2, T], F8, name=f"oth{g}", tag="ot8"))
            ot8l.append(ot_pool.tile([P, 2, T], F8, name=f"otl{g}", tag="ot8"))
            ot16.append(ot_pool.tile([P, 2, T], F16, name=f"ot16{g}",
                                     tag="ot16", bufs=2))

        # ---- per head-pair: O^T via DMA-XBAR transpose + GpSimd fp8 split --
        def transpose_tq(p, o_sb, tq):
            """Blocked transpose of one o_sb q-window into ot8 hi/lo fp8."""
            g, j = p // 2, p % 2
            win = slice(QW * tq, QW * (tq + 1))
            out3 = ot16[g][:, j, win].rearrange("p (b c) -> p b c", b=4)
            nc.sync.dma_start(out=out3, in_=o_sb[:, win], transpose=True)
            nc.gpsimd.tensor_copy(out=ot8h[g][:, j, win], in_=ot16[g][:, j, win])
            nc.gpsimd.tensor_sub(ot8l[g][:, j, win], ot16[g][:, j, win],
                                 ot8h[g][:, j, win])

        def emit_y(tq):
            """Output-projection matmuls for the 4 t-tiles of one tq window."""
            for t in range(4 * tq, 4 * tq + 4):
                ysb = y_pool.tile([P, C], F16, name=f"y{t}", tag="y")
                for n2 in range(2):
                    yps = mm_ps.tile([P, QW], F32, name=f"yps{t}{n2}", tag="mm")
                    cnt = 0
                    for g in range(2):
                        for (osrc, wsrc) in ((ot8h[g], wp_sb[0]),
                                             (ot8h[g], wp_sb[1]),
                                             (ot8l[g], wp_sb[0])):
                            nc.tensor.matmul(
                                yps, osrc[:, :, P * t:P * (t + 1)],
                                wsrc[:, g, :, QW * n2:QW * (n2 + 1)],
                                start=(cnt == 0), stop=(cnt == 5), perf_mode=DR)
                            cnt += 1
                    nc.vector.tensor_scalar(
                        out=ysb[:, QW * n2:QW * (n2 + 1)], in0=yps,
                        scalar1=1.0 / 1024.0, scalar2=None,
                        op0=mybir.AluOpType.mult)
                nc.sync.dma_start(out=y[P * t:P * (t + 1), :], in_=ysb)

        def qk_proj(p, tq):
            """Q/K projection matmuls + fp8 PSUM evac for one t-window."""
            qka, _ = get_qtkt(p)
            for qk, (wsb, bsb) in enumerate(
                    ((wq_sb, qb_all[:, p:p + 1]),
                     (wk_sb, kb_all[:, p:p + 1]))):
                ps = mm_ps.tile([P, QW], F32, name=f"qk{p}{tq}", tag="mm")
                comp_mms(
                    ps, (wsb[0], wsb[1]), (xh_sb, xl_sb),
                    lambda w, i: w[:, i, :, P * p:P * (p + 1)],
                    lambda xs, i: xs[i][:, :, QW * tq:QW * (tq + 1)])
                nc.vector.tensor_scalar(
                    out=qka[:, qk, QW * tq:QW * (tq + 1)], in0=ps,
                    scalar1=1.0 / 16.0, scalar2=bsb,
                    op0=mybir.AluOpType.mult, op1=mybir.AluOpType.add)

        def qk_regroup(p, half):
            """[128, 2, T] channel-major fp8 -> [32(hl base), 2(j), 2(qk), T]
            DoubleRow layout: channel d = 32j + i at partition i, free j.
            One DMA per (hl, j) block moves both Q and K."""
            qka, qk8 = get_qtkt(p)
            win = slice(T // 2 * half, T // 2 * (half + 1))
            for hl in range(2):
                for j in range(2):
                    nc.sync.dma_start(
                        out=qk8[32 * hl:32 * (hl + 1), j, :, win],
                        in_=qka[64 * hl + 32 * j:64 * hl + 32 * (j + 1), :,
                                win])

        qtkt = {}

        def get_qtkt(p):
            if p not in qtkt:
                qtkt[p] = (
                    qtkt_pool.tile([P, 2, T], F8, name=f"qka{p}", tag="qka",
                                   bufs=3),
                    qtkt_pool.tile([64, 2, 2, T], F8, name=f"qk8{p}",
                                   tag="qk8", bufs=3),
                )
            return qtkt[p]

        for tq in range(NQT):
            qk_proj(0, tq)
        qk_regroup(0, 0)
        qk_regroup(0, 1)
        qk_proj(1, 0)
        qk_proj(1, 1)
        qk_regroup(1, 0)

        for p in range(NPAIR):
            _, qk8_sb = get_qtkt(p)
            o_sb = osb_pool.tile([P, T], F16, name=f"o{p}", tag="o")

            def s_unit(qt_i, hl):
                """S^T matmuls + exp + causal mask for one (q-tile, head)."""
                dlo, dhi = 32 * hl, 32 * (hl + 1)
                nkt = 4 * qt_i + 4
                pts = []
                for k0 in range(0, nkt, 2):
                    smin = [min(max(0, (k0 + u) - 4 * qt_i), 2)
                            for u in range(2)]
                    off = [P * s for s in smin]
                    sps = s_ps.tile([P, 2 * QW], F32,
                                    name=f"s{p}{hl}{qt_i}{k0}", tag="s")
                    for u in range(2):
                        k = k0 + u
                        nc.tensor.matmul(
                            sps[:, QW * u + off[u]:QW * (u + 1)],
                            qk8_sb[dlo:dhi, :, 1, P * k:P * (k + 1)],
                            qk8_sb[dlo:dhi, :, 0,
                                   QW * qt_i + off[u]:QW * (qt_i + 1)],
                            start=True, stop=True, perf_mode=DR)
                    pt = pt_pool.tile([P, 2 * QW], F16,
                                      name=f"pt{p}{hl}{qt_i}{k0}", tag="pt")
                    # exp in a single instruction per chunk: for unequal
                    # offsets, exp the union region (extra columns read stale
                    # psum; their pt slots are never consumed downstream)
                    eoff = min(off)
                    if eoff == 0:
                        nc.scalar.activation(
                            out=pt, in_=sps,
                            func=mybir.ActivationFunctionType.Exp,
                            scale=SCALE / 16.0)
                    else:
                        view = lambda ap: ap.rearrange(
                            "p (u c) -> p u c", u=2)[:, :, eoff:QW]
                        nc.scalar.activation(
                            out=view(pt), in_=view(sps),
                            func=mybir.ActivationFunctionType.Exp,
                            scale=SCALE / 16.0)
                    for u in range(2):
                        k = k0 + u
                        for s in range(4):
                            gs = 4 * qt_i + s
                            if gs == k:
                                sl = pt[:, QW * u + P * s:QW * u + P * (s + 1)]
                                nc.vector.tensor_mul(sl, sl, mask_sb)
                    pts.append(pt)
                return pts

            def pv_unit(qt_i, hl, pts):
                """P^T @ V_aug + normalize into o_sb for one unit."""
                hh = 2 * p + hl
                ops_ = o_ps.tile([P, 4 * (HD + 1)], F32,
                                 name=f"o{p}{hl}{qt_i}", tag="o")
                for s in range(4):
                    gs = 4 * qt_i + s
                    for k in range(gs + 1):
                        nc.tensor.matmul(
                            ops_[:, (HD + 1) * s:(HD + 1) * (s + 1)],
                            pts[k // 2][:, QW * (k % 2) + P * s:
                                        QW * (k % 2) + P * (s + 1)],
                            v_sb[k][:, hh, :],
                            start=(k == 0), stop=(k == gs))
                r_ = r_pool.tile([P, 4], F32, name=f"r{p}{hl}{qt_i}", tag="r")
                nc.vector.reciprocal(
                    r_, ops_.rearrange("p (s c) -> p s c", c=HD + 1)[:, :, HD])
                out_ap = o_sb[:, QW * qt_i:QW * (qt_i + 1)].rearrange(
                    "p (s h d) -> p s h d", s=4, h=2)[:, :, hl, :]
                nc.vector.tensor_mul(
                    out_ap,
                    ops_.rearrange("p (s c) -> p s c", c=HD + 1)[:, :, 0:HD],
                    r_.unsqueeze(2).broadcast_to((P, 4, HD)))

            # software pipeline: PV runs one unit behind S/exp. After each
            # q-window (hl == 1) completes: emit the NEXT pair's projection
            # matmuls for that window (PE filler for this ScalarE-bound
            # phase), then this window's transpose chain (DMA + GpSimd, no
            # PE). On the last pair the filler is the output projection,
            # lagged one window behind its transpose.
            last = p == NPAIR - 1
            y_ready = []

            def drain(pend):
                # Q/K projection + regroup run TWO pairs ahead (2nd half of
                # pair p+1 during this pair's first windows, 1st half of
                # pair p+2 during the later ones) so the DoubleRow-layout
                # tiles are ready a full pair before their S matmuls.
                qt_i, hl, pts = pend
                pv_unit(qt_i, hl, pts)
                if hl == 0:
                    if qt_i < 2:
                        if p + 1 < NPAIR:
                            qk_proj(p + 1, qt_i + 2)
                    else:
                        if p + 2 < NPAIR:
                            qk_proj(p + 2, qt_i - 2)
                else:
                    if qt_i == 1 and p + 1 < NPAIR:
                        qk_regroup(p + 1, 1)
                    if qt_i == 3 and p + 2 < NPAIR:
                        qk_regroup(p + 2, 0)
                    transpose_tq(p, o_sb, qt_i)
                    if last:
                        if y_ready:
                            emit_y(y_ready.pop(0))
                        y_ready.append(qt_i)

            units = [(qt_i, hl) for qt_i in range(NQT) for hl in range(2)]
            pq = []
            for (qt_i, hl) in units:
                pts = s_unit(qt_i, hl)
                pq.append((qt_i, hl, pts))
                if len(pq) > 1:
                    drain(pq.pop(0))
            for pend in pq:
                drain(pend)
            for tq in y_ready:
                emit_y(tq)


def _comp8(a):
    hi = a.astype(NPF8)
    lo = (a - hi.astype(np.float32)).astype(NPF8)
    return hi, lo


def _prep_inputs(x, w_attn, b_attn, w_proj):
    """Per-core input maps."""
    in_maps = []
    # contraction layout [unit, p, j, ...]: c = 256*unit + 128*j + p
    def units(a, n_u):
        # a: [n_u*256, M] -> [n_u, 128, 2, M]
        return np.ascontiguousarray(
            a.reshape(n_u, 2, P, -1).transpose(0, 2, 1, 3))

    wq_h = {}
    for g in range(2):
        qs = slice(CPC * g, CPC * (g + 1))
        ks = slice(C + CPC * g, C + CPC * (g + 1))
        vs = slice(2 * C + CPC * g, 2 * C + CPC * (g + 1))
        wqh, wql = _comp8(64.0 * w_attn[:, qs])
        wkh, wkl = _comp8(64.0 * w_attn[:, ks])
        wvh, wvl = _comp8(64.0 * w_attn[:, vs])
        wph, wpl = _comp8(64.0 * w_proj[CPC * g:CPC * (g + 1), :])
        wq_h[g] = dict(
            wq8h=units(wqh, NU), wq8l=units(wql, NU),
            wk8h=units(wkh, NU), wk8l=units(wkl, NU),
            wv8h=units(wvh, NU), wv8l=units(wvl, NU),
            wp8h=units(wph, 2), wp8l=units(wpl, 2),
            qb4=np.ascontiguousarray(4.0 * b_attn[qs]),
            kb4=np.ascontiguousarray(4.0 * b_attn[ks]),
        )
    for c in range(NCORES):
        b = c // 2
        g = c % 2
        xT = np.ascontiguousarray(x[b].T)          # [C, T] fp32
        xh, xl = _comp8(xT)
        in_maps.append({
            "x8h": units(xh, NU),
            "x8l": units(xl, NU),
            **wq_h[g],
            "maskT": np.triu(np.ones((P, P), dtype=np.float16)),
        })
    return in_maps


_CACHED_NC = None


def kernel(x, w_attn, b_attn, w_proj, b_proj):
    global _CACHED_NC
    x = np.asarray(x, dtype=np.float32)
    w_attn = np.asarray(w_attn, dtype=np.float32)
    b_attn = np.asarray(b_attn, dtype=np.float32)
    w_proj = np.asarray(w_proj, dtype=np.float32)
    b_proj = np.asarray(b_proj, dtype=np.float32)

    if _CACHED_NC is None:
        _CACHED_NC = build_kernel(loop_n=1)
    nc = _CACHED_NC
    in_maps = _prep_inputs(x, w_attn, b_attn, w_proj)
    res = run_bass_kernel_spmd(nc, in_maps, core_ids=list(range(NCORES)),
                               trace=False)
    out = np.empty((B, T, C), dtype=np.float32)
    # exact row-vector bias contribution: rows of softmax sum to 1
    for b in range(B):
        out[b] = (res.results[2 * b]["y"].astype(np.float32)
                  + res.results[2 * b + 1]["y"].astype(np.float32))
    bias_row = b_attn[2 * C:3 * C] @ w_proj + b_proj
    out += bias_row[None, None, :]
    return out


# revision 55
# speedup vs baseline: 1.1886x; 1.0365x over previous
"""Causal multi-head attention block (QKV proj -> causal attention -> out proj)
for Trainium2, sharded over 8 NeuronCores.

Sharding: tensor/data hybrid. Core c handles batch b = c//2 and half the heads
(g = c%2, 8 of 16 heads). Per core:
  - QKV projection with error-compensated fp8e4m3 DoubleRow matmuls
    (x = x_hi + x_lo, w = w_hi + w_lo; compute hi*hi + hi*lo + lo*hi,
    each a 256-deep DoubleRow matmul). Weights scaled x64 on host so fp8
    stays in the normal range; rescaled on the PSUM evacuation.
  - flash-style causal attention in S^T = K @ Q^T layout (fp16): exp on
    ScalarE, P^T (fp16) @ V_aug (fp16, scaled ones column appended ->
    0.25/rowsum for free from the DVE reciprocal)
  - DVE normalize, PE transpose of O, fp8-compensated output projection
    -> partial y [T, C]
Host: y[b] = partial[2b] + partial[2b+1] (+ bias terms, see below).

Biases: b_attn Q/K slices are added on-device (fused into the PSUM->SBUF
copies). The V-bias and b_proj contributions are exact row vectors on the
output (rows of softmax sum to 1): y += (b_v @ w_proj + b_proj), added on
host during the unshard.

Scaling ledger (all powers of 2, exact):
  wq8/wk8/wv8 = 64*w (hi+lo fp8 pair)   -> q/k/v psum = 64*true
  qt/kt = psum/16 + 4*qb = 4*true (fp16)
  S psum = 16*S_true; exp scale = HD^-0.5/16
  vt = 64*V (fp16), ones col = 4.0  -> recip gives 0.25/rowsum
  o_sb = (64*O')*(0.25/r) = 16*O (fp16)
  ot8 = fp8 pair of 16*O ; wp8 = 64*w_proj (hi+lo) -> y psum = 1024*y
  y = psum/1024 (fp32)
"""

import math

import numpy as np
import ml_dtypes

import concourse.bass as bass
import concourse.mybir as mybir
import concourse.tile as tile
from concourse import bacc
from concourse.bass_utils import run_bass_kernel_spmd

B, T, C = 4, 2048, 1024
NH, HD = 16, 64
NCORES = 8
HPC = NH // 2          # heads per core = 8
CPC = HPC * HD         # channels per core = 512
P = 128                # partitions
NT = T // P            # 16 t-tiles of 128
NU = C // 256          # 4 DoubleRow contraction units of 256
NPAIR = HPC // 2       # 4 head pairs
QW = 512               # q-tile width
NQT = T // QW          # 4 q-tiles

F32 = mybir.dt.float32
F16 = mybir.dt.float16
F8 = mybir.dt.float8e4
I16 = mybir.dt.int16
DR = mybir.MatmulPerfMode.DoubleRow
NPF8 = ml_dtypes.float8_e4m3
SCALE = HD ** -0.5
# Schraudolph fp16 exp approximation on DVE for a fraction of the softmax:
# exp(s) ~ bitcast_fp16(int16(s*EXP_A + EXP_B)). EXP_B tuned for min rel err
# assuming round-to-nearest int conversion.
EXP_A = (SCALE / 16.0) * 1.4426950408889634 * 1024.0
EXP_B = 15.0 * 1024.0 - 38.5


def build_kernel(loop_n: int = 1):
    nc = bacc.Bacc("TRN2", target_bir_lowering=False, debug=False)
    x8h = nc.dram_tensor("x8h", [NU, P, 2, T], F8, kind="ExternalInput").ap()
    x8l = nc.dram_tensor("x8l", [NU, P, 2, T], F8, kind="ExternalInput").ap()
    wq8h = nc.dram_tensor("wq8h", [NU, P, 2, CPC], F8, kind="ExternalInput").ap()
    wq8l = nc.dram_tensor("wq8l", [NU, P, 2, CPC], F8, kind="ExternalInput").ap()
    wk8h = nc.dram_tensor("wk8h", [NU, P, 2, CPC], F8, kind="ExternalInput").ap()
    wk8l = nc.dram_tensor("wk8l", [NU, P, 2, CPC], F8, kind="ExternalInput").ap()
    wv8h = nc.dram_tensor("wv8h", [NU, P, 2, CPC], F8, kind="ExternalInput").ap()
    wv8l = nc.dram_tensor("wv8l", [NU, P, 2, CPC], F8, kind="ExternalInput").ap()
    wp8h = nc.dram_tensor("wp8h", [2, P, 2, C], F8, kind="ExternalInput").ap()
    wp8l = nc.dram_tensor("wp8l", [2, P, 2, C], F8, kind="ExternalInput").ap()
    qb4 = nc.dram_tensor("qb4", [CPC], F32, kind="ExternalInput").ap()
    kb4 = nc.dram_tensor("kb4", [CPC], F32, kind="ExternalInput").ap()
    maskT = nc.dram_tensor("maskT", [P, P], F16, kind="ExternalInput").ap()
    y = nc.dram_tensor("y", [T, C], F16, kind="ExternalOutput").ap()

    args = (x8h, x8l, wq8h, wq8l, wk8h, wk8l, wv8h, wv8l, wp8h, wp8l,
            qb4, kb4, maskT, y)
    with tile.TileContext(nc) as tc:
        if loop_n == 1:
            _body(tc, nc, *args)
        else:
            with tc.For_i(0, loop_n, 1):
                _body(tc, nc, *args)
    nc.compile()
    return nc


def _body(tc, nc, x8h, x8l, wq8h, wq8l, wk8h, wk8l, wv8h, wv8l,
          wp8h, wp8l, qb4, kb4, maskT, y):
    from contextlib import ExitStack

    ctx = ExitStack()
    with ctx:
        const = ctx.enter_context(tc.tile_pool(name="const", bufs=1))
        x_pool = ctx.enter_context(tc.tile_pool(name="xp", bufs=2 * NU))
        w8_pool = ctx.enter_context(tc.tile_pool(name="w8p", bufs=1))
        v_pool = ctx.enter_context(tc.tile_pool(name="vp", bufs=NT))
        qtkt_pool = ctx.enter_context(tc.tile_pool(name="qtkt", bufs=3))
        bias_pool = ctx.enter_context(tc.tile_pool(name="biasp", bufs=2))
        pt_pool = ctx.enter_context(tc.tile_pool(name="ptp", bufs=17))
        osb_pool = ctx.enter_context(tc.tile_pool(name="osb", bufs=3))
        ot_pool = ctx.enter_context(tc.tile_pool(name="otp", bufs=4))
        r_pool = ctx.enter_context(tc.tile_pool(name="rp", bufs=4))
        y_pool = ctx.enter_context(tc.tile_pool(name="yp", bufs=3))
        mm_ps = ctx.enter_context(tc.tile_pool(name="mmps", bufs=2, space="PSUM"))
        s_ps = ctx.enter_context(tc.tile_pool(name="sps", bufs=2, space="PSUM"))
        o_ps = ctx.enter_context(tc.tile_pool(name="ops", bufs=2, space="PSUM"))

        # ---- weight + x loads (fp8 hi/lo pairs) ----
        # order matters for the startup critical path: wv + x first (phase V
        # needs them, unit-interleaved so the first V matmuls start early),
        # wq/wk next, wp last. Two DGE queues (SP + ACT) in parallel.
        wv_sb, xh_sb, xl_sb = [], [], []
        for tag, dr_ in (("wvh", wv8h), ("wvl", wv8l)):
            t_ = w8_pool.tile([P, NU, 2, CPC], F8, name=tag, tag=tag)
            wv_sb.append(t_)
        for i in range(NU):
            xh_sb.append(x_pool.tile([P, 2, T], F8, name=f"xh{i}", tag="x"))
            xl_sb.append(x_pool.tile([P, 2, T], F8, name=f"xl{i}", tag="x"))
        for i in range(NU):
            nc.sync.dma_start(out=wv_sb[0][:, i], in_=wv8h[i])
            nc.scalar.dma_start(out=wv_sb[1][:, i], in_=wv8l[i])
            sl = slice(0, T // 4)
            nc.sync.dma_start(out=xh_sb[i][:, :, sl], in_=x8h[i][:, :, sl])
            nc.scalar.dma_start(out=xl_sb[i][:, :, sl], in_=x8l[i][:, :, sl])
        mask_sb = const.tile([P, P], F16, tag="mask")
        nc.sync.dma_start(out=mask_sb, in_=maskT)
        for half in range(2):
            sl = slice(T // 4 + T * 3 // 8 * half,
                       T // 4 + T * 3 // 8 * (half + 1))
            for i in range(NU):
                nc.sync.dma_start(out=xh_sb[i][:, :, sl], in_=x8h[i][:, :, sl])
                nc.scalar.dma_start(out=xl_sb[i][:, :, sl], in_=x8l[i][:, :, sl])
        wq_sb, wk_sb = [], []
        for nm, drh, drl, lst in (("wq", wq8h, wq8l, wq_sb),
                                  ("wk", wk8h, wk8l, wk_sb)):
            for tag, dr_ in ((f"{nm}h", drh), (f"{nm}l", drl)):
                t_ = w8_pool.tile([P, NU, 2, CPC], F8, name=tag, tag=tag)
                eng = nc.sync if nm == "wq" else nc.scalar
                eng.dma_start(out=t_, in_=dr_.rearrange("i p j m -> p i j m"))
                lst.append(t_)
        qb_all = bias_pool.tile([P, NPAIR], F32, tag="qb", bufs=1)
        kb_all = bias_pool.tile([P, NPAIR], F32, tag="kb", bufs=1)
        nc.sync.dma_start(out=qb_all, in_=qb4.rearrange("(a p) -> p a", p=P))
        nc.sync.dma_start(out=kb_all, in_=kb4.rearrange("(a p) -> p a", p=P))

        def comp_mms(ps, lhs_hl, rhs_hl, lslice, rslice):
            """hi*hi + lo*hi + hi*lo DoubleRow accumulation over NU units."""
            terms = [(0, 0), (1, 0), (0, 1)]
            n = NU * len(terms) - 1
            cnt = 0
            for i in range(NU):
                for (a, b_) in terms:
                    nc.tensor.matmul(
                        ps, lslice(lhs_hl[a], i), rslice(rhs_hl[b_], i),
                        start=(cnt == 0), stop=(cnt == n), perf_mode=DR)
                    cnt += 1

        # ---- phase V: V for all 8 heads ----
        v_sb = []
        for t in range(NT):
            vps = mm_ps.tile([P, CPC], F32, name=f"vps{t}", tag="mm")
            comp_mms(
                vps, (xh_sb, xl_sb), (wv_sb[0], wv_sb[1]),
                lambda xs, i: xs[i][:, :, P * t:P * (t + 1)],
                lambda w, i: w[:, i])
            vt = v_pool.tile([P, HPC, HD + 1], F16, name=f"v{t}", tag="v")
            nc.vector.memset(vt[:, :, HD], 4.0)
            nc.vector.tensor_copy(
                out=vt[:, :, 0:HD],
                in_=vps.rearrange("p (h d) -> p h d", h=HPC))
            v_sb.append(vt)

        # ---- fp8 wp tiles for the output projection ----
        wp_sb = []
        for tag, dr_ in (("wph", wp8h), ("wpl", wp8l)):
            t_ = w8_pool.tile([P, 2, 2, C], F8, name=tag, tag=tag)
            for g in range(2):
                nc.sync.dma_start(out=t_[:, g], in_=dr_[g])
            wp_sb.append(t_)
        ot8h, ot8l, ot16 = [], [], []
        for g in range(2):
            ot8h.append(ot_pool.tile([P, 2, T], F8, name=f"oth{g}", tag="ot8"))
            ot8l.append(ot_pool.tile([P, 2, T], F8, name=f"otl{g}", tag="ot8"))
            ot16.append(ot_pool.tile([P, 2, T], F16, name=f"ot16{g}",
                                     tag="ot16", bufs=2))

        # ---- per head-pair: O^T via DMA-XBAR transpose + GpSimd fp8 split --
        def transpose_tq(p, o_sb, tq):
            """Blocked transpose of one o_sb q-window into ot8 hi/lo fp8."""
            g, j = p // 2, p % 2
            win = slice(QW * tq, QW * (tq + 1))
            out3 = ot16[g][:, j, win].rearrange("p (b c) -> p b c", b=4)
            nc.sync.dma_start(out=out3, in_=o_sb[:, win], transpose=True)
            nc.gpsimd.tensor_copy(out=ot8h[g][:, j, win], in_=ot16[g][:, j, win])
            nc.gpsimd.tensor_sub(ot8l[g][:, j, win], ot16[g][:, j, win],
                                 ot8h[g][:, j, win])

        def emit_y(tq):
            """Output-projection matmuls for the 4 t-tiles of one tq window."""
            for t in range(4 * tq, 4 * tq + 4):
                ysb = y_pool.tile([P, C], F16, name=f"y{t}", tag="y")
                for n2 in range(2):
                    yps = mm_ps.tile([P, QW], F32, name=f"yps{t}{n2}", tag="mm")
                    cnt = 0
                    for g in range(2):
                        for (osrc, wsrc) in ((ot8h[g], wp_sb[0]),
                                             (ot8h[g], wp_sb[1]),
                                             (ot8l[g], wp_sb[0])):
                            nc.tensor.matmul(
                                yps, osrc[:, :, P * t:P * (t + 1)],
                                wsrc[:, g, :, QW * n2:QW * (n2 + 1)],
                                start=(cnt == 0), stop=(cnt == 5), perf_mode=DR)
                            cnt += 1
                    nc.vector.tensor_scalar(
                        out=ysb[:, QW * n2:QW * (n2 + 1)], in0=yps,
                        scalar1=1.0 / 1024.0, scalar2=None,
                        op0=mybir.AluOpType.mult)
                nc.sync.dma_start(out=y[P * t:P * (t + 1), :], in_=ysb)

        def qk_proj(p, tq):
            """Q/K projection matmuls + fp8 PSUM evac for one t-window."""
            qka, _ = get_qtkt(p)
            for qk, (wsb, bsb) in enumerate(
                    ((wq_sb, qb_all[:, p:p + 1]),
                     (wk_sb, kb_all[:, p:p + 1]))):
                ps = mm_ps.tile([P, QW], F32, name=f"qk{p}{tq}", tag="mm")
                comp_mms(
                    ps, (wsb[0], wsb[1]), (xh_sb, xl_sb),
                    lambda w, i: w[:, i, :, P * p:P * (p + 1)],
                    lambda xs, i: xs[i][:, :, QW * tq:QW * (tq + 1)])
                nc.vector.tensor_scalar(
                    out=qka[:, qk, QW * tq:QW * (tq + 1)], in0=ps,
                    scalar1=1.0 / 16.0, scalar2=bsb,
                    op0=mybir.AluOpType.mult, op1=mybir.AluOpType.add)

        def qk_regroup(p, half):
            """[128, 2, T] channel-major fp8 -> [32(hl base), 2(j), 2(qk), T]
            DoubleRow layout: channel d = 32j + i at partition i, free j.
            One DMA per (hl, j) block moves both Q and K."""
            qka, qk8 = get_qtkt(p)
            win = slice(T // 2 * half, T // 2 * (half + 1))
            for hl in range(2):
                for j in range(2):
                    nc.sync.dma_start(
                        out=qk8[32 * hl:32 * (hl + 1), j, :, win],
                        in_=qka[64 * hl + 32 * j:64 * hl + 32 * (j + 1), :,
                                win])

        qtkt = {}

        def get_qtkt(p):
            if p not in qtkt:
                qtkt[p] = (
                    qtkt_pool.tile([P, 2, T], F8, name=f"qka{p}", tag="qka",
                                   bufs=2),
                    qtkt_pool.tile([64, 2, 2, T], F8, name=f"qk8{p}",
                                   tag="qk8", bufs=2),
                )
            return qtkt[p]

        for tq in range(NQT):
            qk_proj(0, tq)
        qk_regroup(0, 0)
        qk_regroup(0, 1)

        for p in range(NPAIR):
            _, qk8_sb = get_qtkt(p)
            o_sb = osb_pool.tile([P, T], F16, name=f"o{p}", tag="o")

            def s_unit(qt_i, hl):
                """S^T matmuls + exp + causal mask for one (q-tile, head)."""
                dlo, dhi = 32 * hl, 32 * (hl + 1)
                nkt = 4 * qt_i + 4
                pts = []
                for k0 in range(0, nkt, 2):
                    smin = [min(max(0, (k0 + u) - 4 * qt_i), 2)
                            for u in range(2)]
                    off = [P * s for s in smin]
                    sps = s_ps.tile([P, 2 * QW], F32,
                                    name=f"s{p}{hl}{qt_i}{k0}", tag="s")
                    for u in range(2):
                        k = k0 + u
                        nc.tensor.matmul(
                            sps[:, QW * u + off[u]:QW * (u + 1)],
                            qk8_sb[dlo:dhi, :, 1, P * k:P * (k + 1)],
                            qk8_sb[dlo:dhi, :, 0,
                                   QW * qt_i + off[u]:QW * (qt_i + 1)],
                            start=True, stop=True, perf_mode=DR)
                    pt = pt_pool.tile([P, 2 * QW], F16,
                                      name=f"pt{p}{hl}{qt_i}{k0}", tag="pt")
                    # exp in a single instruction per chunk: for unequal
                    # offsets, exp the union region (extra columns read stale
                    # psum; their pt slots are never consumed downstream).
                    # A fraction of full-width chunks runs on DVE via the
                    # Schraudolph bit-trick to unload the ScalarE bottleneck.
                    eoff = min(off)
                    schr = eoff == 0 and k0 % 8 == 2
                    if schr:
                        nc.vector.tensor_scalar(
                            out=pt.bitcast(I16), in0=sps,
                            scalar1=EXP_A, scalar2=EXP_B,
                            op0=mybir.AluOpType.mult,
                            op1=mybir.AluOpType.add)
                    elif eoff == 0:
                        nc.scalar.activation(
                            out=pt, in_=sps,
                            func=mybir.ActivationFunctionType.Exp,
                            scale=SCALE / 16.0)
                    else:
                        view = lambda ap: ap.rearrange(
                            "p (u c) -> p u c", u=2)[:, :, eoff:QW]
                        nc.scalar.activation(
                            out=view(pt), in_=view(sps),
                            func=mybir.ActivationFunctionType.Exp,
                            scale=SCALE / 16.0)
                    for u in range(2):
                        k = k0 + u
                        for s in range(4):
                            gs = 4 * qt_i + s
                            if gs == k:
                                sl = pt[:, QW * u + P * s:QW * u + P * (s + 1)]
                                nc.vector.tensor_mul(sl, sl, mask_sb)
                    pts.append(pt)
                return pts

            def pv_unit(qt_i, hl, pts):
                """P^T @ V_aug + normalize into o_sb for one unit."""
                hh = 2 * p + hl
                ops_ = o_ps.tile([P, 4 * (HD + 1)], F32,
                                 name=f"o{p}{hl}{qt_i}", tag="o")
                for s in range(4):
                    gs = 4 * qt_i + s
                    for k in range(gs + 1):
                        nc.tensor.matmul(
                            ops_[:, (HD + 1) * s:(HD + 1) * (s + 1)],
                            pts[k // 2][:, QW * (k % 2) + P * s:
                                        QW * (k % 2) + P * (s + 1)],
                            v_sb[k][:, hh, :],
                            start=(k == 0), stop=(k == gs))
                r_ = r_pool.tile([P, 4], F32, name=f"r{p}{hl}{qt_i}", tag="r")
                nc.vector.reciprocal(
                    r_, ops_.rearrange("p (s c) -> p s c", c=HD + 1)[:, :, HD])
                out_ap = o_sb[:, QW * qt_i:QW * (qt_i + 1)].rearrange(
                    "p (s h d) -> p s h d", s=4, h=2)[:, :, hl, :]
                nc.vector.tensor_mul(
                    out_ap,
                    ops_.rearrange("p (s c) -> p s c", c=HD + 1)[:, :, 0:HD],
                    r_.unsqueeze(2).broadcast_to((P, 4, HD)))

            # software pipeline: PV runs one unit behind S/exp. After each
            # q-window (hl == 1) completes: emit the NEXT pair's projection
            # matmuls for that window (PE filler for this ScalarE-bound
            # phase), then this window's transpose chain (DMA + GpSimd, no
            # PE). On the last pair the filler is the output projection,
            # lagged one window behind its transpose.
            last = p == NPAIR - 1
            y_ready = []

            def drain(pend):
                qt_i, hl, pts = pend
                pv_unit(qt_i, hl, pts)
                if hl == 0:
                    if not last:
                        qk_proj(p + 1, qt_i)
                        if qt_i == NQT - 1:
                            qk_regroup(p + 1, 1)
                else:
                    if not last and qt_i == 1:
                        qk_regroup(p + 1, 0)
                    transpose_tq(p, o_sb, qt_i)
                    if last:
                        if y_ready:
                            emit_y(y_ready.pop(0))
                        y_ready.append(qt_i)

            units = [(qt_i, hl) for qt_i in range(NQT) for hl in range(2)]
            pq = []
            for (qt_i, hl) in units:
                pts = s_unit(qt_i, hl)
                pq.append((qt_i, hl, pts))
                if len(pq) > 1:
                    drain(pq.pop(0))
            for pend in pq:
                drain(pend)
            for tq in y_ready:
                emit_y(tq)


def _comp8(a):
    hi = a.astype(NPF8)
    lo = (a - hi.astype(np.float32)).astype(NPF8)
    return hi, lo


def _prep_inputs(x, w_attn, b_attn, w_proj):
    """Per-core input maps."""
    in_maps = []
    # contraction layout [unit, p, j, ...]: c = 256*unit + 128*j + p
    def units(a, n_u):
        # a: [n_u*256, M] -> [n_u, 128, 2, M]
        return np.ascontiguousarray(
            a.reshape(n_u, 2, P, -1).transpose(0, 2, 1, 3))

    wq_h = {}
    for g in range(2):
        qs = slice(CPC * g, CPC * (g + 1))
        ks = slice(C + CPC * g, C + CPC * (g + 1))
        vs = slice(2 * C + CPC * g, 2 * C + CPC * (g + 1))
        wqh, wql = _comp8(64.0 * w_attn[:, qs])
        wkh, wkl = _comp8(64.0 * w_attn[:, ks])
        wvh, wvl = _comp8(64.0 * w_attn[:, vs])
        wph, wpl = _comp8(64.0 * w_proj[CPC * g:CPC * (g + 1), :])
        wq_h[g] = dict(
            wq8h=units(wqh, NU), wq8l=units(wql, NU),
            wk8h=units(wkh, NU), wk8l=units(wkl, NU),
            wv8h=units(wvh, NU), wv8l=units(wvl, NU),
            wp8h=units(wph, 2), wp8l=units(wpl, 2),
            qb4=np.ascontiguousarray(4.0 * b_attn[qs]),
            kb4=np.ascontiguousarray(4.0 * b_attn[ks]),
        )
    for c in range(NCORES):
        b = c // 2
        g = c % 2
        xT = np.ascontiguousarray(x[b].T)          # [C, T] fp32
        xh, xl = _comp8(xT)
        in_maps.append({
            "x8h": units(xh, NU),
            "x8l": units(xl, NU),
            **wq_h[g],
            "maskT": np.triu(np.ones((P, P), dtype=np.float16)),
        })
    return in_maps


_CACHED_NC = None


def kernel(x, w_attn, b_attn, w_proj, b_proj):
    global _CACHED_NC
    x = np.asarray(x, dtype=np.float32)
    w_attn = np.asarray(w_attn, dtype=np.float32)
    b_attn = np.asarray(b_attn, dtype=np.float32)
    w_proj = np.asarray(w_proj, dtype=np.float32)
    b_proj = np.asarray(b_proj, dtype=np.float32)

    if _CACHED_NC is None:
        _CACHED_NC = build_kernel(loop_n=1)
    nc = _CACHED_NC
    in_maps = _prep_inputs(x, w_attn, b_attn, w_proj)
    res = run_bass_kernel_spmd(nc, in_maps, core_ids=list(range(NCORES)),
                               trace=False)
    out = np.empty((B, T, C), dtype=np.float32)
    # exact row-vector bias contribution: rows of softmax sum to 1
    for b in range(B):
        out[b] = (res.results[2 * b]["y"].astype(np.float32)
                  + res.results[2 * b + 1]["y"].astype(np.float32))
    bias_row = b_attn[2 * C:3 * C] @ w_proj + b_proj
    out += bias_row[None, None, :]
    return out


# revision 63
# speedup vs baseline: 1.2609x; 1.0608x over previous
"""Causal multi-head attention block (QKV proj -> causal attention -> out proj)
for Trainium2, sharded over 8 NeuronCores.

Sharding: tensor/data hybrid. Core c handles batch b = c//2 and half the heads
(g = c%2, 8 of 16 heads). Per core:
  - QKV projection with error-compensated fp8e4m3 DoubleRow matmuls
    (x = x_hi + x_lo, w = w_hi + w_lo; compute hi*hi + hi*lo + lo*hi,
    each a 256-deep DoubleRow matmul). Weights scaled x64 on host so fp8
    stays in the normal range; rescaled on the PSUM evacuation.
  - flash-style causal attention in S^T = K @ Q^T layout (fp16): exp on
    ScalarE, P^T (fp16) @ V_aug (fp16, scaled ones column appended ->
    0.25/rowsum for free from the DVE reciprocal)
  - DVE normalize, PE transpose of O, fp8-compensated output projection
    -> partial y [T, C]
Host: y[b] = partial[2b] + partial[2b+1] (+ bias terms, see below).

Biases: b_attn Q/K slices are added on-device (fused into the PSUM->SBUF
copies). The V-bias and b_proj contributions are exact row vectors on the
output (rows of softmax sum to 1): y += (b_v @ w_proj + b_proj), added on
host during the unshard.

Scaling ledger (all powers of 2, exact):
  wq8/wk8/wv8 = 64*w (hi+lo fp8 pair)   -> q/k/v psum = 64*true
  qt/kt = psum/16 + 4*qb = 4*true (fp16)
  S psum = 16*S_true; exp scale = HD^-0.5/16
  vt = 64*V (fp16), ones col = 4.0  -> recip gives 0.25/rowsum
  o_sb = (64*O')*(0.25/r) = 16*O (fp16)
  ot8 = fp8 pair of 16*O ; wp8 = 64*w_proj (hi+lo) -> y psum = 1024*y
  y = psum/1024 (fp32)
"""

import math

import numpy as np
import ml_dtypes

import concourse.bass as bass
import concourse.mybir as mybir
import concourse.tile as tile
from concourse import bacc
from concourse.bass_utils import run_bass_kernel_spmd

B, T, C = 4, 2048, 1024
NH, HD = 16, 64
NCORES = 8
HPC = NH // 2          # heads per core = 8
CPC = HPC * HD         # channels per core = 512
P = 128                # partitions
NT = T // P            # 16 t-tiles of 128
NU = C // 256          # 4 DoubleRow contraction units of 256
NPAIR = HPC // 2       # 4 head pairs
QW = 512               # q-tile width
NQT = T // QW          # 4 q-tiles

F32 = mybir.dt.float32
F16 = mybir.dt.float16
F8 = mybir.dt.float8e4
I16 = mybir.dt.int16
DR = mybir.MatmulPerfMode.DoubleRow
NPF8 = ml_dtypes.float8_e4m3
SCALE = HD ** -0.5
# Schraudolph fp16 exp approximation on DVE for a fraction of the softmax:
# exp(s) ~ bitcast_fp16(int16(s*EXP_A + EXP_B)). EXP_B tuned for min rel err
# assuming round-to-nearest int conversion.
EXP_A = (SCALE / 16.0) * 1.4426950408889634 * 1024.0
EXP_B = 15.0 * 1024.0 - 38.5


def build_kernel(loop_n: int = 1):
    nc = bacc.Bacc("TRN2", target_bir_lowering=False, debug=False)
    x8h = nc.dram_tensor("x8h", [NU, P, 2, T], F8, kind="ExternalInput").ap()
    x8l = nc.dram_tensor("x8l", [NU, P, 2, T], F8, kind="ExternalInput").ap()
    wq8h = nc.dram_tensor("wq8h", [NU, P, 2, CPC], F8, kind="ExternalInput").ap()
    wq8l = nc.dram_tensor("wq8l", [NU, P, 2, CPC], F8, kind="ExternalInput").ap()
    wk8h = nc.dram_tensor("wk8h", [NU, P, 2, CPC], F8, kind="ExternalInput").ap()
    wk8l = nc.dram_tensor("wk8l", [NU, P, 2, CPC], F8, kind="ExternalInput").ap()
    wv8h = nc.dram_tensor("wv8h", [NU, P, 2, CPC], F8, kind="ExternalInput").ap()
    wv8l = nc.dram_tensor("wv8l", [NU, P, 2, CPC], F8, kind="ExternalInput").ap()
    wp8h = nc.dram_tensor("wp8h", [2, P, 2, C], F8, kind="ExternalInput").ap()
    wp8l = nc.dram_tensor("wp8l", [2, P, 2, C], F8, kind="ExternalInput").ap()
    qb4 = nc.dram_tensor("qb4", [CPC], F32, kind="ExternalInput").ap()
    kb4 = nc.dram_tensor("kb4", [CPC], F32, kind="ExternalInput").ap()
    maskT = nc.dram_tensor("maskT", [P, P], F16, kind="ExternalInput").ap()
    y = nc.dram_tensor("y", [T, C], F16, kind="ExternalOutput").ap()

    args = (x8h, x8l, wq8h, wq8l, wk8h, wk8l, wv8h, wv8l, wp8h, wp8l,
            qb4, kb4, maskT, y)
    with tile.TileContext(nc) as tc:
        if loop_n == 1:
            _body(tc, nc, *args)
        else:
            with tc.For_i(0, loop_n, 1):
                _body(tc, nc, *args)
    nc.compile()
    return nc


def _body(tc, nc, x8h, x8l, wq8h, wq8l, wk8h, wk8l, wv8h, wv8l,
          wp8h, wp8l, qb4, kb4, maskT, y):
    from contextlib import ExitStack

    ctx = ExitStack()
    with ctx:
        const = ctx.enter_context(tc.tile_pool(name="const", bufs=1))
        x_pool = ctx.enter_context(tc.tile_pool(name="xp", bufs=2 * NU))
        w8_pool = ctx.enter_context(tc.tile_pool(name="w8p", bufs=1))
        v_pool = ctx.enter_context(tc.tile_pool(name="vp", bufs=NT))
        qtkt_pool = ctx.enter_context(tc.tile_pool(name="qtkt", bufs=3))
        bias_pool = ctx.enter_context(tc.tile_pool(name="biasp", bufs=2))
        pt_pool = ctx.enter_context(tc.tile_pool(name="ptp", bufs=17))
        osb_pool = ctx.enter_context(tc.tile_pool(name="osb", bufs=3))
        ot_pool = ctx.enter_context(tc.tile_pool(name="otp", bufs=4))
        r_pool = ctx.enter_context(tc.tile_pool(name="rp", bufs=4))
        y_pool = ctx.enter_context(tc.tile_pool(name="yp", bufs=3))
        mm_ps = ctx.enter_context(tc.tile_pool(name="mmps", bufs=2, space="PSUM"))
        s_ps = ctx.enter_context(tc.tile_pool(name="sps", bufs=2, space="PSUM"))
        o_ps = ctx.enter_context(tc.tile_pool(name="ops", bufs=2, space="PSUM"))

        # ---- weight + x loads (fp8 hi/lo pairs) ----
        # order matters for the startup critical path: wv + x first (phase V
        # needs them, unit-interleaved so the first V matmuls start early),
        # wq/wk next, wp last. Two DGE queues (SP + ACT) in parallel.
        mask_sb = const.tile([P, P], F16, tag="mask")
        nc.sync.dma_start(out=mask_sb, in_=maskT)
        wv_sb, xh_sb, xl_sb = [], [], []
        for tag, dr_ in (("wvh", wv8h), ("wvl", wv8l)):
            t_ = w8_pool.tile([P, NU, 2, CPC], F8, name=tag, tag=tag)
            wv_sb.append(t_)
        for i in range(NU):
            xh_sb.append(x_pool.tile([P, 2, T], F8, name=f"xh{i}", tag="x"))
            xl_sb.append(x_pool.tile([P, 2, T], F8, name=f"xl{i}", tag="x"))
        for i in range(NU):
            nc.sync.dma_start(out=wv_sb[0][:, i], in_=wv8h[i])
            nc.scalar.dma_start(out=wv_sb[1][:, i], in_=wv8l[i])
            sl = slice(0, T // 4)
            nc.sync.dma_start(out=xh_sb[i][:, :, sl], in_=x8h[i][:, :, sl])
            nc.scalar.dma_start(out=xl_sb[i][:, :, sl], in_=x8l[i][:, :, sl])
        for half in range(2):
            sl = slice(T // 4 + T * 3 // 8 * half,
                       T // 4 + T * 3 // 8 * (half + 1))
            for i in range(NU):
                nc.sync.dma_start(out=xh_sb[i][:, :, sl], in_=x8h[i][:, :, sl])
                nc.scalar.dma_start(out=xl_sb[i][:, :, sl], in_=x8l[i][:, :, sl])
        wq_sb, wk_sb = [], []
        for nm, drh, drl, lst in (("wq", wq8h, wq8l, wq_sb),
                                  ("wk", wk8h, wk8l, wk_sb)):
            for tag, dr_ in ((f"{nm}h", drh), (f"{nm}l", drl)):
                t_ = w8_pool.tile([P, NU, 2, CPC], F8, name=tag, tag=tag)
                eng = nc.sync if nm == "wq" else nc.scalar
                eng.dma_start(out=t_, in_=dr_.rearrange("i p j m -> p i j m"))
                lst.append(t_)
        qb_all = bias_pool.tile([P, NPAIR], F32, tag="qb", bufs=1)
        kb_all = bias_pool.tile([P, NPAIR], F32, tag="kb", bufs=1)
        nc.sync.dma_start(out=qb_all, in_=qb4.rearrange("(a p) -> p a", p=P))
        nc.sync.dma_start(out=kb_all, in_=kb4.rearrange("(a p) -> p a", p=P))

        def comp_mms(ps, lhs_hl, rhs_hl, lslice, rslice):
            """hi*hi + lo*hi + hi*lo DoubleRow accumulation over NU units."""
            terms = [(0, 0), (1, 0), (0, 1)]
            n = NU * len(terms) - 1
            cnt = 0
            for i in range(NU):
                for (a, b_) in terms:
                    nc.tensor.matmul(
                        ps, lslice(lhs_hl[a], i), rslice(rhs_hl[b_], i),
                        start=(cnt == 0), stop=(cnt == n), perf_mode=DR)
                    cnt += 1

        # ---- phase V: V for all 8 heads ----
        v_sb = []
        for t in range(NT):
            vps = mm_ps.tile([P, CPC], F32, name=f"vps{t}", tag="mm")
            comp_mms(
                vps, (xh_sb, xl_sb), (wv_sb[0], wv_sb[1]),
                lambda xs, i: xs[i][:, :, P * t:P * (t + 1)],
                lambda w, i: w[:, i])
            vt = v_pool.tile([P, HPC, HD + 1], F16, name=f"v{t}", tag="v")
            nc.vector.memset(vt[:, :, HD], 4.0)
            nc.vector.tensor_copy(
                out=vt[:, :, 0:HD],
                in_=vps.rearrange("p (h d) -> p h d", h=HPC))
            v_sb.append(vt)

        # ---- fp8 wp tiles for the output projection ----
        wp_sb = []
        for tag, dr_ in (("wph", wp8h), ("wpl", wp8l)):
            t_ = w8_pool.tile([P, 2, 2, C], F8, name=tag, tag=tag)
            for g in range(2):
                nc.sync.dma_start(out=t_[:, g], in_=dr_[g])
            wp_sb.append(t_)
        ot8h, ot8l, ot16 = [], [], []
        for g in range(2):
            ot8h.append(ot_pool.tile([P, 2, T], F8, name=f"oth{g}", tag="ot8"))
            ot8l.append(ot_pool.tile([P, 2, T], F8, name=f"otl{g}", tag="ot8"))
            ot16.append(ot_pool.tile([P, 2, T], F16, name=f"ot16{g}",
                                     tag="ot16", bufs=2))

        # ---- per head-pair: O^T via DMA-XBAR transpose + GpSimd fp8 split --
        def transpose_tq(p, o_sb, tq):
            """Blocked transpose of one o_sb q-window into ot8 hi/lo fp8."""
            g, j = p // 2, p % 2
            win = slice(QW * tq, QW * (tq + 1))
            out3 = ot16[g][:, j, win].rearrange("p (b c) -> p b c", b=4)
            nc.sync.dma_start(out=out3, in_=o_sb[:, win], transpose=True)
            nc.gpsimd.tensor_copy(out=ot8h[g][:, j, win], in_=ot16[g][:, j, win])
            nc.gpsimd.tensor_sub(ot8l[g][:, j, win], ot16[g][:, j, win],
                                 ot8h[g][:, j, win])

        def emit_y(tq):
            """Output-projection matmuls for the 4 t-tiles of one tq window."""
            for t in range(4 * tq, 4 * tq + 4):
                ysb = y_pool.tile([P, C], F16, name=f"y{t}", tag="y")
                for n2 in range(2):
                    yps = mm_ps.tile([P, QW], F32, name=f"yps{t}{n2}", tag="mm")
                    cnt = 0
                    for g in range(2):
                        for (osrc, wsrc) in ((ot8h[g], wp_sb[0]),
                                             (ot8h[g], wp_sb[1]),
                                             (ot8l[g], wp_sb[0])):
                            nc.tensor.matmul(
                                yps, osrc[:, :, P * t:P * (t + 1)],
                                wsrc[:, g, :, QW * n2:QW * (n2 + 1)],
                                start=(cnt == 0), stop=(cnt == 5), perf_mode=DR)
                            cnt += 1
                    nc.vector.tensor_scalar(
                        out=ysb[:, QW * n2:QW * (n2 + 1)], in0=yps,
                        scalar1=1.0 / 1024.0, scalar2=None,
                        op0=mybir.AluOpType.mult)
                nc.sync.dma_start(out=y[P * t:P * (t + 1), :], in_=ysb)

        def qk_proj(p, tq):
            """Q/K projection matmuls + fp8 PSUM evac for one t-window."""
            qka, _ = get_qtkt(p)
            for qk, (wsb, bsb) in enumerate(
                    ((wq_sb, qb_all[:, p:p + 1]),
                     (wk_sb, kb_all[:, p:p + 1]))):
                ps = mm_ps.tile([P, QW], F32, name=f"qk{p}{tq}", tag="mm")
                comp_mms(
                    ps, (wsb[0], wsb[1]), (xh_sb, xl_sb),
                    lambda w, i: w[:, i, :, P * p:P * (p + 1)],
                    lambda xs, i: xs[i][:, :, QW * tq:QW * (tq + 1)])
                nc.vector.tensor_scalar(
                    out=qka[:, qk, QW * tq:QW * (tq + 1)], in0=ps,
                    scalar1=1.0 / 16.0, scalar2=bsb,
                    op0=mybir.AluOpType.mult, op1=mybir.AluOpType.add)

        def qk_regroup(p, half):
            """[128, 2, T] channel-major fp8 -> [32(hl base), 2(j), 2(qk), T]
            DoubleRow layout: channel d = 32j + i at partition i, free j.
            One DMA per (hl, j) block moves both Q and K."""
            qka, qk8 = get_qtkt(p)
            win = slice(T // 2 * half, T // 2 * (half + 1))
            for hl in range(2):
                for j in range(2):
                    nc.sync.dma_start(
                        out=qk8[32 * hl:32 * (hl + 1), j, :, win],
                        in_=qka[64 * hl + 32 * j:64 * hl + 32 * (j + 1), :,
                                win])

        qtkt = {}

        def get_qtkt(p):
            if p not in qtkt:
                qtkt[p] = (
                    qtkt_pool.tile([P, 2, T], F8, name=f"qka{p}", tag="qka",
                                   bufs=2),
                    qtkt_pool.tile([64, 2, 2, T], F8, name=f"qk8{p}",
                                   tag="qk8", bufs=2),
                )
            return qtkt[p]

        for tq in range(NQT):
            qk_proj(0, tq)
        qk_regroup(0, 0)
        qk_regroup(0, 1)

        for p in range(NPAIR):
            _, qk8_sb = get_qtkt(p)
            o_sb = osb_pool.tile([P, T], F16, name=f"o{p}", tag="o")

            def s_unit(qt_i, hl):
                """S^T matmuls + exp + causal mask for one (q-tile, head)."""
                dlo, dhi = 32 * hl, 32 * (hl + 1)
                nkt = 4 * qt_i + 4
                pts = []
                for k0 in range(0, nkt, 2):
                    smin = [min(max(0, (k0 + u) - 4 * qt_i), 2)
                            for u in range(2)]
                    off = [P * s for s in smin]
                    sps = s_ps.tile([P, 2 * QW], F32,
                                    name=f"s{p}{hl}{qt_i}{k0}", tag="s")
                    for u in range(2):
                        k = k0 + u
                        nc.tensor.matmul(
                            sps[:, QW * u + off[u]:QW * (u + 1)],
                            qk8_sb[dlo:dhi, :, 1, P * k:P * (k + 1)],
                            qk8_sb[dlo:dhi, :, 0,
                                   QW * qt_i + off[u]:QW * (qt_i + 1)],
                            start=True, stop=True, perf_mode=DR)
                    pt = pt_pool.tile([P, 2 * QW], F16,
                                      name=f"pt{p}{hl}{qt_i}{k0}", tag="pt")
                    # exp in a single instruction per chunk: for unequal
                    # offsets, exp the union region (extra columns read stale
                    # psum; their pt slots are never consumed downstream).
                    # A fraction of full-width chunks runs on DVE via the
                    # Schraudolph bit-trick to unload the ScalarE bottleneck.
                    eoff = min(off)
                    schr = eoff == 0 and k0 % 8 == 2
                    if schr:
                        nc.vector.tensor_scalar(
                            out=pt.bitcast(I16), in0=sps,
                            scalar1=EXP_A, scalar2=EXP_B,
                            op0=mybir.AluOpType.mult,
                            op1=mybir.AluOpType.add)
                    elif eoff == 0:
                        nc.scalar.activation(
                            out=pt, in_=sps,
                            func=mybir.ActivationFunctionType.Exp,
                            scale=SCALE / 16.0)
                    else:
                        view = lambda ap: ap.rearrange(
                            "p (u c) -> p u c", u=2)[:, :, eoff:QW]
                        nc.scalar.activation(
                            out=view(pt), in_=view(sps),
                            func=mybir.ActivationFunctionType.Exp,
                            scale=SCALE / 16.0)
                    for u in range(2):
                        k = k0 + u
                        for s in range(4):
                            gs = 4 * qt_i + s
                            if gs == k:
                                sl = pt[:, QW * u + P * s:QW * u + P * (s + 1)]
                                nc.vector.tensor_mul(sl, sl, mask_sb)
                    pts.append(pt)
                return pts

            def pv_unit(qt_i, hl, pts):
                """P^T @ V_aug + normalize into o_sb for one unit."""
                hh = 2 * p + hl
                ops_ = o_ps.tile([P, 4 * (HD + 1)], F32,
                                 name=f"o{p}{hl}{qt_i}", tag="o")
                for s in range(4):
                    gs = 4 * qt_i + s
                    for k in range(gs + 1):
                        nc.tensor.matmul(
                            ops_[:, (HD + 1) * s:(HD + 1) * (s + 1)],
                            pts[k // 2][:, QW * (k % 2) + P * s:
                                        QW * (k % 2) + P * (s + 1)],
                            v_sb[k][:, hh, :],
                            start=(k == 0), stop=(k == gs))
                r_ = r_pool.tile([P, 4], F32, name=f"r{p}{hl}{qt_i}", tag="r")
                nc.vector.reciprocal(
                    r_, ops_.rearrange("p (s c) -> p s c", c=HD + 1)[:, :, HD])
                out_ap = o_sb[:, QW * qt_i:QW * (qt_i + 1)].rearrange(
                    "p (s h d) -> p s h d", s=4, h=2)[:, :, hl, :]
                nc.vector.tensor_mul(
                    out_ap,
                    ops_.rearrange("p (s c) -> p s c", c=HD + 1)[:, :, 0:HD],
                    r_.unsqueeze(2).broadcast_to((P, 4, HD)))

            # software pipeline: PV runs one unit behind S/exp. After each
            # q-window (hl == 1) completes: emit the NEXT pair's projection
            # matmuls for that window (PE filler for this ScalarE-bound
            # phase), then this window's transpose chain (DMA + GpSimd, no
            # PE). On the last pair the filler is the output projection,
            # lagged one window behind its transpose.
            last = p == NPAIR - 1
            y_ready = []

            def drain(pend):
                qt_i, hl, pts = pend
                pv_unit(qt_i, hl, pts)
                if hl == 0:
                    if not last:
                        qk_proj(p + 1, qt_i)
                        if qt_i == NQT - 1:
                            qk_regroup(p + 1, 1)
                else:
                    if not last and qt_i == 1:
                        qk_regroup(p + 1, 0)
                    transpose_tq(p, o_sb, qt_i)
                    if last:
                        if y_ready:
                            emit_y(y_ready.pop(0))
                        y_ready.append(qt_i)

            units = [(qt_i, hl) for qt_i in range(NQT) for hl in range(2)]
            pq = []
            for (qt_i, hl) in units:
                pts = s_unit(qt_i, hl)
                pq.append((qt_i, hl, pts))
                if len(pq) > 1:
                    drain(pq.pop(0))
            for pend in pq:
                drain(pend)
            for tq in y_ready:
                emit_y(tq)


def _comp8(a):
    hi = a.astype(NPF8)
    lo = (a - hi.astype(np.float32)).astype(NPF8)
    return hi, lo


def _prep_inputs(x, w_attn, b_attn, w_proj):
    """Per-core input maps."""
    in_maps = []
    # contraction layout [unit, p, j, ...]: c = 256*unit + 128*j + p
    def units(a, n_u):
        # a: [n_u*256, M] -> [n_u, 128, 2, M]
        return np.ascontiguousarray(
            a.reshape(n_u, 2, P, -1).transpose(0, 2, 1, 3))

    wq_h = {}
    for g in range(2):
        qs = slice(CPC * g, CPC * (g + 1))
        ks = slice(C + CPC * g, C + CPC * (g + 1))
        vs = slice(2 * C + CPC * g, 2 * C + CPC * (g + 1))
        wqh, wql = _comp8(64.0 * w_attn[:, qs])
        wkh, wkl = _comp8(64.0 * w_attn[:, ks])
        wvh, wvl = _comp8(64.0 * w_attn[:, vs])
        wph, wpl = _comp8(64.0 * w_proj[CPC * g:CPC * (g + 1), :])
        wq_h[g] = dict(
            wq8h=units(wqh, NU), wq8l=units(wql, NU),
            wk8h=units(wkh, NU), wk8l=units(wkl, NU),
            wv8h=units(wvh, NU), wv8l=units(wvl, NU),
            wp8h=units(wph, 2), wp8l=units(wpl, 2),
            qb4=np.ascontiguousarray(4.0 * b_attn[qs]),
            kb4=np.ascontiguousarray(4.0 * b_attn[ks]),
        )
    for c in range(NCORES):
        b = c // 2
        g = c % 2
        xT = np.ascontiguousarray(x[b].T)          # [C, T] fp32
        xh, xl = _comp8(xT)
        in_maps.append({
            "x8h": units(xh, NU),
            "x8l": units(xl, NU),
            **wq_h[g],
            "maskT": np.triu(np.ones((P, P), dtype=np.float16)),
        })
    return in_maps


_CACHED_NC = None


def kernel(x, w_attn, b_attn, w_proj, b_proj):
    global _CACHED_NC
    x = np.asarray(x, dtype=np.float32)
    w_attn = np.asarray(w_attn, dtype=np.float32)
    b_attn = np.asarray(b_attn, dtype=np.float32)
    w_proj = np.asarray(w_proj, dtype=np.float32)
    b_proj = np.asarray(b_proj, dtype=np.float32)

    if _CACHED_NC is None:
        _CACHED_NC = build_kernel(loop_n=1)
    nc = _CACHED_NC
    in_maps = _prep_inputs(x, w_attn, b_attn, w_proj)
    res = run_bass_kernel_spmd(nc, in_maps, core_ids=list(range(NCORES)),
                               trace=False)
    out = np.empty((B, T, C), dtype=np.float32)
    # exact row-vector bias contribution: rows of softmax sum to 1
    for b in range(B):
        out[b] = (res.results[2 * b]["y"].astype(np.float32)
                  + res.results[2 * b + 1]["y"].astype(np.float32))
    bias_row = b_attn[2 * C:3 * C] @ w_proj + b_proj
    out += bias_row[None, None, :]
    return out


# revision 80
# speedup vs baseline: 1.2680x; 1.0056x over previous
"""Causal multi-head attention block (QKV proj -> causal attention -> out proj)
for Trainium2, sharded over 8 NeuronCores.

Sharding: tensor/data hybrid. Core c handles batch b = c//2 and half the heads
(g = c%2, 8 of 16 heads). Per core:
  - QKV projection with error-compensated fp8e4m3 DoubleRow matmuls
    (x = x_hi + x_lo, w = w_hi + w_lo; compute hi*hi + hi*lo + lo*hi,
    each a 256-deep DoubleRow matmul). Weights scaled x64 on host so fp8
    stays in the normal range; rescaled on the PSUM evacuation.
  - causal attention in S^T = K @ Q^T layout: q/k quantized to fp8 at the
    PSUM evacuation, DMA-regrouped into [32p, 2] DoubleRow layout; S
    matmuls are fp8 DoubleRow. exp on ScalarE (a fraction on VectorE via
    the Schraudolph int16 bit-trick); P^T (fp16) @ V_aug (fp16, scaled
    ones column -> 0.25/rowsum for free from the DVE reciprocal)
  - DVE normalize, DMA-XBAR transpose of O + GpSimd fp8 hi/lo split,
    fp8-compensated output projection -> partial y [T, C] (fp16)
Host: y[b] = partial[2b] + partial[2b+1] (+ bias terms, see below).

Biases: b_attn Q/K slices are added on-device (fused into the PSUM->SBUF
copies). The V-bias and b_proj contributions are exact row vectors on the
output (rows of softmax sum to 1): y += (b_v @ w_proj + b_proj), added on
host during the unshard.

Scheduling: pair p's attention units (q-window, head) run S/exp one unit
ahead of PV; pair p+1's projections + regroup DMAs are emitted inside
pair p's attention as PE filler; the output projection pipelines into the
last pair's attention per q-window.

Scaling ledger (all powers of 2, exact):
  wq8/wk8/wv8 = 64*w (hi+lo fp8 pair)   -> q/k/v psum = 64*true
  qka = psum/16 + 4*qb = 4*true (fp8)
  S psum = 16*S_true; exp scale = HD^-0.5/16
  vt = 64*V (fp16), ones col = 4.0  -> recip gives 0.25/rowsum
  o_sb = (64*O')*(0.25/r) = 16*O (fp16)
  ot8 = fp8 pair of 16*O ; wp8 = 64*w_proj (hi+lo) -> y psum = 1024*y
  y = psum/1024 (fp16; summed in fp32 on host)
"""

import math

import numpy as np
import ml_dtypes

import concourse.bass as bass
import concourse.mybir as mybir
import concourse.tile as tile
from concourse import bacc
from concourse.bass_utils import run_bass_kernel_spmd

B, T, C = 4, 2048, 1024
NH, HD = 16, 64
NCORES = 8
HPC = NH // 2          # heads per core = 8
CPC = HPC * HD         # channels per core = 512
P = 128                # partitions
NT = T // P            # 16 t-tiles of 128
NU = C // 256          # 4 DoubleRow contraction units of 256
NPAIR = HPC // 2       # 4 head pairs
QW = 512               # q-tile width
NQT = T // QW          # 4 q-tiles

F32 = mybir.dt.float32
F16 = mybir.dt.float16
F8 = mybir.dt.float8e4
I16 = mybir.dt.int16
DR = mybir.MatmulPerfMode.DoubleRow
NPF8 = ml_dtypes.float8_e4m3
SCALE = HD ** -0.5
# Schraudolph fp16 exp approximation on DVE for a fraction of the softmax:
# exp(s) ~ bitcast_fp16(int16(s*EXP_A + EXP_B)). EXP_B tuned for min rel err
# assuming round-to-nearest int conversion.
EXP_A = (SCALE / 16.0) * 1.4426950408889634 * 1024.0
EXP_B = 15.0 * 1024.0 - 38.5


def build_kernel(loop_n: int = 1):
    nc = bacc.Bacc("TRN2", target_bir_lowering=False, debug=False)
    x8h = nc.dram_tensor("x8h", [NU, P, 2, T], F8, kind="ExternalInput").ap()
    x8l = nc.dram_tensor("x8l", [NU, P, 2, T], F8, kind="ExternalInput").ap()
    wq8h = nc.dram_tensor("wq8h", [NU, P, 2, CPC], F8, kind="ExternalInput").ap()
    wq8l = nc.dram_tensor("wq8l", [NU, P, 2, CPC], F8, kind="ExternalInput").ap()
    wk8h = nc.dram_tensor("wk8h", [NU, P, 2, CPC], F8, kind="ExternalInput").ap()
    wk8l = nc.dram_tensor("wk8l", [NU, P, 2, CPC], F8, kind="ExternalInput").ap()
    wv8h = nc.dram_tensor("wv8h", [NU, P, 2, CPC], F8, kind="ExternalInput").ap()
    wv8l = nc.dram_tensor("wv8l", [NU, P, 2, CPC], F8, kind="ExternalInput").ap()
    wp8h = nc.dram_tensor("wp8h", [2, P, 2, C], F8, kind="ExternalInput").ap()
    wp8l = nc.dram_tensor("wp8l", [2, P, 2, C], F8, kind="ExternalInput").ap()
    qb4 = nc.dram_tensor("qb4", [CPC], F32, kind="ExternalInput").ap()
    kb4 = nc.dram_tensor("kb4", [CPC], F32, kind="ExternalInput").ap()
    maskT = nc.dram_tensor("maskT", [P, P], F16, kind="ExternalInput").ap()
    y = nc.dram_tensor("y", [T, C], F16, kind="ExternalOutput").ap()

    args = (x8h, x8l, wq8h, wq8l, wk8h, wk8l, wv8h, wv8l, wp8h, wp8l,
            qb4, kb4, maskT, y)
    with tile.TileContext(nc) as tc:
        if loop_n == 1:
            _body(tc, nc, *args)
        else:
            with tc.For_i(0, loop_n, 1):
                _body(tc, nc, *args)
    nc.compile()
    return nc


def _body(tc, nc, x8h, x8l, wq8h, wq8l, wk8h, wk8l, wv8h, wv8l,
          wp8h, wp8l, qb4, kb4, maskT, y):
    from contextlib import ExitStack

    ctx = ExitStack()
    with ctx:
        const = ctx.enter_context(tc.tile_pool(name="const", bufs=1))
        x_pool = ctx.enter_context(tc.tile_pool(name="xp", bufs=2 * NU))
        w8_pool = ctx.enter_context(tc.tile_pool(name="w8p", bufs=1))
        v_pool = ctx.enter_context(tc.tile_pool(name="vp", bufs=NT)) if False else ctx.enter_context(tc.tile_pool(name="vp", bufs=NT))
        qtkt_pool = ctx.enter_context(tc.tile_pool(name="qtkt", bufs=3))
        bias_pool = ctx.enter_context(tc.tile_pool(name="biasp", bufs=2))
        pt_pool = ctx.enter_context(tc.tile_pool(name="ptp", bufs=21))
        osb_pool = ctx.enter_context(tc.tile_pool(name="osb", bufs=4))
        ot_pool = ctx.enter_context(tc.tile_pool(name="otp", bufs=4))
        r_pool = ctx.enter_context(tc.tile_pool(name="rp", bufs=6))
        y_pool = ctx.enter_context(tc.tile_pool(name="yp", bufs=4))
        mm_ps = ctx.enter_context(tc.tile_pool(name="mmps", bufs=2, space="PSUM"))
        s_ps = ctx.enter_context(tc.tile_pool(name="sps", bufs=2, space="PSUM"))
        o_ps = ctx.enter_context(tc.tile_pool(name="ops", bufs=2, space="PSUM"))

        # ---- weight + x loads (fp8 hi/lo pairs) ----
        # order matters for the startup critical path: wv + x first (phase V
        # needs them, unit-interleaved so the first V matmuls start early),
        # wq/wk next, wp last. Two DGE queues (SP + ACT) in parallel.
        wv_sb, xh_sb, xl_sb = [], [], []
        for tag, dr_ in (("wvh", wv8h), ("wvl", wv8l)):
            t_ = w8_pool.tile([P, NU, 2, CPC], F8, name=tag, tag=tag)
            wv_sb.append(t_)
        for i in range(NU):
            xh_sb.append(x_pool.tile([P, 2, T], F8, name=f"xh{i}", tag="x"))
            xl_sb.append(x_pool.tile([P, 2, T], F8, name=f"xl{i}", tag="x"))
        for i in range(NU):
            nc.sync.dma_start(out=wv_sb[0][:, i], in_=wv8h[i])
            nc.scalar.dma_start(out=wv_sb[1][:, i], in_=wv8l[i])
            sl = slice(0, T // 4)
            nc.sync.dma_start(out=xh_sb[i][:, :, sl], in_=x8h[i][:, :, sl])
            nc.scalar.dma_start(out=xl_sb[i][:, :, sl], in_=x8l[i][:, :, sl])
        mask_sb = const.tile([P, P], F16, tag="mask")
        nc.sync.dma_start(out=mask_sb, in_=maskT)
        for half in range(2):
            sl = slice(T // 4 + T * 3 // 8 * half,
                       T // 4 + T * 3 // 8 * (half + 1))
            for i in range(NU):
                nc.sync.dma_start(out=xh_sb[i][:, :, sl], in_=x8h[i][:, :, sl])
                nc.scalar.dma_start(out=xl_sb[i][:, :, sl], in_=x8l[i][:, :, sl])
        wq_sb, wk_sb = [], []
        for nm, drh, drl, lst in (("wq", wq8h, wq8l, wq_sb),
                                  ("wk", wk8h, wk8l, wk_sb)):
            for tag, dr_ in ((f"{nm}h", drh), (f"{nm}l", drl)):
                t_ = w8_pool.tile([P, NU, 2, CPC], F8, name=tag, tag=tag)
                eng = nc.sync if nm == "wq" else nc.scalar
                eng.dma_start(out=t_, in_=dr_.rearrange("i p j m -> p i j m"))
                lst.append(t_)
        qb_all = bias_pool.tile([P, NPAIR], F32, tag="qb", bufs=1)
        kb_all = bias_pool.tile([P, NPAIR], F32, tag="kb", bufs=1)
        nc.sync.dma_start(out=qb_all, in_=qb4.rearrange("(a p) -> p a", p=P))
        nc.sync.dma_start(out=kb_all, in_=kb4.rearrange("(a p) -> p a", p=P))

        def comp_mms(ps, lhs_hl, rhs_hl, lslice, rslice):
            """hi*hi + lo*hi + hi*lo DoubleRow accumulation over NU units."""
            terms = [(0, 0), (1, 0), (0, 1)]
            n = NU * len(terms) - 1
            cnt = 0
            for i in range(NU):
                for (a, b_) in terms:
                    nc.tensor.matmul(
                        ps, lslice(lhs_hl[a], i), rslice(rhs_hl[b_], i),
                        start=(cnt == 0), stop=(cnt == n), perf_mode=DR)
                    cnt += 1

        # ---- fp8 wp tiles for the output projection ----
        wp_sb = []
        for tag, dr_ in (("wph", wp8h), ("wpl", wp8l)):
            t_ = w8_pool.tile([P, 2, 2, C], F8, name=tag, tag=tag)
            for g in range(2):
                nc.sync.dma_start(out=t_[:, g], in_=dr_[g])
            wp_sb.append(t_)
        ot8h, ot8l, ot16 = [], [], []
        for g in range(2):
            ot8h.append(ot_pool.tile([P, 2, T], F8, name=f"oth{g}", tag="ot8"))
            ot8l.append(ot_pool.tile([P, 2, T], F8, name=f"otl{g}", tag="ot8"))
            ot16.append(ot_pool.tile([P, 2, T], F16, name=f"ot16{g}",
                                     tag="ot16", bufs=2))

        # ---- per head-pair: O^T via DMA-XBAR transpose + GpSimd fp8 split --
        def transpose_tq(p, o_sb, tq):
            """Blocked transpose of one o_sb q-window into ot8 hi/lo fp8."""
            g, j = p // 2, p % 2
            win = slice(QW * tq, QW * (tq + 1))
            out3 = ot16[g][:, j, win].rearrange("p (b c) -> p b c", b=4)
            nc.sync.dma_start(out=out3, in_=o_sb[:, win], transpose=True)
            nc.gpsimd.tensor_copy(out=ot8h[g][:, j, win], in_=ot16[g][:, j, win])
            nc.gpsimd.tensor_sub(ot8l[g][:, j, win], ot16[g][:, j, win],
                                 ot8h[g][:, j, win])

        def emit_y(tq):
            """Output-projection matmuls for the 4 t-tiles of one tq window."""
            for t in range(4 * tq, 4 * tq + 4):
                ysb = y_pool.tile([P, C], F16, name=f"y{t}", tag="y")
                for n2 in range(2):
                    yps = mm_ps.tile([P, QW], F32, name=f"yps{t}{n2}", tag="mm")
                    cnt = 0
                    for g in range(2):
                        for (osrc, wsrc) in ((ot8h[g], wp_sb[0]),
                                             (ot8h[g], wp_sb[1]),
                                             (ot8l[g], wp_sb[0])):
                            nc.tensor.matmul(
                                yps, osrc[:, :, P * t:P * (t + 1)],
                                wsrc[:, g, :, QW * n2:QW * (n2 + 1)],
                                start=(cnt == 0), stop=(cnt == 5), perf_mode=DR)
                            cnt += 1
                    nc.vector.tensor_scalar(
                        out=ysb[:, QW * n2:QW * (n2 + 1)], in0=yps,
                        scalar1=1.0 / 1024.0, scalar2=None,
                        op0=mybir.AluOpType.mult)
                nc.sync.dma_start(out=y[P * t:P * (t + 1), :], in_=ysb)

        def qk_proj(p, tq):
            """Q/K projection matmuls + fp8 PSUM evac for one t-window."""
            qka, _ = get_qtkt(p)
            for qk, (wsb, bsb) in enumerate(
                    ((wq_sb, qb_all[:, p:p + 1]),
                     (wk_sb, kb_all[:, p:p + 1]))):
                ps = mm_ps.tile([P, QW], F32, name=f"qk{p}{tq}", tag="mm")
                comp_mms(
                    ps, (wsb[0], wsb[1]), (xh_sb, xl_sb),
                    lambda w, i: w[:, i, :, P * p:P * (p + 1)],
                    lambda xs, i: xs[i][:, :, QW * tq:QW * (tq + 1)])
                nc.vector.tensor_scalar(
                    out=qka[:, qk, QW * tq:QW * (tq + 1)], in0=ps,
                    scalar1=1.0 / 16.0, scalar2=bsb,
                    op0=mybir.AluOpType.mult, op1=mybir.AluOpType.add)

        def qk_regroup(p, half):
            """[128, 2, T] channel-major fp8 -> [32(hl base), 2(j), 2(qk), T]
            DoubleRow layout: channel d = 32j + i at partition i, free j.
            One DMA per (hl, j) block moves both Q and K."""
            qka, qk8 = get_qtkt(p)
            win = slice(T // 2 * half, T // 2 * (half + 1))
            for hl in range(2):
                for j in range(2):
                    nc.sync.dma_start(
                        out=qk8[32 * hl:32 * (hl + 1), j, :, win],
                        in_=qka[64 * hl + 32 * j:64 * hl + 32 * (j + 1), :,
                                win])

        qtkt = {}

        def get_qtkt(p):
            if p not in qtkt:
                qtkt[p] = (
                    qtkt_pool.tile([P, 2, T], F8, name=f"qka{p}", tag="qka",
                                   bufs=2),
                    qtkt_pool.tile([64, 2, 2, T], F8, name=f"qk8{p}",
                                   tag="qk8", bufs=2),
                )
            return qtkt[p]

        # ---- phase V: V for all 8 heads, pair-0 Q/K interleaved per
        # quarter (its x window arrives with the same DMA quarter, so PE has
        # runnable work as soon as each quarter lands) ----
        v_sb = []
        for t in range(NT):
            vps = mm_ps.tile([P, CPC], F32, name=f"vps{t}", tag="mm")
            comp_mms(
                vps, (xh_sb, xl_sb), (wv_sb[0], wv_sb[1]),
                lambda xs, i: xs[i][:, :, P * t:P * (t + 1)],
                lambda w, i: w[:, i])
            vt = v_pool.tile([P, HPC, HD + 1], F16, name=f"v{t}", tag="v")
            nc.vector.memset(vt[:, :, HD], 4.0)
            nc.vector.tensor_copy(
                out=vt[:, :, 0:HD],
                in_=vps.rearrange("p (h d) -> p h d", h=HPC))
            v_sb.append(vt)

        for tq in range(NQT):
            qk_proj(0, tq)
        qk_regroup(0, 0)
        qk_regroup(0, 1)

        for p in range(NPAIR):
            _, qk8_sb = get_qtkt(p)
            o_sb = osb_pool.tile([P, T], F16, name=f"o{p}", tag="o")

            def s_unit(qt_i, hl):
                """S^T matmuls + exp + causal mask for one (q-tile, head)."""
                dlo, dhi = 32 * hl, 32 * (hl + 1)
                nkt = 4 * qt_i + 4
                pts = []
                for k0 in range(0, nkt, 2):
                    smin = [min(max(0, (k0 + u) - 4 * qt_i), 2)
                            for u in range(2)]
                    off = [P * s for s in smin]
                    sps = s_ps.tile([P, 2 * QW], F32,
                                    name=f"s{p}{hl}{qt_i}{k0}", tag="s")
                    for u in range(2):
                        k = k0 + u
                        nc.tensor.matmul(
                            sps[:, QW * u + off[u]:QW * (u + 1)],
                            qk8_sb[dlo:dhi, :, 1, P * k:P * (k + 1)],
                            qk8_sb[dlo:dhi, :, 0,
                                   QW * qt_i + off[u]:QW * (qt_i + 1)],
                            start=True, stop=True, perf_mode=DR)
                    pt = pt_pool.tile([P, 2 * QW], F16,
                                      name=f"pt{p}{hl}{qt_i}{k0}", tag="pt")
                    # exp in a single instruction per chunk: for unequal
                    # offsets, exp the union region (extra columns read stale
                    # psum; their pt slots are never consumed downstream).
                    # A fraction of full-width chunks runs on DVE via the
                    # Schraudolph bit-trick to unload the ScalarE bottleneck.
                    eoff = min(off)
                    schr = eoff == 0 and k0 % 8 == 2
                    if schr:
                        nc.vector.tensor_scalar(
                            out=pt.bitcast(I16), in0=sps,
                            scalar1=EXP_A, scalar2=EXP_B,
                            op0=mybir.AluOpType.mult,
                            op1=mybir.AluOpType.add)
                    elif eoff == 0:
                        nc.scalar.activation(
                            out=pt, in_=sps,
                            func=mybir.ActivationFunctionType.Exp,
                            scale=SCALE / 16.0)
                    else:
                        view = lambda ap: ap.rearrange(
                            "p (u c) -> p u c", u=2)[:, :, eoff:QW]
                        nc.scalar.activation(
                            out=view(pt), in_=view(sps),
                            func=mybir.ActivationFunctionType.Exp,
                            scale=SCALE / 16.0)
                    for u in range(2):
                        k = k0 + u
                        for s in range(4):
                            gs = 4 * qt_i + s
                            if gs == k:
                                sl = pt[:, QW * u + P * s:QW * u + P * (s + 1)]
                                nc.vector.tensor_mul(sl, sl, mask_sb)
                    pts.append(pt)
                return pts

            def pv_unit(qt_i, hl, pts):
                """P^T @ V_aug + normalize into o_sb for one unit."""
                hh = 2 * p + hl
                ops_ = o_ps.tile([P, 4 * (HD + 1)], F32,
                                 name=f"o{p}{hl}{qt_i}", tag="o")
                for s in range(4):
                    gs = 4 * qt_i + s
                    for k in range(gs + 1):
                        nc.tensor.matmul(
                            ops_[:, (HD + 1) * s:(HD + 1) * (s + 1)],
                            pts[k // 2][:, QW * (k % 2) + P * s:
                                        QW * (k % 2) + P * (s + 1)],
                            v_sb[k][:, hh, :],
                            start=(k == 0), stop=(k == gs))
                r_ = r_pool.tile([P, 4], F32, name=f"r{p}{hl}{qt_i}", tag="r")
                nc.vector.reciprocal(
                    r_, ops_.rearrange("p (s c) -> p s c", c=HD + 1)[:, :, HD])
                out_ap = o_sb[:, QW * qt_i:QW * (qt_i + 1)].rearrange(
                    "p (s h d) -> p s h d", s=4, h=2)[:, :, hl, :]
                nc.vector.tensor_mul(
                    out_ap,
                    ops_.rearrange("p (s c) -> p s c", c=HD + 1)[:, :, 0:HD],
                    r_.unsqueeze(2).broadcast_to((P, 4, HD)))

            # software pipeline: PV runs one unit behind S/exp. After each
            # q-window (hl == 1) completes: emit the NEXT pair's projection
            # matmuls for that window (PE filler for this ScalarE-bound
            # phase), then this window's transpose chain (DMA + GpSimd, no
            # PE). On the last pair the filler is the output projection,
            # lagged one window behind its transpose.
            last = p == NPAIR - 1
            y_ready = []

            def drain(pend):
                qt_i, hl, pts = pend
                pv_unit(qt_i, hl, pts)
                if hl == 0:
                    if not last:
                        qk_proj(p + 1, qt_i)
                        if qt_i == NQT - 1:
                            qk_regroup(p + 1, 1)
                else:
                    if not last and qt_i == 1:
                        qk_regroup(p + 1, 0)
                    transpose_tq(p, o_sb, qt_i)
                    if last:
                        if y_ready:
                            emit_y(y_ready.pop(0))
                        y_ready.append(qt_i)

            units = [(qt_i, hl) for qt_i in range(NQT) for hl in range(2)]
            pq = []
            for (qt_i, hl) in units:
                pts = s_unit(qt_i, hl)
                pq.append((qt_i, hl, pts))
                if len(pq) > 1:
                    drain(pq.pop(0))
            for pend in pq:
                drain(pend)
            for tq in y_ready:
                emit_y(tq)


def _comp8(a):
    hi = a.astype(NPF8)
    lo = (a - hi.astype(np.float32)).astype(NPF8)
    return hi, lo


def _prep_inputs(x, w_attn, b_attn, w_proj):
    """Per-core input maps."""
    in_maps = []
    # contraction layout [unit, p, j, ...]: c = 256*unit + 128*j + p
    def units(a, n_u):
        # a: [n_u*256, M] -> [n_u, 128, 2, M]
        return np.ascontiguousarray(
            a.reshape(n_u, 2, P, -1).transpose(0, 2, 1, 3))

    wq_h = {}
    for g in range(2):
        qs = slice(CPC * g, CPC * (g + 1))
        ks = slice(C + CPC * g, C + CPC * (g + 1))
        vs = slice(2 * C + CPC * g, 2 * C + CPC * (g + 1))
        wqh, wql = _comp8(64.0 * w_attn[:, qs])
        wkh, wkl = _comp8(64.0 * w_attn[:, ks])
        wvh, wvl = _comp8(64.0 * w_attn[:, vs])
        wph, wpl = _comp8(64.0 * w_proj[CPC * g:CPC * (g + 1), :])
        wq_h[g] = dict(
            wq8h=units(wqh, NU), wq8l=units(wql, NU),
            wk8h=units(wkh, NU), wk8l=units(wkl, NU),
            wv8h=units(wvh, NU), wv8l=units(wvl, NU),
            wp8h=units(wph, 2), wp8l=units(wpl, 2),
            qb4=np.ascontiguousarray(4.0 * b_attn[qs]),
            kb4=np.ascontiguousarray(4.0 * b_attn[ks]),
        )
    for c in range(NCORES):
        b = c // 2
        g = c % 2
        xT = np.ascontiguousarray(x[b].T)          # [C, T] fp32
        xh, xl = _comp8(xT)
        in_maps.append({
            "x8h": units(xh, NU),
            "x8l": units(xl, NU),
            **wq_h[g],
            "maskT": np.triu(np.ones((P, P), dtype=np.float16)),
        })
    return in_maps


_CACHED_NC = None


def kernel(x, w_attn, b_attn, w_proj, b_proj):
    global _CACHED_NC
    x = np.asarray(x, dtype=np.float32)
    w_attn = np.asarray(w_attn, dtype=np.float32)
    b_attn = np.asarray(b_attn, dtype=np.float32)
    w_proj = np.asarray(w_proj, dtype=np.float32)
    b_proj = np.asarray(b_proj, dtype=np.float32)

    if _CACHED_NC is None:
        _CACHED_NC = build_kernel(loop_n=1)
    nc = _CACHED_NC
    in_maps = _prep_inputs(x, w_attn, b_attn, w_proj)
    res = run_bass_kernel_spmd(nc, in_maps, core_ids=list(range(NCORES)),
                               trace=False)
    out = np.empty((B, T, C), dtype=np.float32)
    # exact row-vector bias contribution: rows of softmax sum to 1
    for b in range(B):
        out[b] = (res.results[2 * b]["y"].astype(np.float32)
                  + res.results[2 * b + 1]["y"].astype(np.float32))
    bias_row = b_attn[2 * C:3 * C] @ w_proj + b_proj
    out += bias_row[None, None, :]
    return out


# revision 97
# speedup vs baseline: 1.3035x; 1.0280x over previous
"""Causal multi-head attention block (QKV proj -> causal attention -> out proj)
for Trainium2, sharded over 8 NeuronCores.

Sharding: tensor/data hybrid. Core c handles batch b = c//2 and half the heads
(g = c%2, 8 of 16 heads). Per core:
  - QKV projection with error-compensated fp8e4m3 DoubleRow matmuls
    (x = x_hi + x_lo, w = w_hi + w_lo; compute hi*hi + hi*lo + lo*hi,
    each a 256-deep DoubleRow matmul). Weights scaled x64 on host so fp8
    stays in the normal range; rescaled on the PSUM evacuation.
  - causal attention in S^T = K @ Q^T layout: q/k quantized to fp8 at the
    PSUM evacuation, DMA-regrouped into [32p, 2] DoubleRow layout; S
    matmuls are fp8 DoubleRow. exp on ScalarE (a fraction on VectorE via
    the Schraudolph int16 bit-trick); P^T (fp16) @ V_aug (fp16, scaled
    ones column -> 0.25/rowsum for free from the DVE reciprocal)
  - DVE normalize, DMA-XBAR transpose of O + GpSimd fp8 hi/lo split,
    fp8-compensated output projection -> partial y [T, C] (fp16)
Host: y[b] = partial[2b] + partial[2b+1] (+ bias terms, see below).

Biases: b_attn Q/K slices are added on-device (fused into the PSUM->SBUF
copies). The V-bias and b_proj contributions are exact row vectors on the
output (rows of softmax sum to 1): y += (b_v @ w_proj + b_proj), added on
host during the unshard.

Scheduling: pair p's attention units (q-window, head) run S/exp one unit
ahead of PV; pair p+1's projections + regroup DMAs are emitted inside
pair p's attention as PE filler; the output projection pipelines into the
last pair's attention per q-window.

Scaling ledger (all powers of 2, exact):
  wq8/wk8/wv8 = 64*w (hi+lo fp8 pair)   -> q/k/v psum = 64*true
  qka = psum/16 + 4*qb = 4*true (fp8)
  S psum = 16*S_true; exp scale = HD^-0.5/16
  vt = 64*V (fp16), ones col = 4.0  -> recip gives 0.25/rowsum
  o_sb = (64*O')*(0.25/r) = 16*O (fp16)
  ot8 = fp8 pair of 16*O ; wp8 = 64*w_proj (hi+lo) -> y psum = 1024*y
  y = psum/1024 (fp16; summed in fp32 on host)
"""

import math

import numpy as np
import ml_dtypes

import concourse.bass as bass
import concourse.mybir as mybir
import concourse.tile as tile
from concourse import bacc
from concourse.bass_utils import run_bass_kernel_spmd

B, T, C = 4, 2048, 1024
NH, HD = 16, 64
NCORES = 8
HPC = NH // 2          # heads per core = 8
CPC = HPC * HD         # channels per core = 512
P = 128                # partitions
NT = T // P            # 16 t-tiles of 128
NU = C // 256          # 4 DoubleRow contraction units of 256
NPAIR = HPC // 2       # 4 head pairs
QW = 512               # q-tile width
NQT = T // QW          # 4 q-tiles

F32 = mybir.dt.float32
F16 = mybir.dt.float16
F8 = mybir.dt.float8e4
I16 = mybir.dt.int16
DR = mybir.MatmulPerfMode.DoubleRow
NPF8 = ml_dtypes.float8_e4m3
SCALE = HD ** -0.5
# Schraudolph fp16 exp approximation on DVE for a fraction of the softmax:
# exp(s) ~ bitcast_fp16(int16(s*EXP_A + EXP_B)). EXP_B tuned for min rel err
# assuming round-to-nearest int conversion.
EXP_A = (SCALE / 16.0) * 1.4426950408889634 * 1024.0
EXP_B = 15.0 * 1024.0 - 38.5


def build_kernel(loop_n: int = 1):
    nc = bacc.Bacc("TRN2", target_bir_lowering=False, debug=False)
    x8c = nc.dram_tensor("x8c", [NU, P, 2, 2, T], F8, kind="ExternalInput").ap()
    wq8h = nc.dram_tensor("wq8h", [NU, P, 2, CPC], F8, kind="ExternalInput").ap()
    wq8l = nc.dram_tensor("wq8l", [NU, P, 2, CPC], F8, kind="ExternalInput").ap()
    wk8h = nc.dram_tensor("wk8h", [NU, P, 2, CPC], F8, kind="ExternalInput").ap()
    wk8l = nc.dram_tensor("wk8l", [NU, P, 2, CPC], F8, kind="ExternalInput").ap()
    wv8c = nc.dram_tensor("wv8c", [NU, P, 2, 2, CPC], F8,
                          kind="ExternalInput").ap()
    wp8h = nc.dram_tensor("wp8h", [2, P, 2, C], F8, kind="ExternalInput").ap()
    wp8l = nc.dram_tensor("wp8l", [2, P, 2, C], F8, kind="ExternalInput").ap()
    qb4 = nc.dram_tensor("qb4", [CPC], F32, kind="ExternalInput").ap()
    kb4 = nc.dram_tensor("kb4", [CPC], F32, kind="ExternalInput").ap()
    maskT = nc.dram_tensor("maskT", [P, P], F16, kind="ExternalInput").ap()
    y = nc.dram_tensor("y", [T, C], F16, kind="ExternalOutput").ap()

    args = (x8c, wq8h, wq8l, wk8h, wk8l, wv8c, wp8h, wp8l,
            qb4, kb4, maskT, y)
    with tile.TileContext(nc) as tc:
        if loop_n == 1:
            _body(tc, nc, *args)
        else:
            with tc.For_i(0, loop_n, 1):
                _body(tc, nc, *args)
    nc.compile()
    return nc


def _body(tc, nc, x8c, wq8h, wq8l, wk8h, wk8l, wv8c,
          wp8h, wp8l, qb4, kb4, maskT, y):
    from contextlib import ExitStack

    ctx = ExitStack()
    with ctx:
        const = ctx.enter_context(tc.tile_pool(name="const", bufs=1))
        x_pool = ctx.enter_context(tc.tile_pool(name="xp", bufs=NU))
        w8_pool = ctx.enter_context(tc.tile_pool(name="w8p", bufs=1))
        v_pool = ctx.enter_context(tc.tile_pool(name="vp", bufs=NT)) if False else ctx.enter_context(tc.tile_pool(name="vp", bufs=NT))
        qtkt_pool = ctx.enter_context(tc.tile_pool(name="qtkt", bufs=3))
        bias_pool = ctx.enter_context(tc.tile_pool(name="biasp", bufs=2))
        pt_pool = ctx.enter_context(tc.tile_pool(name="ptp", bufs=21))
        osb_pool = ctx.enter_context(tc.tile_pool(name="osb", bufs=4))
        ot_pool = ctx.enter_context(tc.tile_pool(name="otp", bufs=4))
        r_pool = ctx.enter_context(tc.tile_pool(name="rp", bufs=6))
        y_pool = ctx.enter_context(tc.tile_pool(name="yp", bufs=4))
        mm_ps = ctx.enter_context(tc.tile_pool(name="mmps", bufs=2, space="PSUM"))
        s_ps = ctx.enter_context(tc.tile_pool(name="sps", bufs=2, space="PSUM"))
        o_ps = ctx.enter_context(tc.tile_pool(name="ops", bufs=2, space="PSUM"))

        # ---- weight + x loads (fp8 hi/lo pairs) ----
        # order matters for the startup critical path: wv + x first (phase V
        # needs them, unit-interleaved so the first V matmuls start early),
        # wq/wk next, wp last. Two DGE queues (SP + ACT) in parallel.
        xh_sb, xl_sb = [], []
        wvc = w8_pool.tile([P, NU, 2, 2, CPC], F8, name="wvc", tag="wvc")
        wv_sb = [wvc[:, :, 0], wvc[:, :, 1]]
        xc_sb = []
        for i in range(NU):
            xc = x_pool.tile([P, 2, 2, T], F8, name=f"xc{i}", tag="x")
            xc_sb.append(xc)
            xh_sb.append(xc[:, 0])
            xl_sb.append(xc[:, 1])
        for i in range(NU):
            eng, eng2 = ((nc.sync, nc.scalar) if i % 2 == 0
                         else (nc.scalar, nc.sync))
            eng2.dma_start(out=wvc[:, i], in_=wv8c[i])
            sl = slice(0, T // 4)
            eng.dma_start(out=xc_sb[i][:, :, :, sl], in_=x8c[i][:, :, :, sl])
        mask_sb = const.tile([P, P], F16, tag="mask")
        nc.sync.dma_start(out=mask_sb, in_=maskT)
        for half in range(2):
            sl = slice(T // 4 + T * 3 // 8 * half,
                       T // 4 + T * 3 // 8 * (half + 1))
            for i in range(NU):
                eng = nc.sync if i % 2 == 0 else nc.scalar
                eng.dma_start(out=xc_sb[i][:, :, :, sl],
                              in_=x8c[i][:, :, :, sl])
        wq_sb, wk_sb = [], []
        for nm, drh, drl, lst in (("wq", wq8h, wq8l, wq_sb),
                                  ("wk", wk8h, wk8l, wk_sb)):
            for tag, dr_ in ((f"{nm}h", drh), (f"{nm}l", drl)):
                t_ = w8_pool.tile([P, NU, 2, CPC], F8, name=tag, tag=tag)
                eng = nc.sync if nm == "wq" else nc.scalar
                eng.dma_start(out=t_, in_=dr_.rearrange("i p j m -> p i j m"))
                lst.append(t_)
        qb_all = bias_pool.tile([P, NPAIR], F32, tag="qb", bufs=1)
        kb_all = bias_pool.tile([P, NPAIR], F32, tag="kb", bufs=1)
        nc.sync.dma_start(out=qb_all, in_=qb4.rearrange("(a p) -> p a", p=P))
        nc.sync.dma_start(out=kb_all, in_=kb4.rearrange("(a p) -> p a", p=P))

        def comp_mms(ps, lhs_hl, rhs_hl, lslice, rslice):
            """hi*hi + lo*hi + hi*lo DoubleRow accumulation over NU units."""
            terms = [(0, 0), (1, 0), (0, 1)]
            n = NU * len(terms) - 1
            cnt = 0
            for i in range(NU):
                for (a, b_) in terms:
                    nc.tensor.matmul(
                        ps, lslice(lhs_hl[a], i), rslice(rhs_hl[b_], i),
                        start=(cnt == 0), stop=(cnt == n), perf_mode=DR)
                    cnt += 1

        # ---- fp8 wp tiles for the output projection ----
        wp_sb = []
        for tag, dr_ in (("wph", wp8h), ("wpl", wp8l)):
            t_ = w8_pool.tile([P, 2, 2, C], F8, name=tag, tag=tag)
            for g in range(2):
                nc.sync.dma_start(out=t_[:, g], in_=dr_[g])
            wp_sb.append(t_)
        ot8h, ot8l, ot16 = [], [], []
        for g in range(2):
            ot8h.append(ot_pool.tile([P, 2, T], F8, name=f"oth{g}", tag="ot8"))
            ot8l.append(ot_pool.tile([P, 2, T], F8, name=f"otl{g}", tag="ot8"))
            ot16.append(ot_pool.tile([P, 2, T], F16, name=f"ot16{g}",
                                     tag="ot16", bufs=2))

        # ---- per head-pair: O^T via DMA-XBAR transpose + GpSimd fp8 split --
        def transpose_tq(p, o_sb, tq):
            """Blocked transpose of one o_sb q-window into ot8 hi/lo fp8."""
            g, j = p // 2, p % 2
            win = slice(QW * tq, QW * (tq + 1))
            out3 = ot16[g][:, j, win].rearrange("p (b c) -> p b c", b=4)
            nc.sync.dma_start(out=out3, in_=o_sb[:, win], transpose=True)
            nc.gpsimd.tensor_copy(out=ot8h[g][:, j, win], in_=ot16[g][:, j, win])
            nc.gpsimd.tensor_sub(ot8l[g][:, j, win], ot16[g][:, j, win],
                                 ot8h[g][:, j, win])

        def emit_y(tq):
            """Output-projection matmuls for the 4 t-tiles of one tq window."""
            for t in range(4 * tq, 4 * tq + 4):
                ysb = y_pool.tile([P, C], F16, name=f"y{t}", tag="y")
                for n2 in range(2):
                    yps = mm_ps.tile([P, QW], F32, name=f"yps{t}{n2}", tag="mm")
                    cnt = 0
                    for g in range(2):
                        for (osrc, wsrc) in ((ot8h[g], wp_sb[0]),
                                             (ot8h[g], wp_sb[1]),
                                             (ot8l[g], wp_sb[0])):
                            nc.tensor.matmul(
                                yps, osrc[:, :, P * t:P * (t + 1)],
                                wsrc[:, g, :, QW * n2:QW * (n2 + 1)],
                                start=(cnt == 0), stop=(cnt == 5), perf_mode=DR)
                            cnt += 1
                    # last window runs after attention ends: ScalarE is idle
                    # there, so split its evacuations and pipeline the y DMA
                    # per half-tile to shorten the serial tail
                    if tq == NQT - 1 and n2 == 1:
                        nc.scalar.mul(ysb[:, QW:C], yps, 1.0 / 1024.0)
                    else:
                        nc.vector.tensor_scalar(
                            out=ysb[:, QW * n2:QW * (n2 + 1)], in0=yps,
                            scalar1=1.0 / 1024.0, scalar2=None,
                            op0=mybir.AluOpType.mult)
                    if tq == NQT - 1:
                        nc.sync.dma_start(
                            out=y[P * t:P * (t + 1), QW * n2:QW * (n2 + 1)],
                            in_=ysb[:, QW * n2:QW * (n2 + 1)])
                if tq != NQT - 1:
                    nc.sync.dma_start(out=y[P * t:P * (t + 1), :], in_=ysb)

        def qk_proj(p, tq):
            """Q/K projection matmuls + fp8 PSUM evac for one t-window."""
            qka, _ = get_qtkt(p)
            for qk, (wsb, bsb) in enumerate(
                    ((wq_sb, qb_all[:, p:p + 1]),
                     (wk_sb, kb_all[:, p:p + 1]))):
                ps = mm_ps.tile([P, QW], F32, name=f"qk{p}{tq}", tag="mm")
                comp_mms(
                    ps, (wsb[0], wsb[1]), (xh_sb, xl_sb),
                    lambda w, i: w[:, i, :, P * p:P * (p + 1)],
                    lambda xs, i: xs[i][:, :, QW * tq:QW * (tq + 1)])
                nc.vector.tensor_scalar(
                    out=qka[:, qk, QW * tq:QW * (tq + 1)], in0=ps,
                    scalar1=1.0 / 16.0, scalar2=bsb,
                    op0=mybir.AluOpType.mult, op1=mybir.AluOpType.add)

        def qk_regroup(p, half):
            """[128, 2, T] channel-major fp8 -> [32(hl base), 2(j), 2(qk), T]
            DoubleRow layout: channel d = 32j + i at partition i, free j.
            One DMA per (hl, j) block moves both Q and K."""
            qka, qk8 = get_qtkt(p)
            win = slice(T // 2 * half, T // 2 * (half + 1))
            for hl in range(2):
                for j in range(2):
                    nc.sync.dma_start(
                        out=qk8[32 * hl:32 * (hl + 1), j, :, win],
                        in_=qka[64 * hl + 32 * j:64 * hl + 32 * (j + 1), :,
                                win])

        qtkt = {}

        def get_qtkt(p):
            if p not in qtkt:
                qtkt[p] = (
                    qtkt_pool.tile([P, 2, T], F8, name=f"qka{p}", tag="qka",
                                   bufs=2),
                    qtkt_pool.tile([64, 2, 2, T], F8, name=f"qk8{p}",
                                   tag="qk8", bufs=2),
                )
            return qtkt[p]

        # ---- phase V: V for all 8 heads, pair-0 Q/K interleaved per
        # quarter (its x window arrives with the same DMA quarter, so PE has
        # runnable work as soon as each quarter lands) ----
        v_sb = []
        for t in range(NT):
            vps = mm_ps.tile([P, CPC], F32, name=f"vps{t}", tag="mm")
            comp_mms(
                vps, (xh_sb, xl_sb), (wv_sb[0], wv_sb[1]),
                lambda xs, i: xs[i][:, :, P * t:P * (t + 1)],
                lambda w, i: w[:, i])
            vt = v_pool.tile([P, HPC, HD + 1], F16, name=f"v{t}", tag="v")
            nc.vector.memset(vt[:, :, HD], 4.0)
            nc.vector.tensor_copy(
                out=vt[:, :, 0:HD],
                in_=vps.rearrange("p (h d) -> p h d", h=HPC))
            v_sb.append(vt)

        for tq in range(NQT):
            qk_proj(0, tq)
        qk_regroup(0, 0)
        qk_regroup(0, 1)

        for p in range(NPAIR):
            _, qk8_sb = get_qtkt(p)
            o_sb = osb_pool.tile([P, T], F16, name=f"o{p}", tag="o")

            def s_unit(qt_i, hl):
                """S^T matmuls + exp + causal mask for one (q-tile, head)."""
                dlo, dhi = 32 * hl, 32 * (hl + 1)
                nkt = 4 * qt_i + 4
                pts = []
                for k0 in range(0, nkt, 2):
                    smin = [min(max(0, (k0 + u) - 4 * qt_i), 2)
                            for u in range(2)]
                    off = [P * s for s in smin]
                    sps = s_ps.tile([P, 2 * QW], F32,
                                    name=f"s{p}{hl}{qt_i}{k0}", tag="s")
                    for u in range(2):
                        k = k0 + u
                        nc.tensor.matmul(
                            sps[:, QW * u + off[u]:QW * (u + 1)],
                            qk8_sb[dlo:dhi, :, 1, P * k:P * (k + 1)],
                            qk8_sb[dlo:dhi, :, 0,
                                   QW * qt_i + off[u]:QW * (qt_i + 1)],
                            start=True, stop=True, perf_mode=DR)
                    pt = pt_pool.tile([P, 2 * QW], F16,
                                      name=f"pt{p}{hl}{qt_i}{k0}", tag="pt")
                    # exp in a single instruction per chunk: for unequal
                    # offsets, exp the union region (extra columns read stale
                    # psum; their pt slots are never consumed downstream).
                    # A fraction of full-width chunks runs on DVE via the
                    # Schraudolph bit-trick to unload the ScalarE bottleneck.
                    eoff = min(off)
                    schr = eoff == 0 and k0 % 8 == 2
                    if schr:
                        nc.vector.tensor_scalar(
                            out=pt.bitcast(I16), in0=sps,
                            scalar1=EXP_A, scalar2=EXP_B,
                            op0=mybir.AluOpType.mult,
                            op1=mybir.AluOpType.add)
                    elif eoff == 0:
                        nc.scalar.activation(
                            out=pt, in_=sps,
                            func=mybir.ActivationFunctionType.Exp,
                            scale=SCALE / 16.0)
                    else:
                        view = lambda ap: ap.rearrange(
                            "p (u c) -> p u c", u=2)[:, :, eoff:QW]
                        nc.scalar.activation(
                            out=view(pt), in_=view(sps),
                            func=mybir.ActivationFunctionType.Exp,
                            scale=SCALE / 16.0)
                    for u in range(2):
                        k = k0 + u
                        for s in range(4):
                            gs = 4 * qt_i + s
                            if gs == k:
                                sl = pt[:, QW * u + P * s:QW * u + P * (s + 1)]
                                nc.vector.tensor_mul(sl, sl, mask_sb)
                    pts.append(pt)
                return pts

            def pv_unit(qt_i, hl, pts):
                """P^T @ V_aug + normalize into o_sb for one unit."""
                hh = 2 * p + hl
                ops_ = o_ps.tile([P, 4 * (HD + 1)], F32,
                                 name=f"o{p}{hl}{qt_i}", tag="o")
                for s in range(4):
                    gs = 4 * qt_i + s
                    for k in range(gs + 1):
                        nc.tensor.matmul(
                            ops_[:, (HD + 1) * s:(HD + 1) * (s + 1)],
                            pts[k // 2][:, QW * (k % 2) + P * s:
                                        QW * (k % 2) + P * (s + 1)],
                            v_sb[k][:, hh, :],
                            start=(k == 0), stop=(k == gs))
                r_ = r_pool.tile([P, 4], F32, name=f"r{p}{hl}{qt_i}", tag="r")
                nc.vector.reciprocal(
                    r_, ops_.rearrange("p (s c) -> p s c", c=HD + 1)[:, :, HD])
                out_ap = o_sb[:, QW * qt_i:QW * (qt_i + 1)].rearrange(
                    "p (s h d) -> p s h d", s=4, h=2)[:, :, hl, :]
                nc.vector.tensor_mul(
                    out_ap,
                    ops_.rearrange("p (s c) -> p s c", c=HD + 1)[:, :, 0:HD],
                    r_.unsqueeze(2).broadcast_to((P, 4, HD)))

            # software pipeline: PV runs one unit behind S/exp. After each
            # q-window (hl == 1) completes: emit the NEXT pair's projection
            # matmuls for that window (PE filler for this ScalarE-bound
            # phase), then this window's transpose chain (DMA + GpSimd, no
            # PE). On the last pair the filler is the output projection,
            # lagged one window behind its transpose.
            last = p == NPAIR - 1
            y_ready = []

            def drain(pend):
                qt_i, hl, pts = pend
                pv_unit(qt_i, hl, pts)
                if hl == 0:
                    if not last:
                        qk_proj(p + 1, qt_i)
                        if qt_i == NQT - 1:
                            qk_regroup(p + 1, 1)
                else:
                    if not last and qt_i == 1:
                        qk_regroup(p + 1, 0)
                    transpose_tq(p, o_sb, qt_i)
                    if last:
                        if y_ready:
                            emit_y(y_ready.pop(0))
                        y_ready.append(qt_i)

            units = [(qt_i, hl) for qt_i in range(NQT) for hl in range(2)]
            pq = []
            for (qt_i, hl) in units:
                pts = s_unit(qt_i, hl)
                pq.append((qt_i, hl, pts))
                if len(pq) > 1:
                    drain(pq.pop(0))
            for pend in pq:
                drain(pend)
            for tq in y_ready:
                emit_y(tq)


def _comp8(a):
    hi = a.astype(NPF8)
    lo = (a - hi.astype(np.float32)).astype(NPF8)
    return hi, lo


def _prep_inputs(x, w_attn, b_attn, w_proj):
    """Per-core input maps."""
    in_maps = []
    # contraction layout [unit, p, j, ...]: c = 256*unit + 128*j + p
    def units(a, n_u):
        # a: [n_u*256, M] -> [n_u, 128, 2, M]
        return np.ascontiguousarray(
            a.reshape(n_u, 2, P, -1).transpose(0, 2, 1, 3))

    wq_h = {}
    for g in range(2):
        qs = slice(CPC * g, CPC * (g + 1))
        ks = slice(C + CPC * g, C + CPC * (g + 1))
        vs = slice(2 * C + CPC * g, 2 * C + CPC * (g + 1))
        wqh, wql = _comp8(64.0 * w_attn[:, qs])
        wkh, wkl = _comp8(64.0 * w_attn[:, ks])
        wvh, wvl = _comp8(64.0 * w_attn[:, vs])
        wph, wpl = _comp8(64.0 * w_proj[CPC * g:CPC * (g + 1), :])
        wq_h[g] = dict(
            wq8h=units(wqh, NU), wq8l=units(wql, NU),
            wk8h=units(wkh, NU), wk8l=units(wkl, NU),
            wv8c=np.ascontiguousarray(
                np.stack([units(wvh, NU), units(wvl, NU)], axis=1)
                .transpose(0, 2, 1, 3, 4)),
            wp8h=units(wph, 2), wp8l=units(wpl, 2),
            qb4=np.ascontiguousarray(4.0 * b_attn[qs]),
            kb4=np.ascontiguousarray(4.0 * b_attn[ks]),
        )
    for c in range(NCORES):
        b = c // 2
        g = c % 2
        xT = np.ascontiguousarray(x[b].T)          # [C, T] fp32
        xh, xl = _comp8(xT)
        in_maps.append({
            "x8c": np.ascontiguousarray(
                np.stack([units(xh, NU), units(xl, NU)], axis=1)
                .transpose(0, 2, 1, 3, 4)),
            **wq_h[g],
            "maskT": np.triu(np.ones((P, P), dtype=np.float16)),
        })
    return in_maps


_CACHED_NC = None


def kernel(x, w_attn, b_attn, w_proj, b_proj):
    global _CACHED_NC
    x = np.asarray(x, dtype=np.float32)
    w_attn = np.asarray(w_attn, dtype=np.float32)
    b_attn = np.asarray(b_attn, dtype=np.float32)
    w_proj = np.asarray(w_proj, dtype=np.float32)
    b_proj = np.asarray(b_proj, dtype=np.float32)

    if _CACHED_NC is None:
        _CACHED_NC = build_kernel(loop_n=1)
    nc = _CACHED_NC
    in_maps = _prep_inputs(x, w_attn, b_attn, w_proj)
    res = run_bass_kernel_spmd(nc, in_maps, core_ids=list(range(NCORES)),
                               trace=False)
    out = np.empty((B, T, C), dtype=np.float32)
    # exact row-vector bias contribution: rows of softmax sum to 1
    for b in range(B):
        out[b] = (res.results[2 * b]["y"].astype(np.float32)
                  + res.results[2 * b + 1]["y"].astype(np.float32))
    bias_row = b_attn[2 * C:3 * C] @ w_proj + b_proj
    out += bias_row[None, None, :]
    return out


# revision 109
# speedup vs baseline: 1.3157x; 1.0094x over previous
"""Causal multi-head attention block (QKV proj -> causal attention -> out proj)
for Trainium2, sharded over 8 NeuronCores.

Sharding: tensor/data hybrid. Core c handles batch b = c//2 and half the heads
(g = c%2, 8 of 16 heads). Per core:
  - QKV projection with error-compensated fp8e4m3 DoubleRow matmuls
    (x = x_hi + x_lo, w = w_hi + w_lo; compute hi*hi + hi*lo + lo*hi,
    each a 256-deep DoubleRow matmul). Weights scaled x64 on host so fp8
    stays in the normal range; rescaled on the PSUM evacuation.
  - causal attention in S^T = K @ Q^T layout: q/k quantized to fp8 at the
    PSUM evacuation, DMA-regrouped into [32p, 2] DoubleRow layout; S
    matmuls are fp8 DoubleRow. exp on ScalarE (a fraction on VectorE via
    the Schraudolph int16 bit-trick); P^T (fp16) @ V_aug (fp16, scaled
    ones column -> 0.25/rowsum for free from the DVE reciprocal)
  - DVE normalize, DMA-XBAR transpose of O + GpSimd fp8 hi/lo split,
    fp8-compensated output projection -> partial y [T, C] (fp16)
Host: y[b] = partial[2b] + partial[2b+1] (+ bias terms, see below).

Biases: b_attn Q/K slices are added on-device (fused into the PSUM->SBUF
copies). The V-bias and b_proj contributions are exact row vectors on the
output (rows of softmax sum to 1): y += (b_v @ w_proj + b_proj), added on
host during the unshard.

Scheduling: pair p's attention units (q-window, head) run S/exp one unit
ahead of PV; pair p+1's projections + regroup DMAs are emitted inside
pair p's attention as PE filler; the output projection pipelines into the
last pair's attention per q-window.

Scaling ledger (all powers of 2, exact):
  wq8/wk8/wv8 = 64*w (hi+lo fp8 pair)   -> q/k/v psum = 64*true
  qka = psum/16 + 4*qb = 4*true (fp8)
  S psum = 16*S_true; exp scale = HD^-0.5/16
  vt = 64*V (fp16), ones col = 4.0  -> recip gives 0.25/rowsum
  o_sb = (64*O')*(0.25/r) = 16*O (fp16)
  ot8 = fp8 pair of 16*O ; wp8 = 64*w_proj (hi+lo) -> y psum = 1024*y
  y = psum/1024 (fp16; summed in fp32 on host)
"""

import math

import numpy as np
import ml_dtypes

import concourse.bass as bass
import concourse.mybir as mybir
import concourse.tile as tile
from concourse import bacc
from concourse.bass_utils import run_bass_kernel_spmd

B, T, C = 4, 2048, 1024
NH, HD = 16, 64
NCORES = 8
HPC = NH // 2          # heads per core = 8
CPC = HPC * HD         # channels per core = 512
P = 128                # partitions
NT = T // P            # 16 t-tiles of 128
NU = C // 256          # 4 DoubleRow contraction units of 256
NPAIR = HPC // 2       # 4 head pairs
QW = 512               # q-tile width
NQT = T // QW          # 4 q-tiles

F32 = mybir.dt.float32
F16 = mybir.dt.float16
F8 = mybir.dt.float8e4
I16 = mybir.dt.int16
DR = mybir.MatmulPerfMode.DoubleRow
NPF8 = ml_dtypes.float8_e4m3
SCALE = HD ** -0.5
# Schraudolph fp16 exp approximation on DVE for a fraction of the softmax:
# exp(s) ~ bitcast_fp16(int16(s*EXP_A + EXP_B)). EXP_B tuned for min rel err
# assuming round-to-nearest int conversion.
EXP_A = (SCALE / 16.0) * 1.4426950408889634 * 1024.0
EXP_B = 15.0 * 1024.0 - 38.5


def build_kernel(loop_n: int = 1):
    nc = bacc.Bacc("TRN2", target_bir_lowering=False, debug=False)
    x8c = nc.dram_tensor("x8c", [NU, P, 2, 2, T], F8, kind="ExternalInput").ap()
    wq8h = nc.dram_tensor("wq8h", [NU, P, 2, CPC], F8, kind="ExternalInput").ap()
    wq8l = nc.dram_tensor("wq8l", [NU, P, 2, CPC], F8, kind="ExternalInput").ap()
    wk8h = nc.dram_tensor("wk8h", [NU, P, 2, CPC], F8, kind="ExternalInput").ap()
    wk8l = nc.dram_tensor("wk8l", [NU, P, 2, CPC], F8, kind="ExternalInput").ap()
    wv8c = nc.dram_tensor("wv8c", [NU, P, 2, 2, CPC], F8,
                          kind="ExternalInput").ap()
    wp8h = nc.dram_tensor("wp8h", [2, P, 2, C], F8, kind="ExternalInput").ap()
    wp8l = nc.dram_tensor("wp8l", [2, P, 2, C], F8, kind="ExternalInput").ap()
    qb4 = nc.dram_tensor("qb4", [CPC], F32, kind="ExternalInput").ap()
    kb4 = nc.dram_tensor("kb4", [CPC], F32, kind="ExternalInput").ap()
    maskT = nc.dram_tensor("maskT", [P, P], F16, kind="ExternalInput").ap()
    y = nc.dram_tensor("y", [T, C], F16, kind="ExternalOutput").ap()

    args = (x8c, wq8h, wq8l, wk8h, wk8l, wv8c, wp8h, wp8l,
            qb4, kb4, maskT, y)
    with tile.TileContext(nc) as tc:
        if loop_n == 1:
            _body(tc, nc, *args)
        else:
            with tc.For_i(0, loop_n, 1):
                _body(tc, nc, *args)
    nc.compile()
    return nc


def _body(tc, nc, x8c, wq8h, wq8l, wk8h, wk8l, wv8c,
          wp8h, wp8l, qb4, kb4, maskT, y):
    from contextlib import ExitStack

    ctx = ExitStack()
    with ctx:
        const = ctx.enter_context(tc.tile_pool(name="const", bufs=1))
        x_pool = ctx.enter_context(tc.tile_pool(name="xp", bufs=NU))
        w8_pool = ctx.enter_context(tc.tile_pool(name="w8p", bufs=1))
        v_pool = ctx.enter_context(tc.tile_pool(name="vp", bufs=NT)) if False else ctx.enter_context(tc.tile_pool(name="vp", bufs=NT))
        qtkt_pool = ctx.enter_context(tc.tile_pool(name="qtkt", bufs=3))
        bias_pool = ctx.enter_context(tc.tile_pool(name="biasp", bufs=2))
        pt_pool = ctx.enter_context(tc.tile_pool(name="ptp", bufs=21))
        osb_pool = ctx.enter_context(tc.tile_pool(name="osb", bufs=4))
        ot_pool = ctx.enter_context(tc.tile_pool(name="otp", bufs=4))
        r_pool = ctx.enter_context(tc.tile_pool(name="rp", bufs=6))
        y_pool = ctx.enter_context(tc.tile_pool(name="yp", bufs=4))
        mm_ps = ctx.enter_context(tc.tile_pool(name="mmps", bufs=2, space="PSUM"))
        s_ps = ctx.enter_context(tc.tile_pool(name="sps", bufs=2, space="PSUM"))
        o_ps = ctx.enter_context(tc.tile_pool(name="ops", bufs=2, space="PSUM"))

        # ---- weight + x loads (fp8 hi/lo pairs) ----
        # order matters for the startup critical path: wv + x first (phase V
        # needs them, unit-interleaved so the first V matmuls start early),
        # wq/wk next, wp last. Two DGE queues (SP + ACT) in parallel.
        xh_sb, xl_sb = [], []
        wvc = w8_pool.tile([P, NU, 2, 2, CPC], F8, name="wvc", tag="wvc")
        wv_sb = [wvc[:, :, 0], wvc[:, :, 1]]
        xc_sb = []
        for i in range(NU):
            xc = x_pool.tile([P, 2, 2, T], F8, name=f"xc{i}", tag="x")
            xc_sb.append(xc)
            xh_sb.append(xc[:, 0])
            xl_sb.append(xc[:, 1])
        for i in range(NU):
            eng, eng2 = ((nc.sync, nc.scalar) if i % 2 == 0
                         else (nc.scalar, nc.sync))
            eng2.dma_start(out=wvc[:, i], in_=wv8c[i])
            sl = slice(0, T // 4)
            eng.dma_start(out=xc_sb[i][:, :, :, sl], in_=x8c[i][:, :, :, sl])
        mask_sb = const.tile([P, P], F16, tag="mask")
        nc.sync.dma_start(out=mask_sb, in_=maskT)
        for half in range(2):
            sl = slice(T // 4 + T * 3 // 8 * half,
                       T // 4 + T * 3 // 8 * (half + 1))
            for i in range(NU):
                eng = nc.sync if i % 2 == 0 else nc.scalar
                eng.dma_start(out=xc_sb[i][:, :, :, sl],
                              in_=x8c[i][:, :, :, sl])
        wq_sb, wk_sb = [], []
        for nm, drh, drl, lst in (("wq", wq8h, wq8l, wq_sb),
                                  ("wk", wk8h, wk8l, wk_sb)):
            for tag, dr_ in ((f"{nm}h", drh), (f"{nm}l", drl)):
                t_ = w8_pool.tile([P, NU, 2, CPC], F8, name=tag, tag=tag)
                eng = nc.sync if nm == "wq" else nc.scalar
                eng.dma_start(out=t_, in_=dr_.rearrange("i p j m -> p i j m"))
                lst.append(t_)
        qb_all = bias_pool.tile([P, NPAIR], F32, tag="qb", bufs=1)
        kb_all = bias_pool.tile([P, NPAIR], F32, tag="kb", bufs=1)
        nc.sync.dma_start(out=qb_all, in_=qb4.rearrange("(a p) -> p a", p=P))
        nc.sync.dma_start(out=kb_all, in_=kb4.rearrange("(a p) -> p a", p=P))

        def comp_mms(ps, lhs_hl, rhs_hl, lslice, rslice):
            """hi*hi + lo*hi + hi*lo DoubleRow accumulation over NU units."""
            terms = [(0, 0), (1, 0), (0, 1)]
            n = NU * len(terms) - 1
            cnt = 0
            for i in range(NU):
                for (a, b_) in terms:
                    nc.tensor.matmul(
                        ps, lslice(lhs_hl[a], i), rslice(rhs_hl[b_], i),
                        start=(cnt == 0), stop=(cnt == n), perf_mode=DR)
                    cnt += 1

        # ---- fp8 wp tiles for the output projection ----
        wp_sb = []
        for tag, dr_ in (("wph", wp8h), ("wpl", wp8l)):
            t_ = w8_pool.tile([P, 2, 2, C], F8, name=tag, tag=tag)
            for g in range(2):
                nc.sync.dma_start(out=t_[:, g], in_=dr_[g])
            wp_sb.append(t_)
        ot8h, ot8l, ot16 = [], [], []
        for g in range(2):
            ot8h.append(ot_pool.tile([P, 2, T], F8, name=f"oth{g}", tag="ot8"))
            ot8l.append(ot_pool.tile([P, 2, T], F8, name=f"otl{g}", tag="ot8"))
            ot16.append(ot_pool.tile([P, 2, T], F16, name=f"ot16{g}",
                                     tag="ot16", bufs=2))

        # ---- per head-pair: O^T via DMA-XBAR transpose + GpSimd fp8 split --
        def transpose_tq(p, o_sb, tq, fine=False):
            """Blocked transpose of one o_sb q-window into ot8 hi/lo fp8.
            fine=True splits the fp8 hi/lo pass per 128-block so downstream
            output-projection tiles unblock as each block lands (used for
            the strictly-serial final window)."""
            g, j = p // 2, p % 2
            win = slice(QW * tq, QW * (tq + 1))
            out3 = ot16[g][:, j, win].rearrange("p (b c) -> p b c", b=4)
            nc.sync.dma_start(out=out3, in_=o_sb[:, win], transpose=True)
            blocks = ([slice(QW * tq + P * b, QW * tq + P * (b + 1))
                       for b in range(4)] if fine else [win])
            for bw in blocks:
                nc.gpsimd.tensor_copy(out=ot8h[g][:, j, bw],
                                      in_=ot16[g][:, j, bw])
                nc.gpsimd.tensor_sub(ot8l[g][:, j, bw], ot16[g][:, j, bw],
                                     ot8h[g][:, j, bw])

        def emit_y(tq):
            """Output-projection matmuls for the 4 t-tiles of one tq window."""
            for t in range(4 * tq, 4 * tq + 4):
                ysb = y_pool.tile([P, C], F16, name=f"y{t}", tag="y")
                for n2 in range(2):
                    yps = mm_ps.tile([P, QW], F32, name=f"yps{t}{n2}", tag="mm")
                    cnt = 0
                    for g in range(2):
                        for (osrc, wsrc) in ((ot8h[g], wp_sb[0]),
                                             (ot8h[g], wp_sb[1]),
                                             (ot8l[g], wp_sb[0])):
                            nc.tensor.matmul(
                                yps, osrc[:, :, P * t:P * (t + 1)],
                                wsrc[:, g, :, QW * n2:QW * (n2 + 1)],
                                start=(cnt == 0), stop=(cnt == 5), perf_mode=DR)
                            cnt += 1
                    # last window runs after attention ends: ScalarE is idle
                    # there, so split its evacuations and pipeline the y DMA
                    # per half-tile to shorten the serial tail
                    if tq == NQT - 1 and n2 == 1:
                        nc.scalar.mul(ysb[:, QW:C], yps, 1.0 / 1024.0)
                    else:
                        nc.vector.tensor_scalar(
                            out=ysb[:, QW * n2:QW * (n2 + 1)], in0=yps,
                            scalar1=1.0 / 1024.0, scalar2=None,
                            op0=mybir.AluOpType.mult)
                    if tq == NQT - 1:
                        nc.sync.dma_start(
                            out=y[P * t:P * (t + 1), QW * n2:QW * (n2 + 1)],
                            in_=ysb[:, QW * n2:QW * (n2 + 1)])
                if tq != NQT - 1:
                    nc.sync.dma_start(out=y[P * t:P * (t + 1), :], in_=ysb)

        def qk_proj(p, tq):
            """Q/K projection matmuls + fp8 PSUM evac for one t-window."""
            qka, _ = get_qtkt(p)
            for qk, (wsb, bsb) in enumerate(
                    ((wq_sb, qb_all[:, p:p + 1]),
                     (wk_sb, kb_all[:, p:p + 1]))):
                ps = mm_ps.tile([P, QW], F32, name=f"qk{p}{tq}", tag="mm")
                comp_mms(
                    ps, (wsb[0], wsb[1]), (xh_sb, xl_sb),
                    lambda w, i: w[:, i, :, P * p:P * (p + 1)],
                    lambda xs, i: xs[i][:, :, QW * tq:QW * (tq + 1)])
                nc.vector.tensor_scalar(
                    out=qka[:, qk, QW * tq:QW * (tq + 1)], in0=ps,
                    scalar1=1.0 / 16.0, scalar2=bsb,
                    op0=mybir.AluOpType.mult, op1=mybir.AluOpType.add)

        def qk_regroup(p, half):
            """[128, 2, T] channel-major fp8 -> [32(hl base), 2(j), 2(qk), T]
            DoubleRow layout: channel d = 32j + i at partition i, free j.
            One DMA per (hl, j) block moves both Q and K."""
            qka, qk8 = get_qtkt(p)
            win = slice(T // 2 * half, T // 2 * (half + 1))
            for hl in range(2):
                for j in range(2):
                    nc.sync.dma_start(
                        out=qk8[32 * hl:32 * (hl + 1), j, :, win],
                        in_=qka[64 * hl + 32 * j:64 * hl + 32 * (j + 1), :,
                                win])

        qtkt = {}

        def get_qtkt(p):
            if p not in qtkt:
                qtkt[p] = (
                    qtkt_pool.tile([P, 2, T], F8, name=f"qka{p}", tag="qka",
                                   bufs=2),
                    qtkt_pool.tile([64, 2, 2, T], F8, name=f"qk8{p}",
                                   tag="qk8", bufs=2),
                )
            return qtkt[p]

        # ---- phase V: V for all 8 heads, pair-0 Q/K interleaved per
        # quarter (its x window arrives with the same DMA quarter, so PE has
        # runnable work as soon as each quarter lands) ----
        v_sb = []
        for t in range(NT):
            vps = mm_ps.tile([P, CPC], F32, name=f"vps{t}", tag="mm")
            comp_mms(
                vps, (xh_sb, xl_sb), (wv_sb[0], wv_sb[1]),
                lambda xs, i: xs[i][:, :, P * t:P * (t + 1)],
                lambda w, i: w[:, i])
            vt = v_pool.tile([P, HPC, HD + 1], F16, name=f"v{t}", tag="v")
            nc.vector.memset(vt[:, :, HD], 4.0)
            nc.vector.tensor_copy(
                out=vt[:, :, 0:HD],
                in_=vps.rearrange("p (h d) -> p h d", h=HPC))
            v_sb.append(vt)

        for tq in range(NQT):
            qk_proj(0, tq)
        qk_regroup(0, 0)
        qk_regroup(0, 1)

        for p in range(NPAIR):
            _, qk8_sb = get_qtkt(p)
            o_sb = osb_pool.tile([P, T], F16, name=f"o{p}", tag="o")

            def s_unit(qt_i, hl):
                """S^T matmuls + exp + causal mask for one (q-tile, head)."""
                dlo, dhi = 32 * hl, 32 * (hl + 1)
                nkt = 4 * qt_i + 4
                pts = []
                for k0 in range(0, nkt, 2):
                    smin = [min(max(0, (k0 + u) - 4 * qt_i), 2)
                            for u in range(2)]
                    off = [P * s for s in smin]
                    sps = s_ps.tile([P, 2 * QW], F32,
                                    name=f"s{p}{hl}{qt_i}{k0}", tag="s")
                    for u in range(2):
                        k = k0 + u
                        nc.tensor.matmul(
                            sps[:, QW * u + off[u]:QW * (u + 1)],
                            qk8_sb[dlo:dhi, :, 1, P * k:P * (k + 1)],
                            qk8_sb[dlo:dhi, :, 0,
                                   QW * qt_i + off[u]:QW * (qt_i + 1)],
                            start=True, stop=True, perf_mode=DR)
                    pt = pt_pool.tile([P, 2 * QW], F16,
                                      name=f"pt{p}{hl}{qt_i}{k0}", tag="pt")
                    # exp in a single instruction per chunk: for unequal
                    # offsets, exp the union region (extra columns read stale
                    # psum; their pt slots are never consumed downstream).
                    # A fraction of full-width chunks runs on DVE via the
                    # Schraudolph bit-trick to unload the ScalarE bottleneck.
                    eoff = min(off)
                    schr = eoff == 0 and k0 % 8 == 2
                    if schr:
                        nc.vector.tensor_scalar(
                            out=pt.bitcast(I16), in0=sps,
                            scalar1=EXP_A, scalar2=EXP_B,
                            op0=mybir.AluOpType.mult,
                            op1=mybir.AluOpType.add)
                    elif eoff == 0:
                        nc.scalar.activation(
                            out=pt, in_=sps,
                            func=mybir.ActivationFunctionType.Exp,
                            scale=SCALE / 16.0)
                    else:
                        view = lambda ap: ap.rearrange(
                            "p (u c) -> p u c", u=2)[:, :, eoff:QW]
                        nc.scalar.activation(
                            out=view(pt), in_=view(sps),
                            func=mybir.ActivationFunctionType.Exp,
                            scale=SCALE / 16.0)
                    for u in range(2):
                        k = k0 + u
                        for s in range(4):
                            gs = 4 * qt_i + s
                            if gs == k:
                                sl = pt[:, QW * u + P * s:QW * u + P * (s + 1)]
                                nc.vector.tensor_mul(sl, sl, mask_sb)
                    pts.append(pt)
                return pts

            def pv_unit(qt_i, hl, pts):
                """P^T @ V_aug + normalize into o_sb for one unit."""
                hh = 2 * p + hl
                ops_ = o_ps.tile([P, 4 * (HD + 1)], F32,
                                 name=f"o{p}{hl}{qt_i}", tag="o")
                for s in range(4):
                    gs = 4 * qt_i + s
                    for k in range(gs + 1):
                        nc.tensor.matmul(
                            ops_[:, (HD + 1) * s:(HD + 1) * (s + 1)],
                            pts[k // 2][:, QW * (k % 2) + P * s:
                                        QW * (k % 2) + P * (s + 1)],
                            v_sb[k][:, hh, :],
                            start=(k == 0), stop=(k == gs))
                r_ = r_pool.tile([P, 4], F32, name=f"r{p}{hl}{qt_i}", tag="r")
                nc.vector.reciprocal(
                    r_, ops_.rearrange("p (s c) -> p s c", c=HD + 1)[:, :, HD])
                out_ap = o_sb[:, QW * qt_i:QW * (qt_i + 1)].rearrange(
                    "p (s h d) -> p s h d", s=4, h=2)[:, :, hl, :]
                nc.vector.tensor_mul(
                    out_ap,
                    ops_.rearrange("p (s c) -> p s c", c=HD + 1)[:, :, 0:HD],
                    r_.unsqueeze(2).broadcast_to((P, 4, HD)))

            # software pipeline: PV runs one unit behind S/exp. After each
            # q-window (hl == 1) completes: emit the NEXT pair's projection
            # matmuls for that window (PE filler for this ScalarE-bound
            # phase), then this window's transpose chain (DMA + GpSimd, no
            # PE). On the last pair the filler is the output projection,
            # lagged one window behind its transpose.
            last = p == NPAIR - 1
            y_ready = []

            def drain(pend):
                qt_i, hl, pts = pend
                pv_unit(qt_i, hl, pts)
                if hl == 0:
                    if not last:
                        qk_proj(p + 1, qt_i)
                        if qt_i == NQT - 1:
                            qk_regroup(p + 1, 1)
                else:
                    if not last and qt_i == 1:
                        qk_regroup(p + 1, 0)
                    transpose_tq(p, o_sb, qt_i,
                                 fine=last)
                    if last:
                        if y_ready:
                            emit_y(y_ready.pop(0))
                        y_ready.append(qt_i)

            units = [(qt_i, hl) for qt_i in range(NQT) for hl in range(2)]
            pq = []
            for (qt_i, hl) in units:
                pts = s_unit(qt_i, hl)
                pq.append((qt_i, hl, pts))
                if len(pq) > 1:
                    drain(pq.pop(0))
            for pend in pq:
                drain(pend)
            for tq in y_ready:
                emit_y(tq)


def _comp8(a):
    hi = a.astype(NPF8)
    lo = (a - hi.astype(np.float32)).astype(NPF8)
    return hi, lo


def _prep_inputs(x, w_attn, b_attn, w_proj):
    """Per-core input maps."""
    in_maps = []
    # contraction layout [unit, p, j, ...]: c = 256*unit + 128*j + p
    def units(a, n_u):
        # a: [n_u*256, M] -> [n_u, 128, 2, M]
        return np.ascontiguousarray(
            a.reshape(n_u, 2, P, -1).transpose(0, 2, 1, 3))

    wq_h = {}
    for g in range(2):
        qs = slice(CPC * g, CPC * (g + 1))
        ks = slice(C + CPC * g, C + CPC * (g + 1))
        vs = slice(2 * C + CPC * g, 2 * C + CPC * (g + 1))
        wqh, wql = _comp8(64.0 * w_attn[:, qs])
        wkh, wkl = _comp8(64.0 * w_attn[:, ks])
        wvh, wvl = _comp8(64.0 * w_attn[:, vs])
        wph, wpl = _comp8(64.0 * w_proj[CPC * g:CPC * (g + 1), :])
        wq_h[g] = dict(
            wq8h=units(wqh, NU), wq8l=units(wql, NU),
            wk8h=units(wkh, NU), wk8l=units(wkl, NU),
            wv8c=np.ascontiguousarray(
                np.stack([units(wvh, NU), units(wvl, NU)], axis=1)
                .transpose(0, 2, 1, 3, 4)),
            wp8h=units(wph, 2), wp8l=units(wpl, 2),
            qb4=np.ascontiguousarray(4.0 * b_attn[qs]),
            kb4=np.ascontiguousarray(4.0 * b_attn[ks]),
        )
    for c in range(NCORES):
        b = c // 2
        g = c % 2
        xT = np.ascontiguousarray(x[b].T)          # [C, T] fp32
        xh, xl = _comp8(xT)
        in_maps.append({
            "x8c": np.ascontiguousarray(
                np.stack([units(xh, NU), units(xl, NU)], axis=1)
                .transpose(0, 2, 1, 3, 4)),
            **wq_h[g],
            "maskT": np.triu(np.ones((P, P), dtype=np.float16)),
        })
    return in_maps


_CACHED_NC = None


def kernel(x, w_attn, b_attn, w_proj, b_proj):
    global _CACHED_NC
    x = np.asarray(x, dtype=np.float32)
    w_attn = np.asarray(w_attn, dtype=np.float32)
    b_attn = np.asarray(b_attn, dtype=np.float32)
    w_proj = np.asarray(w_proj, dtype=np.float32)
    b_proj = np.asarray(b_proj, dtype=np.float32)

    if _CACHED_NC is None:
        _CACHED_NC = build_kernel(loop_n=1)
    nc = _CACHED_NC
    in_maps = _prep_inputs(x, w_attn, b_attn, w_proj)
    res = run_bass_kernel_spmd(nc, in_maps, core_ids=list(range(NCORES)),
                               trace=False)
    out = np.empty((B, T, C), dtype=np.float32)
    # exact row-vector bias contribution: rows of softmax sum to 1
    for b in range(B):
        out[b] = (res.results[2 * b]["y"].astype(np.float32)
                  + res.results[2 * b + 1]["y"].astype(np.float32))
    bias_row = b_attn[2 * C:3 * C] @ w_proj + b_proj
    out += bias_row[None, None, :]
    return out


# revision 112
# speedup vs baseline: 1.3167x; 1.0008x over previous
"""Causal multi-head attention block (QKV proj -> causal attention -> out proj)
for Trainium2, sharded over 8 NeuronCores.

Sharding: tensor/data hybrid. Core c handles batch b = c//2 and half the heads
(g = c%2, 8 of 16 heads). Per core:
  - QKV projection with error-compensated fp8e4m3 DoubleRow matmuls
    (x = x_hi + x_lo, w = w_hi + w_lo; compute hi*hi + hi*lo + lo*hi,
    each a 256-deep DoubleRow matmul). Weights scaled x64 on host so fp8
    stays in the normal range; rescaled on the PSUM evacuation.
  - causal attention in S^T = K @ Q^T layout: q/k quantized to fp8 at the
    PSUM evacuation, DMA-regrouped into [32p, 2] DoubleRow layout; S
    matmuls are fp8 DoubleRow. exp on ScalarE (a fraction on VectorE via
    the Schraudolph int16 bit-trick); P^T (fp16) @ V_aug (fp16, scaled
    ones column -> 0.25/rowsum for free from the DVE reciprocal)
  - DVE normalize, DMA-XBAR transpose of O + GpSimd fp8 hi/lo split,
    fp8-compensated output projection -> partial y [T, C] (fp16)
Host: y[b] = partial[2b] + partial[2b+1] (+ bias terms, see below).

Biases: b_attn Q/K slices are added on-device (fused into the PSUM->SBUF
copies). The V-bias and b_proj contributions are exact row vectors on the
output (rows of softmax sum to 1): y += (b_v @ w_proj + b_proj), added on
host during the unshard.

Scheduling: pair p's attention units (q-window, head) run S/exp one unit
ahead of PV; pair p+1's projections + regroup DMAs are emitted inside
pair p's attention as PE filler; the output projection pipelines into the
last pair's attention per q-window.

Scaling ledger (all powers of 2, exact):
  wq8/wk8/wv8 = 64*w (hi+lo fp8 pair)   -> q/k/v psum = 64*true
  qka = psum/16 + 4*qb = 4*true (fp8)
  S psum = 16*S_true; exp scale = HD^-0.5/16
  vt = 64*V (fp16), ones col = 4.0  -> recip gives 0.25/rowsum
  o_sb = (64*O')*(0.25/r) = 16*O (fp16)
  ot8 = fp8 pair of 16*O ; wp8 = 64*w_proj (hi+lo) -> y psum = 1024*y
  y = psum/1024 (fp16; summed in fp32 on host)
"""

import math

import numpy as np
import ml_dtypes

import concourse.bass as bass
import concourse.mybir as mybir
import concourse.tile as tile
from concourse import bacc
from concourse.bass_utils import run_bass_kernel_spmd

B, T, C = 4, 2048, 1024
NH, HD = 16, 64
NCORES = 8
HPC = NH // 2          # heads per core = 8
CPC = HPC * HD         # channels per core = 512
P = 128                # partitions
NT = T // P            # 16 t-tiles of 128
NU = C // 256          # 4 DoubleRow contraction units of 256
NPAIR = HPC // 2       # 4 head pairs
QW = 512               # q-tile width
NQT = T // QW          # 4 q-tiles

F32 = mybir.dt.float32
F16 = mybir.dt.float16
F8 = mybir.dt.float8e4
I16 = mybir.dt.int16
DR = mybir.MatmulPerfMode.DoubleRow
NPF8 = ml_dtypes.float8_e4m3
SCALE = HD ** -0.5
# Schraudolph fp16 exp approximation on DVE for a fraction of the softmax:
# exp(s) ~ bitcast_fp16(int16(s*EXP_A + EXP_B)). EXP_B tuned for min rel err
# assuming round-to-nearest int conversion.
EXP_A = (SCALE / 16.0) * 1.4426950408889634 * 1024.0
EXP_B = 15.0 * 1024.0 - 38.5


def build_kernel(loop_n: int = 1):
    nc = bacc.Bacc("TRN2", target_bir_lowering=False, debug=False)
    x8c = nc.dram_tensor("x8c", [NU, P, 2, 2, T], F8, kind="ExternalInput").ap()
    wq8h = nc.dram_tensor("wq8h", [NU, P, 2, CPC], F8, kind="ExternalInput").ap()
    wq8l = nc.dram_tensor("wq8l", [NU, P, 2, CPC], F8, kind="ExternalInput").ap()
    wk8h = nc.dram_tensor("wk8h", [NU, P, 2, CPC], F8, kind="ExternalInput").ap()
    wk8l = nc.dram_tensor("wk8l", [NU, P, 2, CPC], F8, kind="ExternalInput").ap()
    wv8c = nc.dram_tensor("wv8c", [NU, P, 2, 2, CPC], F8,
                          kind="ExternalInput").ap()
    wp8h = nc.dram_tensor("wp8h", [2, P, 2, C], F8, kind="ExternalInput").ap()
    wp8l = nc.dram_tensor("wp8l", [2, P, 2, C], F8, kind="ExternalInput").ap()
    qb4 = nc.dram_tensor("qb4", [CPC], F32, kind="ExternalInput").ap()
    kb4 = nc.dram_tensor("kb4", [CPC], F32, kind="ExternalInput").ap()
    maskT = nc.dram_tensor("maskT", [P, P], F16, kind="ExternalInput").ap()
    y = nc.dram_tensor("y", [T, C], F16, kind="ExternalOutput").ap()

    args = (x8c, wq8h, wq8l, wk8h, wk8l, wv8c, wp8h, wp8l,
            qb4, kb4, maskT, y)
    with tile.TileContext(nc) as tc:
        if loop_n == 1:
            _body(tc, nc, *args)
        else:
            with tc.For_i(0, loop_n, 1):
                _body(tc, nc, *args)
    nc.compile()
    return nc


def _body(tc, nc, x8c, wq8h, wq8l, wk8h, wk8l, wv8c,
          wp8h, wp8l, qb4, kb4, maskT, y):
    from contextlib import ExitStack

    ctx = ExitStack()
    with ctx:
        const = ctx.enter_context(tc.tile_pool(name="const", bufs=1))
        x_pool = ctx.enter_context(tc.tile_pool(name="xp", bufs=NU))
        w8_pool = ctx.enter_context(tc.tile_pool(name="w8p", bufs=1))
        v_pool = ctx.enter_context(tc.tile_pool(name="vp", bufs=NT)) if False else ctx.enter_context(tc.tile_pool(name="vp", bufs=NT))
        qtkt_pool = ctx.enter_context(tc.tile_pool(name="qtkt", bufs=3))
        bias_pool = ctx.enter_context(tc.tile_pool(name="biasp", bufs=2))
        pt_pool = ctx.enter_context(tc.tile_pool(name="ptp", bufs=21))
        osb_pool = ctx.enter_context(tc.tile_pool(name="osb", bufs=4))
        ot_pool = ctx.enter_context(tc.tile_pool(name="otp", bufs=4))
        r_pool = ctx.enter_context(tc.tile_pool(name="rp", bufs=6))
        y_pool = ctx.enter_context(tc.tile_pool(name="yp", bufs=4))
        mm_ps = ctx.enter_context(tc.tile_pool(name="mmps", bufs=2, space="PSUM"))
        s_ps = ctx.enter_context(tc.tile_pool(name="sps", bufs=2, space="PSUM"))
        o_ps = ctx.enter_context(tc.tile_pool(name="ops", bufs=2, space="PSUM"))

        # ---- weight + x loads (fp8 hi/lo pairs) ----
        # order matters for the startup critical path: wv + x first (phase V
        # needs them, unit-interleaved so the first V matmuls start early),
        # wq/wk next, wp last. Two DGE queues (SP + ACT) in parallel.
        xh_sb, xl_sb = [], []
        wvc = w8_pool.tile([P, NU, 2, 2, CPC], F8, name="wvc", tag="wvc")
        wv_sb = [wvc[:, :, 0], wvc[:, :, 1]]
        xc_sb = []
        for i in range(NU):
            xc = x_pool.tile([P, 2, 2, T], F8, name=f"xc{i}", tag="x")
            xc_sb.append(xc)
            xh_sb.append(xc[:, 0])
            xl_sb.append(xc[:, 1])
        for i in range(NU):
            eng, eng2 = ((nc.sync, nc.scalar) if i % 2 == 0
                         else (nc.scalar, nc.sync))
            eng2.dma_start(out=wvc[:, i], in_=wv8c[i])
            sl = slice(0, T // 4)
            eng.dma_start(out=xc_sb[i][:, :, :, sl], in_=x8c[i][:, :, :, sl])
        mask_sb = const.tile([P, P], F16, tag="mask")
        nc.sync.dma_start(out=mask_sb, in_=maskT)
        for half in range(2):
            sl = slice(T // 4 + T * 3 // 8 * half,
                       T // 4 + T * 3 // 8 * (half + 1))
            for i in range(NU):
                eng = nc.sync if i % 2 == 0 else nc.scalar
                eng.dma_start(out=xc_sb[i][:, :, :, sl],
                              in_=x8c[i][:, :, :, sl])
        wq_sb, wk_sb = [], []
        for nm, drh, drl, lst in (("wq", wq8h, wq8l, wq_sb),
                                  ("wk", wk8h, wk8l, wk_sb)):
            for tag, dr_ in ((f"{nm}h", drh), (f"{nm}l", drl)):
                t_ = w8_pool.tile([P, NU, 2, CPC], F8, name=tag, tag=tag)
                eng = nc.sync if nm == "wq" else nc.scalar
                eng.dma_start(out=t_, in_=dr_.rearrange("i p j m -> p i j m"))
                lst.append(t_)
        qb_all = bias_pool.tile([P, NPAIR], F32, tag="qb", bufs=1)
        kb_all = bias_pool.tile([P, NPAIR], F32, tag="kb", bufs=1)
        nc.sync.dma_start(out=qb_all, in_=qb4.rearrange("(a p) -> p a", p=P))
        nc.sync.dma_start(out=kb_all, in_=kb4.rearrange("(a p) -> p a", p=P))

        def comp_mms(ps, lhs_hl, rhs_hl, lslice, rslice):
            """hi*hi + lo*hi + hi*lo DoubleRow accumulation over NU units."""
            terms = [(0, 0), (1, 0), (0, 1)]
            n = NU * len(terms) - 1
            cnt = 0
            for i in range(NU):
                for (a, b_) in terms:
                    nc.tensor.matmul(
                        ps, lslice(lhs_hl[a], i), rslice(rhs_hl[b_], i),
                        start=(cnt == 0), stop=(cnt == n), perf_mode=DR)
                    cnt += 1

        # ---- fp8 wp tiles for the output projection ----
        wp_sb = []
        for tag, dr_ in (("wph", wp8h), ("wpl", wp8l)):
            t_ = w8_pool.tile([P, 2, 2, C], F8, name=tag, tag=tag)
            for g in range(2):
                nc.sync.dma_start(out=t_[:, g], in_=dr_[g])
            wp_sb.append(t_)
        ot8h, ot8l, ot16 = [], [], []
        for g in range(2):
            ot8h.append(ot_pool.tile([P, 2, T], F8, name=f"oth{g}", tag="ot8"))
            ot8l.append(ot_pool.tile([P, 2, T], F8, name=f"otl{g}", tag="ot8"))
            ot16.append(ot_pool.tile([P, 2, T], F16, name=f"ot16{g}",
                                     tag="ot16", bufs=2))

        # ---- per head-pair: O^T via DMA-XBAR transpose + GpSimd fp8 split --
        def transpose_tq(p, o_sb, tq, fine=False):
            """Blocked transpose of one o_sb q-window into ot8 hi/lo fp8.
            fine=True splits the fp8 hi/lo pass per 128-block so downstream
            output-projection tiles unblock as each block lands (used for
            the strictly-serial final window)."""
            g, j = p // 2, p % 2
            win = slice(QW * tq, QW * (tq + 1))
            out3 = ot16[g][:, j, win].rearrange("p (b c) -> p b c", b=4)
            nc.sync.dma_start(out=out3, in_=o_sb[:, win], transpose=True)
            blocks = ([slice(QW * tq + P * b, QW * tq + P * (b + 1))
                       for b in range(4)] if fine else [win])
            for bw in blocks:
                nc.gpsimd.tensor_copy(out=ot8h[g][:, j, bw],
                                      in_=ot16[g][:, j, bw])
                nc.gpsimd.tensor_sub(ot8l[g][:, j, bw], ot16[g][:, j, bw],
                                     ot8h[g][:, j, bw])

        def emit_y(tq):
            """Output-projection matmuls for the 4 t-tiles of one tq window."""
            for t in range(4 * tq, 4 * tq + 4):
                ysb = y_pool.tile([P, C], F16, name=f"y{t}", tag="y")
                for n2 in range(2):
                    yps = mm_ps.tile([P, QW], F32, name=f"yps{t}{n2}", tag="mm")
                    cnt = 0
                    for g in range(2):
                        for (osrc, wsrc) in ((ot8h[g], wp_sb[0]),
                                             (ot8h[g], wp_sb[1]),
                                             (ot8l[g], wp_sb[0])):
                            nc.tensor.matmul(
                                yps, osrc[:, :, P * t:P * (t + 1)],
                                wsrc[:, g, :, QW * n2:QW * (n2 + 1)],
                                start=(cnt == 0), stop=(cnt == 5), perf_mode=DR)
                            cnt += 1
                    # last window runs after attention ends: ScalarE is idle
                    # there, so split its evacuations and pipeline the y DMA
                    # per half-tile to shorten the serial tail
                    if tq >= NQT - 2 and n2 == 1:
                        nc.scalar.mul(ysb[:, QW:C], yps, 1.0 / 1024.0)
                    else:
                        nc.vector.tensor_scalar(
                            out=ysb[:, QW * n2:QW * (n2 + 1)], in0=yps,
                            scalar1=1.0 / 1024.0, scalar2=None,
                            op0=mybir.AluOpType.mult)
                    if tq == NQT - 1:
                        nc.sync.dma_start(
                            out=y[P * t:P * (t + 1), QW * n2:QW * (n2 + 1)],
                            in_=ysb[:, QW * n2:QW * (n2 + 1)])
                if tq != NQT - 1:
                    nc.sync.dma_start(out=y[P * t:P * (t + 1), :], in_=ysb)

        def qk_proj(p, tq):
            """Q/K projection matmuls + fp8 PSUM evac for one t-window."""
            qka, _ = get_qtkt(p)
            for qk, (wsb, bsb) in enumerate(
                    ((wq_sb, qb_all[:, p:p + 1]),
                     (wk_sb, kb_all[:, p:p + 1]))):
                ps = mm_ps.tile([P, QW], F32, name=f"qk{p}{tq}", tag="mm")
                comp_mms(
                    ps, (wsb[0], wsb[1]), (xh_sb, xl_sb),
                    lambda w, i: w[:, i, :, P * p:P * (p + 1)],
                    lambda xs, i: xs[i][:, :, QW * tq:QW * (tq + 1)])
                nc.vector.tensor_scalar(
                    out=qka[:, qk, QW * tq:QW * (tq + 1)], in0=ps,
                    scalar1=1.0 / 16.0, scalar2=bsb,
                    op0=mybir.AluOpType.mult, op1=mybir.AluOpType.add)

        def qk_regroup(p, half):
            """[128, 2, T] channel-major fp8 -> [32(hl base), 2(j), 2(qk), T]
            DoubleRow layout: channel d = 32j + i at partition i, free j.
            One DMA per (hl, j) block moves both Q and K."""
            qka, qk8 = get_qtkt(p)
            win = slice(T // 2 * half, T // 2 * (half + 1))
            for hl in range(2):
                for j in range(2):
                    nc.sync.dma_start(
                        out=qk8[32 * hl:32 * (hl + 1), j, :, win],
                        in_=qka[64 * hl + 32 * j:64 * hl + 32 * (j + 1), :,
                                win])

        qtkt = {}

        def get_qtkt(p):
            if p not in qtkt:
                qtkt[p] = (
                    qtkt_pool.tile([P, 2, T], F8, name=f"qka{p}", tag="qka",
                                   bufs=2),
                    qtkt_pool.tile([64, 2, 2, T], F8, name=f"qk8{p}",
                                   tag="qk8", bufs=2),
                )
            return qtkt[p]

        # ---- phase V: V for all 8 heads, pair-0 Q/K interleaved per
        # quarter (its x window arrives with the same DMA quarter, so PE has
        # runnable work as soon as each quarter lands) ----
        v_sb = []
        for t in range(NT):
            vps = mm_ps.tile([P, CPC], F32, name=f"vps{t}", tag="mm")
            comp_mms(
                vps, (xh_sb, xl_sb), (wv_sb[0], wv_sb[1]),
                lambda xs, i: xs[i][:, :, P * t:P * (t + 1)],
                lambda w, i: w[:, i])
            vt = v_pool.tile([P, HPC, HD + 1], F16, name=f"v{t}", tag="v")
            nc.vector.memset(vt[:, :, HD], 4.0)
            nc.vector.tensor_copy(
                out=vt[:, :, 0:HD],
                in_=vps.rearrange("p (h d) -> p h d", h=HPC))
            v_sb.append(vt)

        for tq in range(NQT):
            qk_proj(0, tq)
        qk_regroup(0, 0)
        qk_regroup(0, 1)

        for p in range(NPAIR):
            _, qk8_sb = get_qtkt(p)
            o_sb = osb_pool.tile([P, T], F16, name=f"o{p}", tag="o")

            def s_unit(qt_i, hl):
                """S^T matmuls + exp + causal mask for one (q-tile, head)."""
                dlo, dhi = 32 * hl, 32 * (hl + 1)
                nkt = 4 * qt_i + 4
                pts = []
                for k0 in range(0, nkt, 2):
                    smin = [min(max(0, (k0 + u) - 4 * qt_i), 2)
                            for u in range(2)]
                    off = [P * s for s in smin]
                    sps = s_ps.tile([P, 2 * QW], F32,
                                    name=f"s{p}{hl}{qt_i}{k0}", tag="s")
                    for u in range(2):
                        k = k0 + u
                        nc.tensor.matmul(
                            sps[:, QW * u + off[u]:QW * (u + 1)],
                            qk8_sb[dlo:dhi, :, 1, P * k:P * (k + 1)],
                            qk8_sb[dlo:dhi, :, 0,
                                   QW * qt_i + off[u]:QW * (qt_i + 1)],
                            start=True, stop=True, perf_mode=DR)
                    pt = pt_pool.tile([P, 2 * QW], F16,
                                      name=f"pt{p}{hl}{qt_i}{k0}", tag="pt")
                    # exp in a single instruction per chunk: for unequal
                    # offsets, exp the union region (extra columns read stale
                    # psum; their pt slots are never consumed downstream).
                    # A fraction of full-width chunks runs on DVE via the
                    # Schraudolph bit-trick to unload the ScalarE bottleneck.
                    eoff = min(off)
                    schr = eoff == 0 and k0 % 8 == 2
                    if schr:
                        nc.vector.tensor_scalar(
                            out=pt.bitcast(I16), in0=sps,
                            scalar1=EXP_A, scalar2=EXP_B,
                            op0=mybir.AluOpType.mult,
                            op1=mybir.AluOpType.add)
                    elif eoff == 0:
                        nc.scalar.activation(
                            out=pt, in_=sps,
                            func=mybir.ActivationFunctionType.Exp,
                            scale=SCALE / 16.0)
                    else:
                        view = lambda ap: ap.rearrange(
                            "p (u c) -> p u c", u=2)[:, :, eoff:QW]
                        nc.scalar.activation(
                            out=view(pt), in_=view(sps),
                            func=mybir.ActivationFunctionType.Exp,
                            scale=SCALE / 16.0)
                    for u in range(2):
                        k = k0 + u
                        for s in range(4):
                            gs = 4 * qt_i + s
                            if gs == k:
                                sl = pt[:, QW * u + P * s:QW * u + P * (s + 1)]
                                nc.vector.tensor_mul(sl, sl, mask_sb)
                    pts.append(pt)
                return pts

            def pv_unit(qt_i, hl, pts):
                """P^T @ V_aug + normalize into o_sb for one unit."""
                hh = 2 * p + hl
                ops_ = o_ps.tile([P, 4 * (HD + 1)], F32,
                                 name=f"o{p}{hl}{qt_i}", tag="o")
                for s in range(4):
                    gs = 4 * qt_i + s
                    for k in range(gs + 1):
                        nc.tensor.matmul(
                            ops_[:, (HD + 1) * s:(HD + 1) * (s + 1)],
                            pts[k // 2][:, QW * (k % 2) + P * s:
                                        QW * (k % 2) + P * (s + 1)],
                            v_sb[k][:, hh, :],
                            start=(k == 0), stop=(k == gs))
                r_ = r_pool.tile([P, 4], F32, name=f"r{p}{hl}{qt_i}", tag="r")
                nc.vector.reciprocal(
                    r_, ops_.rearrange("p (s c) -> p s c", c=HD + 1)[:, :, HD])
                out_ap = o_sb[:, QW * qt_i:QW * (qt_i + 1)].rearrange(
                    "p (s h d) -> p s h d", s=4, h=2)[:, :, hl, :]
                nc.vector.tensor_mul(
                    out_ap,
                    ops_.rearrange("p (s c) -> p s c", c=HD + 1)[:, :, 0:HD],
                    r_.unsqueeze(2).broadcast_to((P, 4, HD)))

            # software pipeline: PV runs one unit behind S/exp. After each
            # q-window (hl == 1) completes: emit the NEXT pair's projection
            # matmuls for that window (PE filler for this ScalarE-bound
            # phase), then this window's transpose chain (DMA + GpSimd, no
            # PE). On the last pair the filler is the output projection,
            # lagged one window behind its transpose.
            last = p == NPAIR - 1
            y_ready = []

            def drain(pend):
                qt_i, hl, pts = pend
                pv_unit(qt_i, hl, pts)
                if hl == 0:
                    if not last:
                        qk_proj(p + 1, qt_i)
                        if qt_i == NQT - 1:
                            qk_regroup(p + 1, 1)
                else:
                    if not last and qt_i == 1:
                        qk_regroup(p + 1, 0)
                    transpose_tq(p, o_sb, qt_i,
                                 fine=last)
                    if last:
                        if y_ready:
                            emit_y(y_ready.pop(0))
                        y_ready.append(qt_i)

            units = [(qt_i, hl) for qt_i in range(NQT) for hl in range(2)]
            pq = []
            for (qt_i, hl) in units:
                pts = s_unit(qt_i, hl)
                pq.append((qt_i, hl, pts))
                if len(pq) > 1:
                    drain(pq.pop(0))
            for pend in pq:
                drain(pend)
            for tq in y_ready:
                emit_y(tq)


def _comp8(a):
    hi = a.astype(NPF8)
    lo = (a - hi.astype(np.float32)).astype(NPF8)
    return hi, lo


def _prep_inputs(x, w_attn, b_attn, w_proj):
    """Per-core input maps."""
    in_maps = []
    # contraction layout [unit, p, j, ...]: c = 256*unit + 128*j + p
    def units(a, n_u):
        # a: [n_u*256, M] -> [n_u, 128, 2, M]
        return np.ascontiguousarray(
            a.reshape(n_u, 2, P, -1).transpose(0, 2, 1, 3))

    wq_h = {}
    for g in range(2):
        qs = slice(CPC * g, CPC * (g + 1))
        ks = slice(C + CPC * g, C + CPC * (g + 1))
        vs = slice(2 * C + CPC * g, 2 * C + CPC * (g + 1))
        wqh, wql = _comp8(64.0 * w_attn[:, qs])
        wkh, wkl = _comp8(64.0 * w_attn[:, ks])
        wvh, wvl = _comp8(64.0 * w_attn[:, vs])
        wph, wpl = _comp8(64.0 * w_proj[CPC * g:CPC * (g + 1), :])
        wq_h[g] = dict(
            wq8h=units(wqh, NU), wq8l=units(wql, NU),
            wk8h=units(wkh, NU), wk8l=units(wkl, NU),
            wv8c=np.ascontiguousarray(
                np.stack([units(wvh, NU), units(wvl, NU)], axis=1)
                .transpose(0, 2, 1, 3, 4)),
            wp8h=units(wph, 2), wp8l=units(wpl, 2),
            qb4=np.ascontiguousarray(4.0 * b_attn[qs]),
            kb4=np.ascontiguousarray(4.0 * b_attn[ks]),
        )
    for c in range(NCORES):
        b = c // 2
        g = c % 2
        xT = np.ascontiguousarray(x[b].T)          # [C, T] fp32
        xh, xl = _comp8(xT)
        in_maps.append({
            "x8c": np.ascontiguousarray(
                np.stack([units(xh, NU), units(xl, NU)], axis=1)
                .transpose(0, 2, 1, 3, 4)),
            **wq_h[g],
            "maskT": np.triu(np.ones((P, P), dtype=np.float16)),
        })
    return in_maps


_CACHED_NC = None


def kernel(x, w_attn, b_attn, w_proj, b_proj):
    global _CACHED_NC
    x = np.asarray(x, dtype=np.float32)
    w_attn = np.asarray(w_attn, dtype=np.float32)
    b_attn = np.asarray(b_attn, dtype=np.float32)
    w_proj = np.asarray(w_proj, dtype=np.float32)
    b_proj = np.asarray(b_proj, dtype=np.float32)

    if _CACHED_NC is None:
        _CACHED_NC = build_kernel(loop_n=1)
    nc = _CACHED_NC
    in_maps = _prep_inputs(x, w_attn, b_attn, w_proj)
    res = run_bass_kernel_spmd(nc, in_maps, core_ids=list(range(NCORES)),
                               trace=False)
    out = np.empty((B, T, C), dtype=np.float32)
    # exact row-vector bias contribution: rows of softmax sum to 1
    for b in range(B):
        out[b] = (res.results[2 * b]["y"].astype(np.float32)
                  + res.results[2 * b + 1]["y"].astype(np.float32))
    bias_row = b_attn[2 * C:3 * C] @ w_proj + b_proj
    out += bias_row[None, None, :]
    return out
